# revision 1
# baseline (speedup 1.0000x reference)
"""Trainium2 Bass kernel for nn_CrossAttention (dense transformer block).

Sharding: data-parallel over batch — 8 batch elements, one per NeuronCore.
Each core runs the full block for its batch element:
  bias = Conv1x1(gelu(Conv1x1(log(attn_map[1:,1:] + eps))))
  MHA(q, kv) with bias added to scores; residual + LN; FFN; residual + LN.

Self-contained: hardcodes all shapes; host-side numpy prepares transposed /
packed weight layouts per core.
"""

import numpy as np
import ml_dtypes

import concourse.bass as bass
import concourse.mybir as mybir
import concourse.tile as tile
from concourse import bacc
from concourse.bass import ts
from concourse.bass_utils import run_bass_kernel_spmd
from concourse.masks import make_identity

AF = mybir.ActivationFunctionType
ALU = mybir.AluOpType

B, S, D, H, DH, FF = 8, 512, 1024, 16, 64, 4096
CH, CHID = 16, 32
EPS_LOG = 1e-6
EPS_LN = 1e-6
P = 128
NQT = S // P          # 4 q-tiles
ND = D // P           # 8 d-blocks
NFF = FF // P         # 32 ff-blocks
AM = 513              # attn_map edge

fp32 = mybir.dt.float32
fp32r = mybir.dt.float32r
bf16 = mybir.dt.bfloat16

_CACHED = {}


def _layernorm(nc, pool, out_ap, x_ap, gb, bb, eps_c):
    """out = (x - mean(x)) * rsqrt(var(x) + eps) * g + b over free dim (D)."""
    nsub = D // 512
    stats = pool.tile([P, nsub, nc.vector.BN_STATS_DIM], fp32, tag="ln_stats")
    for i in range(nsub):
        nc.vector.bn_stats(out=stats[:, i, :], in_=x_ap[:, ts(i, 512)])
    mv = pool.tile([P, nc.vector.BN_AGGR_DIM], fp32, tag="ln_mv")
    nc.vector.bn_aggr(out=mv, in_=stats)
    rstd = pool.tile([P, 1], fp32, tag="ln_rstd")
    nc.scalar.activation(rstd, mv[:, 1:2], AF.Sqrt, bias=eps_c, scale=1.0)
    nc.vector.reciprocal(out=rstd, in_=rstd)
    u = pool.tile([P, D], fp32, tag="ln_u")
    nc.vector.scalar_tensor_tensor(
        out=u, in0=x_ap, scalar=mv[:, 0:1], in1=gb,
        op0=ALU.subtract, op1=ALU.mult,
    )
    nc.vector.scalar_tensor_tensor(
        out=out_ap, in0=u, scalar=rstd[:, 0:1], in1=bb,
        op0=ALU.mult, op1=ALU.add,
    )


def build_program(debug=False):
    nc = bacc.Bacc(None)

    # ---------------- DRAM I/O ----------------
    qT_e = nc.dram_tensor("qT", [D, S], fp32r, kind="ExternalInput")
    kvT_e = nc.dram_tensor("kvT", [D, S], fp32r, kind="ExternalInput")
    qin_e = nc.dram_tensor("qin", [S, D], fp32, kind="ExternalInput")
    amap_e = nc.dram_tensor("amap", [CH, AM, AM], fp32, kind="ExternalInput")
    wqT_e = nc.dram_tensor("wqT", [D, D], fp32r, kind="ExternalInput")
    wkT_e = nc.dram_tensor("wkT", [D, D], fp32r, kind="ExternalInput")
    wvT_e = nc.dram_tensor("wvT", [D, D], fp32r, kind="ExternalInput")
    wmT_e = nc.dram_tensor("wmT", [D, D], fp32r, kind="ExternalInput")
    wf1T_e = nc.dram_tensor("wf1T", [D, FF], bf16, kind="ExternalInput")
    wf2T_e = nc.dram_tensor("wf2T", [FF, D], bf16, kind="ExternalInput")
    c1A_e = nc.dram_tensor("c1A", [P, P], bf16, kind="ExternalInput")
    c1B_e = nc.dram_tensor("c1B", [P, P], bf16, kind="ExternalInput")
    c2A_e = nc.dram_tensor("c2A", [P, P], bf16, kind="ExternalInput")
    c2B_e = nc.dram_tensor("c2B", [P, P], bf16, kind="ExternalInput")
    # per-partition bias columns: [128, nblk]
    bqc_e = nc.dram_tensor("bqc", [P, ND], fp32, kind="ExternalInput")   # bq/8
    bkc_e = nc.dram_tensor("bkc", [P, ND], fp32, kind="ExternalInput")
    bc1A_e = nc.dram_tensor("bc1A", [P, 1], fp32, kind="ExternalInput")
    bc1B_e = nc.dram_tensor("bc1B", [P, 1], fp32, kind="ExternalInput")
    bc2c_e = nc.dram_tensor("bc2c", [P, 1], fp32, kind="ExternalInput")
    bf1c_e = nc.dram_tensor("bf1c", [P, NFF], fp32, kind="ExternalInput")
    # bias rows (K=1 matmul trick)
    bvr_e = nc.dram_tensor("bvr", [1, D], fp32r, kind="ExternalInput")
    bmr_e = nc.dram_tensor("bmr", [1, D], fp32r, kind="ExternalInput")
    bf2r_e = nc.dram_tensor("bf2r", [1, D], bf16, kind="ExternalInput")
    onesr_e = nc.dram_tensor("onesr", [1, P], fp32r, kind="ExternalInput")
    onesb_e = nc.dram_tensor("onesb", [1, P], bf16, kind="ExternalInput")
    # LN params as rows
    g1r_e = nc.dram_tensor("g1r", [1, D], fp32, kind="ExternalInput")
    b1r_e = nc.dram_tensor("b1r", [1, D], fp32, kind="ExternalInput")
    g2r_e = nc.dram_tensor("g2r", [1, D], fp32, kind="ExternalInput")
    b2r_e = nc.dram_tensor("b2r", [1, D], fp32, kind="ExternalInput")

    out_e = nc.dram_tensor("out", [S, D], fp32, kind="ExternalOutput")
    if debug:
        dbg_qt_e = nc.dram_tensor("dbg_qt", [P, ND, S], fp32, kind="ExternalOutput")
        dbg_kt_e = nc.dram_tensor("dbg_kt", [P, ND, S], fp32, kind="ExternalOutput")
        dbg_v_e = nc.dram_tensor("dbg_v", [P, NQT, D], fp32, kind="ExternalOutput")
        dbg_bias_e = nc.dram_tensor("dbg_bias", [P, H, S], fp32, kind="ExternalOutput")
        dbg_c2_e = nc.dram_tensor("dbg_c2", [P, 4 * S], fp32, kind="ExternalOutput")
        dbg_scb_e = nc.dram_tensor("dbg_scb", [P, S], fp32, kind="ExternalOutput")
        dbg_attn_e = nc.dram_tensor("dbg_attn", [P, S], fp32, kind="ExternalOutput")
        dbg_ctx_e = nc.dram_tensor("dbg_ctx", [P, ND, S], fp32, kind="ExternalOutput")
        dbg_bst_e = nc.dram_tensor("dbg_bst", [P, H, S], fp32, kind="ExternalOutput")

    with tile.TileContext(nc) as tc:
        # ------------- persistent pools -------------
        const_cm = tc.tile_pool(name="const", bufs=1)
        const = const_cm.__enter__()
        dram_cm = tc.tile_pool(name="dstage", bufs=1, space="DRAM")
        dram = dram_cm.__enter__()
        bstage = dram.tile([S, H, S], bf16)
        bigE_cm = tc.tile_pool(name="bigE", bufs=1)   # Qt/Kt/V/ctxT (ph1-4)
        bigE = bigE_cm.__enter__()

        ident_b = const.tile([P, P], bf16)
        make_identity(nc, ident_b)
        ident_f = const.tile([P, P], fp32)
        make_identity(nc, ident_f)

        eps_log_c = const.tile([P, 1], fp32)
        nc.vector.memset(eps_log_c, EPS_LOG)
        eps_ln_c = const.tile([P, 1], fp32)
        nc.vector.memset(eps_ln_c, EPS_LN)

        c1A = const.tile([P, P], bf16)
        c1B = const.tile([P, P], bf16)
        c2A = const.tile([P, P], bf16)
        c2B = const.tile([P, P], bf16)
        nc.sync.dma_start(out=c1A, in_=c1A_e[:, :])
        nc.sync.dma_start(out=c1B, in_=c1B_e[:, :])
        nc.sync.dma_start(out=c2A, in_=c2A_e[:, :])
        nc.sync.dma_start(out=c2B, in_=c2B_e[:, :])
        bc1A = const.tile([P, 1], fp32)
        bc1B = const.tile([P, 1], fp32)
        bc2c = const.tile([P, 1], fp32)
        nc.sync.dma_start(out=bc1A, in_=bc1A_e[:, :])
        nc.sync.dma_start(out=bc1B, in_=bc1B_e[:, :])
        nc.sync.dma_start(out=bc2c, in_=bc2c_e[:, :])
        bqc = const.tile([P, ND], fp32)
        bkc = const.tile([P, ND], fp32)
        bf1c = const.tile([P, NFF], fp32)
        nc.sync.dma_start(out=bqc, in_=bqc_e[:, :])
        nc.sync.dma_start(out=bkc, in_=bkc_e[:, :])
        nc.sync.dma_start(out=bf1c, in_=bf1c_e[:, :])
        bvr = const.tile([1, D], fp32r)
        bmr = const.tile([1, D], fp32r)
        bf2r = const.tile([1, D], bf16)
        onesr = const.tile([1, P], fp32r)
        onesb = const.tile([1, P], bf16)
        nc.sync.dma_start(out=bvr, in_=bvr_e[:, :])
        nc.sync.dma_start(out=bmr, in_=bmr_e[:, :])
        nc.sync.dma_start(out=bf2r, in_=bf2r_e[:, :])
        nc.sync.dma_start(out=onesr, in_=onesr_e[:, :])
        nc.sync.dma_start(out=onesb, in_=onesb_e[:, :])

        # LN param broadcast tiles [128, D] + xln (whole-program residents)
        g1b = const.tile([P, D], fp32)
        b1b = const.tile([P, D], fp32)
        g2b = const.tile([P, D], fp32)
        b2b = const.tile([P, D], fp32)
        for dst, src_e in ((g1b, g1r_e), (b1b, b1r_e), (g2b, g2r_e), (b2b, b2r_e)):
            row = const.tile([1, D], fp32, tag="lnrow", name="lnrow")
            nc.sync.dma_start(out=row, in_=src_e[:, :])
            nc.gpsimd.partition_broadcast(dst, row[0:1, :])
        xln = const.tile([P, NQT, D], fp32)    # LN1 out [s-part, s-blk, d]

        # attention-phase residents (partition dim first!)
        QtT = bigE.tile([P, ND, S], fp32r)     # [o-part, o-blk, s]  (Wq x /8 + bq/8)
        KtT = bigE.tile([P, ND, S], fp32r)
        Vsb = bigE.tile([P, NQT, D], bf16)     # [k-part, k-blk, o]
        ctxT = bigE.tile([P, ND, S], fp32r)    # [(h,dh)-part, blk, q]

        # =========== Phase 1: projections ===========
        with (
            tc.tile_pool(name="p1x", bufs=1) as p1x,
            tc.tile_pool(name="p1w", bufs=2) as p1w,
            tc.tile_pool(name="p1ps", bufs=1, space="PSUM") as p1ps,
        ):
            qT = p1x.tile([P, ND, S], fp32r)
            nc.sync.dma_start(out=qT, in_=qT_e.rearrange("(n p) s -> p n s", p=P))
            kvT = p1x.tile([P, ND, S], fp32r)
            nc.sync.dma_start(out=kvT, in_=kvT_e.rearrange("(n p) s -> p n s", p=P))

            # Qt / Kt: psum[o-blk] [128, 512] += wT[d-blk][:, o-cols].T @ xT[d-blk]
            for wsrc, xsb, dst, bcol, scl in (
                (wqT_e, qT, QtT, bqc, 0.125),
                (wkT_e, kvT, KtT, bkc, 1.0),
            ):
                psums = [p1ps.tile([P, S], fp32, tag=f"pp{i}", name=f"pp{i}") for i in range(ND)]
                for dblk in range(ND):
                    wch = p1w.tile([P, D], fp32r, tag="wch")
                    nc.sync.dma_start(
                        out=wch, in_=wsrc[dblk * P : (dblk + 1) * P, :]
                    )
                    for ob in range(ND):
                        nc.tensor.matmul(
                            psums[ob],
                            wch[:, ts(ob, P)],
                            xsb[:, dblk, :],
                            start=(dblk == 0),
                            stop=(dblk == ND - 1),
                        )
                for ob in range(ND):
                    nc.scalar.activation(
                        dst[:, ob, :], psums[ob], AF.Identity,
                        bias=bcol[:, ob : ob + 1], scale=scl,
                    )

            # V: psum[(s-tile, o-half)] += kvT[d-blk][:, s-cols].T @ wvT[d-blk][:, o-half]
            vps = [
                [p1ps.tile([P, S], fp32, tag=f"pp{st * 2 + oh}", name=f"vp{st}{oh}") for oh in range(2)]
                for st in range(NQT)
            ]
            for st in range(NQT):
                for oh in range(2):
                    nc.tensor.matmul(
                        vps[st][oh], onesr, bvr[:, ts(oh, S)],
                        start=True, stop=False,
                    )
            for dblk in range(ND):
                wch = p1w.tile([P, D], fp32r, tag="wch")
                nc.sync.dma_start(out=wch, in_=wvT_e[dblk * P : (dblk + 1) * P, :])
                for st in range(NQT):
                    for oh in range(2):
                        nc.tensor.matmul(
                            vps[st][oh],
                            kvT[:, dblk, ts(st, P)],
                            wch[:, ts(oh, S)],
                            start=False,
                            stop=(dblk == ND - 1),
                        )
            for st in range(NQT):
                for oh in range(2):
                    nc.scalar.activation(
                        Vsb[:, st, ts(oh, S)], vps[st][oh], AF.Copy
                    )

        if debug:
            nc.sync.dma_start(out=dbg_qt_e[:, :, :], in_=QtT.bitcast(fp32))
            nc.sync.dma_start(out=dbg_kt_e[:, :, :], in_=KtT.bitcast(fp32))
            nc.gpsimd.dma_start(out=dbg_v_e[:, :, :], in_=Vsb)

        # =========== Phase 2+3: per-qtile conv bias + attention ===========
        with (
            tc.tile_pool(name="pbias", bufs=2) as pbias,
            tc.tile_pool(name="p2sb", bufs=2) as p2sb,
            tc.tile_pool(name="p2ps", bufs=1, space="PSUM") as p2ps,
            tc.tile_pool(name="p3sb", bufs=3) as p3sb,
            tc.tile_pool(name="p3ps", bufs=2, space="PSUM") as p3ps,
        ):
            NQI = 4
            CF = NQI * S  # conv tile free size (4 qi x 512 k)
            for qt_i in range(NQT):
                biasq = pbias.tile([P, H, S], bf16, tag="biasq")
                for half in range(4):
                    qbase = qt_i * P + half * (8 * NQI)
                    amt = p2sb.tile([P, NQI, S], fp32, tag="amt")
                    for g in range(8):
                        src = bass.AP(
                            tensor=amap_e,
                            offset=(1 + qbase + NQI * g) * AM + 1,
                            ap=[[AM * AM, CH], [AM, NQI], [1, S]],
                        )
                        nc.sync.dma_start(out=amt[CH * g : CH * (g + 1)], in_=src)
                    logm = p2sb.tile([P, CF], bf16, tag="logm", bufs=1)
                    nc.scalar.activation(
                        logm, amt.rearrange("p a b -> p (a b)"), AF.Ln,
                        bias=eps_log_c, scale=1.0,
                    )
                    c2sb = p2sb.tile([P, CF], bf16, tag="c2sb")
                    for chk in range(CF // S):
                        pA = p2ps.tile([P, S], fp32, tag="pA")
                        pB = p2ps.tile([P, S], fp32, tag="pB")
                        nc.tensor.matmul(
                            pA, c1A, logm[:, ts(chk, S)], start=True, stop=True
                        )
                        nc.tensor.matmul(
                            pB, c1B, logm[:, ts(chk, S)], start=True, stop=True
                        )
                        gA = p2sb.tile([P, S], bf16, tag="gA")
                        gB = p2sb.tile([P, S], bf16, tag="gB")
                        nc.scalar.activation(gA, pA, AF.Gelu, bias=bc1A, scale=1.0)
                        nc.scalar.activation(gB, pB, AF.Gelu, bias=bc1B, scale=1.0)
                        pC = p2ps.tile([P, S], fp32, tag="pC")
                        nc.tensor.matmul(pC, c2A, gA, start=True, stop=False)
                        nc.tensor.matmul(pC, c2B, gB, start=False, stop=True)
                        nc.scalar.activation(
                            c2sb[:, ts(chk, S)], pC, AF.Identity,
                            bias=bc2c, scale=1.0,
                        )
                    if debug and qt_i == 0 and half == 0:
                        nc.gpsimd.dma_start(out=dbg_c2_e[:, :], in_=c2sb)
                    # stage to DRAM in [q, h, k] order:
                    #   bstage[qbase+NQI*g+qi, h, k] = c2sb[16g+h, (qi, k)]
                    c2v = c2sb.rearrange("p (i k) -> p i k", k=S)
                    for g in range(8):
                        q0 = qbase + NQI * g
                        nc.sync.dma_start(
                            out=bstage[q0 : q0 + NQI].rearrange("i h k -> h i k"),
                            in_=c2v[CH * g : CH * (g + 1)],
                        )

                nc.sync.dma_start(
                    out=biasq.rearrange("p h k -> p (h k)"),
                    in_=bstage[qt_i * P : (qt_i + 1) * P].rearrange(
                        "q h k -> q (h k)"
                    ),
                )

                if debug and qt_i == 0:
                    nc.gpsimd.dma_start(out=dbg_bias_e[:, :, :], in_=biasq)

                # ---- attention for this qtile ----
                for h in range(H):
                    hb, ho = (h * DH) // P, (h * DH) % P
                    sc_ps = p3ps.tile([P, S], fp32, tag="sc")
                    nc.tensor.matmul(
                        sc_ps,
                        QtT[ho : ho + DH, hb, ts(qt_i, P)],
                        KtT[ho : ho + DH, hb, :],
                        start=True, stop=True,
                    )
                    scb = p3sb.tile([P, S], fp32, tag="scb")
                    nc.vector.tensor_tensor(
                        out=scb, in0=sc_ps, in1=biasq[:, h, :], op=ALU.add
                    )
                    att = p3sb.tile([P, S], bf16, tag="att")
                    den = p3sb.tile([P, 1], fp32, tag="den")
                    nc.scalar.activation(att, scb, AF.Exp, accum_out=den)
                    rec = p3sb.tile([P, 1], fp32, tag="rec")
                    nc.vector.reciprocal(out=rec, in_=den)
                    attn = p3sb.tile([P, S], bf16, tag="attn")
                    nc.vector.tensor_scalar_mul(attn, att, rec[:, 0:1])
                    if debug and qt_i == 0 and h == 0:
                        nc.sync.dma_start(out=dbg_scb_e[:, :], in_=scb)
                        nc.gpsimd.dma_start(out=dbg_attn_e[:, :], in_=attn)
                    atT_ps = p3ps.tile([P, S], bf16, tag="atT", bufs=1)
                    for kt in range(NQT):
                        nc.tensor.transpose(
                            atT_ps[:, ts(kt, P)], attn[:, ts(kt, P)], ident_b
                        )
                    atT = p3sb.tile([P, S], bf16, tag="atTs")
                    nc.vector.tensor_copy(atT, atT_ps)
                    cx_ps = p3ps.tile([DH, P], fp32, tag="cx")
                    for kt in range(NQT):
                        nc.tensor.matmul(
                            cx_ps,
                            Vsb[:, kt, h * DH : (h + 1) * DH],
                            atT[:, ts(kt, P)],
                            start=(kt == 0), stop=(kt == NQT - 1),
                        )
                    nc.scalar.activation(
                        ctxT[ho : ho + DH, hb, ts(qt_i, P)], cx_ps, AF.Copy
                    )


        if debug:
            nc.sync.dma_start(out=dbg_ctx_e[:, :, :], in_=ctxT.bitcast(fp32))
            nc.gpsimd.dma_start(out=dbg_bst_e[:, :, :], in_=bstage[0:P])

        # =========== Phase 4: merge + residual + LN1 ===========
        with (
            tc.tile_pool(name="p4sb", bufs=2) as p4sb,
            tc.tile_pool(name="p4ps", bufs=1, space="PSUM") as p4ps,
        ):
            mps = [
                [p4ps.tile([P, S], fp32, tag=f"mp{st * 2 + oh}", name=f"mp{st}{oh}") for oh in range(2)]
                for st in range(NQT)
            ]
            for st in range(NQT):
                for oh in range(2):
                    nc.tensor.matmul(
                        mps[st][oh], onesr, bmr[:, ts(oh, S)], start=True, stop=False
                    )
            for dblk in range(ND):
                wch = p4sb.tile([P, D], fp32r, tag="wch")
                nc.sync.dma_start(out=wch, in_=wmT_e[dblk * P : (dblk + 1) * P, :])
                for st in range(NQT):
                    for oh in range(2):
                        nc.tensor.matmul(
                            mps[st][oh],
                            ctxT[:, dblk, ts(st, P)],
                            wch[:, ts(oh, S)],
                            start=False,
                            stop=(dblk == ND - 1),
                        )
            for st in range(NQT):
                qtile = p4sb.tile([P, D], fp32, tag="qtile")
                nc.sync.dma_start(out=qtile, in_=qin_e[st * P : (st + 1) * P, :])
                x1 = p4sb.tile([P, D], fp32, tag="x1")
                for oh in range(2):
                    nc.vector.tensor_tensor(
                        out=x1[:, ts(oh, S)], in0=mps[st][oh],
                        in1=qtile[:, ts(oh, S)], op=ALU.add,
                    )
                _layernorm(nc, p4sb, xln[:, st, :], x1, g1b, b1b, eps_ln_c)

        # free Qt/Kt/V/ctxT space before FFN phases
        bigE_cm.__exit__(None, None, None)
        bigL_cm = tc.tile_pool(name="bigL", bufs=1)
        bigL = bigL_cm.__enter__()
        xlnT = bigL.tile([P, ND, S], bf16)
        y1T = bigL.tile([P, NFF, S], bf16)

        # =========== Phase 5: transpose x_ln ===========
        with tc.tile_pool(name="p5ps", bufs=2, space="PSUM") as p5ps:
            for dblk in range(ND):
                tp = p5ps.tile([P, S], fp32, tag="tp")
                for st in range(NQT):
                    nc.tensor.transpose(
                        tp[:, ts(st, P)], xln[:, st, ts(dblk, P)], ident_f
                    )
                nc.scalar.activation(xlnT[:, dblk, :], tp, AF.Copy)

        # =========== Phase 6: FFN1 + relu ===========
        with (
            tc.tile_pool(name="p6w", bufs=1) as p6w,
            tc.tile_pool(name="p6ps", bufs=2, space="PSUM") as p6ps,
        ):
            wf1 = p6w.tile([P, ND, FF], bf16)
            nc.sync.dma_start(out=wf1, in_=wf1T_e.rearrange("(n p) f -> p n f", p=P))
            for ffb in range(NFF):
                fps = p6ps.tile([P, S], fp32, tag="fps")
                for dblk in range(ND):
                    nc.tensor.matmul(
                        fps,
                        wf1[:, dblk, ts(ffb, P)],
                        xlnT[:, dblk, :],
                        start=(dblk == 0), stop=(dblk == ND - 1),
                    )
                nc.scalar.activation(
                    y1T[:, ffb, :], fps, AF.Relu,
                    bias=bf1c[:, ffb : ffb + 1], scale=1.0,
                )

        # =========== Phase 7: FFN2 + residual + LN2 + out ===========
        with (
            tc.tile_pool(name="p7sb", bufs=2) as p7sb,
            tc.tile_pool(name="p7ps", bufs=1, space="PSUM") as p7ps,
        ):
            fps2 = [
                [p7ps.tile([P, S], fp32, tag=f"f2{st * 2 + oh}", name=f"f2{st}{oh}") for oh in range(2)]
                for st in range(NQT)
            ]
            for st in range(NQT):
                for oh in range(2):
                    nc.tensor.matmul(
                        fps2[st][oh], onesb, bf2r[:, ts(oh, S)],
                        start=True, stop=False,
                    )
            for ffb in range(NFF):
                wch = p7sb.tile([P, D], bf16, tag="wch")
                nc.sync.dma_start(out=wch, in_=wf2T_e[ffb * P : (ffb + 1) * P, :])
                for st in range(NQT):
                    for oh in range(2):
                        nc.tensor.matmul(
                            fps2[st][oh],
                            y1T[:, ffb, ts(st, P)],
                            wch[:, ts(oh, S)],
                            start=False,
                            stop=(ffb == NFF - 1),
                        )
            for st in range(NQT):
                x2 = p7sb.tile([P, D], fp32, tag="x2")
                for oh in range(2):
                    nc.vector.tensor_tensor(
                        out=x2[:, ts(oh, S)], in0=fps2[st][oh],
                        in1=xln[:, st, ts(oh, S)], op=ALU.add,
                    )
                xout = p7sb.tile([P, D], fp32, tag="xout")
                _layernorm(nc, p7sb, xout, x2, g2b, b2b, eps_ln_c)
                nc.sync.dma_start(out=out_e[st * P : (st + 1) * P, :], in_=xout)

        bigL_cm.__exit__(None, None, None)
        dram_cm.__exit__(None, None, None)
        const_cm.__exit__(None, None, None)

    nc.finalize()
    return nc


def _prep_inputs(q, kv, attn_map, Wq, bq, Wk, bk, Wv, bv, Wm, bm,
                 Wc1, bc1, Wc2, bc2, Wf1, bf1, Wf2, bf2, g1, b1, g2, b2):
    """Host-side packing. Returns (shared dict, per-core list of dicts)."""
    f32 = np.float32
    bf = ml_dtypes.bfloat16

    def c(a):
        return np.ascontiguousarray(np.asarray(a), dtype=f32)

    Wq, Wk, Wv, Wm = c(Wq), c(Wk), c(Wv), c(Wm)
    Wc1, Wc2, Wf1, Wf2 = c(Wc1), c(Wc2), c(Wf1), c(Wf2)
    bq, bk, bv, bm = c(bq), c(bk), c(bv), c(bm)
    bc1, bc2, bf1, bf2 = c(bc1), c(bc2), c(bf1), c(bf2)
    g1, b1, g2, b2 = c(g1), c(b1), c(g2), c(b2)

    shared = {
        "wqT": c(Wq.T), "wkT": c(Wk.T), "wvT": c(Wv.T), "wmT": c(Wm.T),
        "wf1T": np.ascontiguousarray(Wf1.T).astype(bf),
        "wf2T": np.ascontiguousarray(Wf2.T).astype(bf),
        "bqc": c((bq / 8.0).reshape(ND, P).T),
        "bkc": c(bk.reshape(ND, P).T),
        "bf1c": c(bf1.reshape(NFF, P).T),
        "bvr": bv.reshape(1, D), "bmr": bm.reshape(1, D),
        "bf2r": bf2.reshape(1, D).astype(bf),
        "onesr": np.ones((1, P), f32),
        "onesb": np.ones((1, P), bf),
        "g1r": g1.reshape(1, D), "b1r": b1.reshape(1, D),
        "g2r": g2.reshape(1, D), "b2r": b2.reshape(1, D),
    }
    # conv block-diag lhsT [K, M]: out[(g,oh)] = sum_c lhsT[(g,c),(g,oh)] rhs[(g,c)]
    c1A = np.zeros((P, P), f32)
    c1B = np.zeros((P, P), f32)
    c2A = np.zeros((P, P), f32)
    c2B = np.zeros((P, P), f32)
    for g in range(8):
        sl = slice(g * 16, g * 16 + 16)
        c1A[sl, sl] = Wc1[0:16, :].T     # [c, oh]
        c1B[sl, sl] = Wc1[16:32, :].T
        c2A[sl, sl] = Wc2[:, 0:16].T     # [ci, h]
        c2B[sl, sl] = Wc2[:, 16:32].T
    shared["c1A"] = c1A.astype(bf)
    shared["c1B"] = c1B.astype(bf)
    shared["c2A"] = c2A.astype(bf)
    shared["c2B"] = c2B.astype(bf)
    shared["bc1A"] = np.tile(bc1[0:16], 8).reshape(P, 1).astype(f32)
    shared["bc1B"] = np.tile(bc1[16:32], 8).reshape(P, 1).astype(f32)
    shared["bc2c"] = np.tile(bc2, 8).reshape(P, 1).astype(f32)

    q = c(q)
    kv = c(kv)
    attn_map = np.asarray(attn_map)
    per_core = []
    for b in range(B):
        per_core.append({
            "qT": c(q[b].T), "kvT": c(kv[b].T), "qin": q[b],
            "amap": c(attn_map[b]),
        })
    return shared, per_core


def kernel(**inputs):
    if "nc" not in _CACHED:
        _CACHED["nc"] = build_program()
    nc = _CACHED["nc"]
    shared, per_core = _prep_inputs(**inputs)
    in_maps = [dict(shared, **pc) for pc in per_core]
    res = run_bass_kernel_spmd(nc, in_maps, list(range(B)))
    out = np.stack([res.results[i]["out"] for i in range(B)], axis=0)
    return out.astype(np.float32)



# revision 12
# speedup vs baseline: 1.0199x; 1.0199x over previous
"""Trainium2 Bass kernel for nn_CrossAttention (dense transformer block).

Sharding: data-parallel over batch - 8 batch elements, one per NeuronCore.
Each core runs the full block for its batch element:
  bias = Conv1x1(gelu(Conv1x1(log(attn_map[1:,1:] + eps))))
  MHA(q, kv) with bias added to scores; residual + LN; FFN; residual + LN.

Perf structure (v2):
  - all matmuls bf16 (moving+stationary) with fp32 PSUM accumulation
  - strict phase order so the scalar engine loads each activation table once
    (ln -> gelu -> exp -> rsqrt)
  - conv bias kept SBUF-resident in q-major layout via SBUF->SBUF DMA shuffle
  - score bias added by PSUM-init matmul (identity x bias) instead of vector add
  - software-pipelined attention (head h+1 scores issued before head h ctx)
  - bulk DMA (attn_map loads + bias shuffle) on the idle gpsimd queue
"""

import numpy as np
import ml_dtypes

import concourse.bass as bass
import concourse.mybir as mybir
import concourse.tile as tile
from concourse import bacc
from concourse.bass import ts
from concourse.bass_utils import run_bass_kernel_spmd
from concourse.masks import make_identity

AF = mybir.ActivationFunctionType
ALU = mybir.AluOpType

B, S, D, H, DH, FF = 8, 512, 1024, 16, 64, 4096
CH, CHID = 16, 32
EPS_LOG = 1e-6
EPS_LN = 1e-6
P = 128
NQT = S // P          # 4 q-tiles
ND = D // P           # 8 d-blocks
NFF = FF // P         # 32 ff-blocks
AM = 513              # attn_map edge
NQI = 4               # q rows per partition-group in conv
NHALF = S // 32       # 16 conv halves (32 q rows each)

fp32 = mybir.dt.float32
bf16 = mybir.dt.bfloat16

_CACHED = {}


def _layernorm(nc, pool, out_ap, x_ap, gb, bb, eps_c):
    """out = (x - mean(x)) * rsqrt(var(x) + eps) * g + b over free dim (D)."""
    nsub = D // 512
    stats = pool.tile([P, nsub, nc.vector.BN_STATS_DIM], fp32, tag="ln_stats")
    for i in range(nsub):
        nc.vector.bn_stats(out=stats[:, i, :], in_=x_ap[:, ts(i, 512)])
    mv = pool.tile([P, nc.vector.BN_AGGR_DIM], fp32, tag="ln_mv")
    nc.vector.bn_aggr(out=mv, in_=stats)
    rstd = pool.tile([P, 1], fp32, tag="ln_rstd")
    nc.scalar.activation(rstd, mv[:, 1:2], AF.Sqrt, bias=eps_c, scale=1.0)
    nc.vector.reciprocal(out=rstd, in_=rstd)
    u = pool.tile([P, D], fp32, tag="ln_u")
    nc.vector.scalar_tensor_tensor(
        out=u, in0=x_ap, scalar=mv[:, 0:1], in1=gb,
        op0=ALU.subtract, op1=ALU.mult,
    )
    nc.vector.scalar_tensor_tensor(
        out=out_ap, in0=u, scalar=rstd[:, 0:1], in1=bb,
        op0=ALU.mult, op1=ALU.add,
    )


def build_program(debug=False):
    nc = bacc.Bacc(None)

    # ---------------- DRAM I/O ----------------
    qTb_e = nc.dram_tensor("qTb", [D, S], bf16, kind="ExternalInput")
    kvTb_e = nc.dram_tensor("kvTb", [D, S], bf16, kind="ExternalInput")
    qbm_e = nc.dram_tensor("qbm", [S, D], fp32, kind="ExternalInput")  # q + bm
    amapb_e = nc.dram_tensor("amapb", [CH, AM, AM], bf16, kind="ExternalInput")
    wqTb_e = nc.dram_tensor("wqTb", [D, D], bf16, kind="ExternalInput")
    wkTb_e = nc.dram_tensor("wkTb", [D, D], bf16, kind="ExternalInput")
    wvTb_e = nc.dram_tensor("wvTb", [D, D], bf16, kind="ExternalInput")
    wmTb_e = nc.dram_tensor("wmTb", [D, D], bf16, kind="ExternalInput")
    wf1Tb_e = nc.dram_tensor("wf1Tb", [D, FF], bf16, kind="ExternalInput")
    wf2Tb_e = nc.dram_tensor("wf2Tb", [FF, D], bf16, kind="ExternalInput")
    c1A_e = nc.dram_tensor("c1A", [P, P], bf16, kind="ExternalInput")
    c1B_e = nc.dram_tensor("c1B", [P, P], bf16, kind="ExternalInput")
    c2A_e = nc.dram_tensor("c2A", [P, P], bf16, kind="ExternalInput")
    c2B_e = nc.dram_tensor("c2B", [P, P], bf16, kind="ExternalInput")
    # per-partition bias columns
    bqc_e = nc.dram_tensor("bqc", [P, ND], fp32, kind="ExternalInput")   # bq/8
    bkc_e = nc.dram_tensor("bkc", [P, ND], fp32, kind="ExternalInput")
    bc1Ar_e = nc.dram_tensor("bc1Ar", [1, P], bf16, kind="ExternalInput")
    bc1Br_e = nc.dram_tensor("bc1Br", [1, P], bf16, kind="ExternalInput")
    bc2c_e = nc.dram_tensor("bc2c", [P, 1], fp32, kind="ExternalInput")
    bf1c_e = nc.dram_tensor("bf1c", [P, NFF], fp32, kind="ExternalInput")
    # bias rows (K=1 matmul trick)
    bvr_e = nc.dram_tensor("bvr", [1, D], bf16, kind="ExternalInput")
    bf2r_e = nc.dram_tensor("bf2r", [1, D], bf16, kind="ExternalInput")
    onesb_e = nc.dram_tensor("onesb", [1, S], bf16, kind="ExternalInput")
    # LN params as rows
    g1r_e = nc.dram_tensor("g1r", [1, D], fp32, kind="ExternalInput")
    b1r_e = nc.dram_tensor("b1r", [1, D], fp32, kind="ExternalInput")
    g2r_e = nc.dram_tensor("g2r", [1, D], fp32, kind="ExternalInput")
    b2r_e = nc.dram_tensor("b2r", [1, D], fp32, kind="ExternalInput")

    out_e = nc.dram_tensor("out", [S, D], fp32, kind="ExternalOutput")
    if debug:
        dbg_qt_e = nc.dram_tensor("dbg_qt", [P, ND, S], fp32, kind="ExternalOutput")
        dbg_kt_e = nc.dram_tensor("dbg_kt", [P, ND, S], fp32, kind="ExternalOutput")
        dbg_v_e = nc.dram_tensor("dbg_v", [P, NQT, D], fp32, kind="ExternalOutput")
        dbg_bias_e = nc.dram_tensor("dbg_bias", [P, H, S], fp32, kind="ExternalOutput")
        dbg_ctx_e = nc.dram_tensor("dbg_ctx", [P, ND, S], fp32, kind="ExternalOutput")
        dbg_xln_e = nc.dram_tensor("dbg_xln", [P, NQT, D], fp32, kind="ExternalOutput")

    with tile.TileContext(nc) as tc:
        # ------------- persistent pools -------------
        const_cm = tc.tile_pool(name="const", bufs=1)
        const = const_cm.__enter__()

        ident_b = const.tile([P, P], bf16)
        make_identity(nc, ident_b)

        eps_log_c = const.tile([P, 1], fp32)
        nc.vector.memset(eps_log_c, EPS_LOG)
        eps_ln_c = const.tile([P, 1], fp32)
        nc.vector.memset(eps_ln_c, EPS_LN)

        c1A = const.tile([P, P], bf16)
        c1B = const.tile([P, P], bf16)
        c2A = const.tile([P, P], bf16)
        c2B = const.tile([P, P], bf16)
        nc.sync.dma_start(out=c1A, in_=c1A_e[:, :])
        nc.sync.dma_start(out=c1B, in_=c1B_e[:, :])
        nc.sync.dma_start(out=c2A, in_=c2A_e[:, :])
        nc.sync.dma_start(out=c2B, in_=c2B_e[:, :])
        bc1Ar = const.tile([1, P], bf16)
        bc1Br = const.tile([1, P], bf16)
        bc2c = const.tile([P, 1], fp32)
        nc.sync.dma_start(out=bc1Ar, in_=bc1Ar_e[:, :])
        nc.sync.dma_start(out=bc1Br, in_=bc1Br_e[:, :])
        nc.sync.dma_start(out=bc2c, in_=bc2c_e[:, :])
        bqc = const.tile([P, ND], fp32)
        bkc = const.tile([P, ND], fp32)
        bf1c = const.tile([P, NFF], fp32)
        nc.sync.dma_start(out=bqc, in_=bqc_e[:, :])
        nc.sync.dma_start(out=bkc, in_=bkc_e[:, :])
        nc.sync.dma_start(out=bf1c, in_=bf1c_e[:, :])
        bvr = const.tile([1, D], bf16)
        bf2r = const.tile([1, D], bf16)
        onesb = const.tile([1, S], bf16)
        nc.sync.dma_start(out=bvr, in_=bvr_e[:, :])
        nc.sync.dma_start(out=bf2r, in_=bf2r_e[:, :])
        nc.sync.dma_start(out=onesb, in_=onesb_e[:, :])

        # ctxT outlives attp (merge reads it); entered first for stack order
        midp_cm = tc.tile_pool(name="midp", bufs=1)
        midp = midp_cm.__enter__()
        ctxT = midp.tile([P, ND, S], bf16)     # [(h,dh)-part, blk, q]

        # ========== attention-lifetime pool ==========
        attp_cm = tc.tile_pool(name="attp", bufs=1)
        attp = attp_cm.__enter__()
        QtT = attp.tile([P, ND, S], bf16)      # [o-part, o-blk, s]  ((Wq x + bq)/8)
        KtT = attp.tile([P, ND, S], bf16)
        Vsb = attp.tile([P, NQT, D], bf16)     # [k-part, k-blk, (h dh)]
        biasq = attp.tile([P, NQT, H, S], bf16)  # [q-part, qt, h, k]

        # =========== Phase 1: projections ===========
        with (
            tc.tile_pool(name="p1x", bufs=1) as p1x,
            tc.tile_pool(name="p1w", bufs=2) as p1w,
            tc.tile_pool(name="p1ps", bufs=2, space="PSUM") as p1ps,
        ):
            qTb = p1x.tile([P, ND, S], bf16)
            nc.sync.dma_start(out=qTb, in_=qTb_e.rearrange("(n p) s -> p n s", p=P))
            kvTb = p1x.tile([P, ND, S], bf16)
            nc.sync.dma_start(out=kvTb, in_=kvTb_e.rearrange("(n p) s -> p n s", p=P))

            # Q/K: out[o, s] += w[d-blk, o].T @ xT[d-blk, s]
            for wsrc, xsb, dst, bcol, scl in (
                (wqTb_e, qTb, QtT, bqc, 0.125),
                (wkTb_e, kvTb, KtT, bkc, 1.0),
            ):
                wres = p1w.tile([P, ND, D], bf16, tag="wres")
                nc.sync.dma_start(
                    out=wres, in_=wsrc.rearrange("(n p) d -> p n d", p=P)
                )
                for ob in range(ND):
                    ps = p1ps.tile([P, S], fp32, tag="pjps")
                    for dblk in range(ND):
                        nc.tensor.matmul(
                            ps,
                            wres[:, dblk, ts(ob, P)],
                            xsb[:, dblk, :],
                            start=(dblk == 0),
                            stop=(dblk == ND - 1),
                        )
                    nc.vector.tensor_scalar(
                        out=dst[:, ob, :], in0=ps,
                        scalar1=scl, scalar2=bcol[:, ob : ob + 1],
                        op0=ALU.mult, op1=ALU.add,
                    )

            # V: out[k, o] += kvT[d-blk, k-tile].T @ wv[d-blk, o-half]
            wvres = p1w.tile([P, ND, D], bf16, tag="wres")
            nc.sync.dma_start(
                out=wvres, in_=wvTb_e.rearrange("(n p) d -> p n d", p=P)
            )
            for kt in range(NQT):
                for oh in range(2):
                    ps = p1ps.tile([P, S], fp32, tag="pjps")
                    nc.tensor.matmul(
                        ps, onesb[:, 0:P], bvr[:, ts(oh, S)],
                        start=True, stop=False,
                    )
                    for dblk in range(ND):
                        nc.tensor.matmul(
                            ps,
                            kvTb[:, dblk, ts(kt, P)],
                            wvres[:, dblk, ts(oh, S)],
                            start=False,
                            stop=(dblk == ND - 1),
                        )
                    nc.vector.tensor_copy(Vsb[:, kt, ts(oh, S)], ps)

            # =========== Phase 2: conv bias for all halves ===========
            # (shares pool scope so conv can overlap projection tail)
            with (
                tc.tile_pool(name="p2sb", bufs=2) as p2sb,
                tc.tile_pool(name="p2ps", bufs=1, space="PSUM") as p2ps,
            ):
                # stage 1: log of all halves (one table: Ln)
                logms = []
                for half in range(NHALF):
                    qbase = half * 32
                    amt = p2sb.tile([P, NQI, S], bf16, tag="amt")
                    for g in range(8):
                        src = bass.AP(
                            tensor=amapb_e,
                            offset=(1 + qbase + NQI * g) * AM + 1,
                            ap=[[AM * AM, CH], [AM, NQI], [1, S]],
                        )
                        nc.gpsimd.dma_start(out=amt[CH * g : CH * (g + 1)], in_=src)
                    logm = p2sb.tile([P, NQI * S], bf16, tag="logm", bufs=6)
                    nc.scalar.activation(
                        logm, amt.rearrange("p a b -> p (a b)"), AF.Ln,
                        bias=eps_log_c, scale=1.0,
                    )
                    logms.append(logm)

                # stage 2: conv chain (one table: Gelu)
                for half in range(NHALF):
                    qt = half // 4
                    qoff = (half % 4) * 32
                    logm = logms[half]
                    c2sb = p2sb.tile([P, NQI, S], bf16, tag="c2sb")
                    for j in range(2):  # two [P, 1024] chunks
                        pA = p2ps.tile([P, 2 * S], fp32, tag="pA")
                        pB = p2ps.tile([P, 2 * S], fp32, tag="pB")
                        for c in range(2):
                            chk = 2 * j + c
                            nc.tensor.matmul(
                                pA[:, ts(c, S)], bc1Ar, onesb[:, 0:S],
                                start=True, stop=False,
                            )
                            nc.tensor.matmul(
                                pA[:, ts(c, S)], c1A, logm[:, ts(chk, S)],
                                start=False, stop=True,
                            )
                            nc.tensor.matmul(
                                pB[:, ts(c, S)], bc1Br, onesb[:, 0:S],
                                start=True, stop=False,
                            )
                            nc.tensor.matmul(
                                pB[:, ts(c, S)], c1B, logm[:, ts(chk, S)],
                                start=False, stop=True,
                            )
                        gA = p2sb.tile([P, 2 * S], bf16, tag="gA")
                        gB = p2sb.tile([P, 2 * S], bf16, tag="gB")
                        nc.scalar.activation(gA, pA, AF.Gelu)
                        nc.scalar.activation(gB, pB, AF.Gelu)
                        pC = p2ps.tile([P, 2 * S], fp32, tag="pC")
                        for c in range(2):
                            nc.tensor.matmul(
                                pC[:, ts(c, S)], c2A, gA[:, ts(c, S)],
                                start=True, stop=False,
                            )
                            nc.tensor.matmul(
                                pC[:, ts(c, S)], c2B, gB[:, ts(c, S)],
                                start=False, stop=True,
                            )
                        nc.vector.tensor_scalar(
                            out=c2sb[:, 2 * j : 2 * j + 2, :].rearrange(
                                "p a b -> p (a b)"
                            ),
                            in0=pC, scalar1=bc2c[:, 0:1], scalar2=None,
                            op0=ALU.add,
                        )
                    # shuffle into q-major resident bias (SBUF->SBUF DMA)
                    for g in range(8):
                        for qi in range(NQI):
                            qp = qoff + NQI * g + qi
                            nc.gpsimd.dma_start(
                                out=biasq[qp : qp + 1, qt],
                                in_=c2sb[CH * g : CH * (g + 1), qi, :],
                            )

        if debug:
            dbgq = const.tile([P, ND, S], fp32, name="dbgq")
            nc.vector.tensor_copy(dbgq.rearrange("p a b -> p (a b)"),
                                  QtT.rearrange("p a b -> p (a b)"))
            nc.sync.dma_start(out=dbg_qt_e[:, :, :], in_=dbgq)
            nc.vector.tensor_copy(dbgq.rearrange("p a b -> p (a b)"),
                                  KtT.rearrange("p a b -> p (a b)"))
            nc.sync.dma_start(out=dbg_kt_e[:, :, :], in_=dbgq)
            dbgv = const.tile([P, NQT, D], fp32, name="dbgv")
            nc.vector.tensor_copy(dbgv.rearrange("p a b -> p (a b)"),
                                  Vsb.rearrange("p a b -> p (a b)"))
            nc.sync.dma_start(out=dbg_v_e[:, :, :], in_=dbgv)
            dbgb = const.tile([P, H, S], fp32, name="dbgb")
            nc.vector.tensor_copy(dbgb.rearrange("p a b -> p (a b)"),
                                  biasq[:, 0].rearrange("p a b -> p (a b)"))
            nc.sync.dma_start(out=dbg_bias_e[:, :, :], in_=dbgb)

        # =========== Phase 3: attention (one table: Exp) ===========
        with (
            tc.tile_pool(name="p3sb", bufs=1) as p3sb,
            tc.tile_pool(name="p3ps", bufs=1, space="PSUM") as p3ps,
        ):
            sc_pool = [p3ps.tile([P, S], fp32, tag=f"sc{i}", name=f"sc{i}")
                       for i in range(4)]
            atu_pool = [p3ps.tile([P, NQT, P], bf16, tag=f"atu{i}", name=f"atu{i}")
                        for i in range(2)]
            cx_pool = [p3ps.tile([P, S], fp32, tag=f"cx{i}", name=f"cx{i}")
                       for i in range(2)]
            att_pool = [p3sb.tile([P, S], bf16, tag=f"att{i}", name=f"att{i}")
                        for i in range(4)]
            attn_pool = [p3sb.tile([P, S], bf16, tag=f"attn{i}", name=f"attn{i}")
                         for i in range(4)]
            den_pool = [p3sb.tile([P, 1], fp32, tag=f"den{i}", name=f"den{i}")
                        for i in range(4)]
            rec_pool = [p3sb.tile([P, 1], fp32, tag=f"rec{i}", name=f"rec{i}")
                        for i in range(4)]
            ath_pool = [p3sb.tile([P, NQT, S], bf16, tag=f"ath{i}", name=f"ath{i}")
                        for i in range(2)]

            def issue_scores(h):
                hb, ho = (h * DH) // P, (h * DH) % P
                for qt in range(NQT):
                    slot = (h * NQT + qt) % 4
                    sc = sc_pool[slot]
                    # psum <- bias, then += Qt^T K (Qt pre-scaled by 1/8)
                    nc.tensor.matmul(
                        sc, ident_b, biasq[:, qt, h, :],
                        start=True, stop=False,
                    )
                    nc.tensor.matmul(
                        sc,
                        QtT[ho : ho + DH, hb, ts(qt, P)],
                        KtT[ho : ho + DH, hb, :],
                        start=False, stop=True,
                    )
                    nc.scalar.activation(
                        att_pool[slot], sc, AF.Exp, accum_out=den_pool[slot]
                    )
                    nc.vector.reciprocal(out=rec_pool[slot], in_=den_pool[slot])
                    nc.vector.tensor_scalar_mul(
                        attn_pool[slot], att_pool[slot], rec_pool[slot][:, 0:1]
                    )

            def issue_transp(h):
                ath = ath_pool[h % 2]
                for qt in range(NQT):
                    slot = (h * NQT + qt) % 4
                    atu = atu_pool[qt % 2]
                    for kt in range(NQT):
                        nc.tensor.transpose(
                            atu[:, kt, :], attn_pool[slot][:, ts(kt, P)], ident_b
                        )
                    nc.vector.tensor_copy(ath[:, :, ts(qt, P)], atu)

            def issue_ctx(h):
                ath = ath_pool[h % 2]
                cx = cx_pool[(h // 2) % 2]
                prange = cx[(h % 2) * DH : (h % 2) * DH + DH, :]
                for kt in range(NQT):
                    nc.tensor.matmul(
                        prange,
                        Vsb[:, kt, h * DH : (h + 1) * DH],
                        ath[:, kt, :],
                        start=(kt == 0), stop=(kt == NQT - 1),
                    )
                if h % 2 == 1:
                    nc.vector.tensor_copy(ctxT[:, h // 2, :], cx)

            # software pipeline: transp(h-1) | scores(h) | ctx(h-1)
            issue_scores(0)
            for h in range(1, H):
                issue_transp(h - 1)
                issue_scores(h)
                issue_ctx(h - 1)
            issue_transp(H - 1)
            issue_ctx(H - 1)

        if debug:
            dbgc = const.tile([P, ND, S], fp32, name="dbgc")
            nc.vector.tensor_copy(dbgc.rearrange("p a b -> p (a b)"),
                                  ctxT.rearrange("p a b -> p (a b)"))
            nc.sync.dma_start(out=dbg_ctx_e[:, :, :], in_=dbgc)

        # free attention residents before FFN
        attp_cm.__exit__(None, None, None)

        ffp_cm = tc.tile_pool(name="ffp", bufs=1)
        ffp = ffp_cm.__enter__()
        xln = ffp.tile([P, NQT, D], fp32)
        xlnb = ffp.tile([P, NQT, D], bf16)
        xlnT = ffp.tile([P, ND, S], bf16)
        y1T = ffp.tile([P, NFF, S], bf16)
        # LN param broadcast rows -> [P, D]
        g1b = ffp.tile([P, D], fp32)
        b1b = ffp.tile([P, D], fp32)
        g2b = ffp.tile([P, D], fp32)
        b2b = ffp.tile([P, D], fp32)
        for dst, src_e in ((g1b, g1r_e), (b1b, b1r_e), (g2b, g2r_e), (b2b, b2r_e)):
            row = ffp.tile([1, D], fp32, tag="lnrow", name="lnrow")
            nc.sync.dma_start(out=row, in_=src_e[:, :])
            nc.gpsimd.partition_broadcast(dst, row[0:1, :])

        # =========== Phase 4: merge + residual + LN1 (+ transpose) ===========
        with (
            tc.tile_pool(name="p4sb", bufs=2) as p4sb,
            tc.tile_pool(name="p4w", bufs=1) as p4w,
            tc.tile_pool(name="p4ps", bufs=2, space="PSUM") as p4ps,
            tc.tile_pool(name="p4tp", bufs=2, space="PSUM") as p4tp,
        ):
            wmres = p4w.tile([P, ND, D], bf16)
            nc.sync.dma_start(out=wmres, in_=wmTb_e.rearrange("(n p) d -> p n d", p=P))
            for st in range(NQT):
                qtile = p4sb.tile([P, D], fp32, tag="qtile")
                nc.sync.dma_start(out=qtile, in_=qbm_e[st * P : (st + 1) * P, :])
                x1 = p4sb.tile([P, D], fp32, tag="x1")
                for oh in range(2):
                    ps = p4ps.tile([P, S], fp32, tag="mps")
                    for dblk in range(ND):
                        nc.tensor.matmul(
                            ps,
                            ctxT[:, dblk, ts(st, P)],
                            wmres[:, dblk, ts(oh, S)],
                            start=(dblk == 0),
                            stop=(dblk == ND - 1),
                        )
                    nc.vector.tensor_tensor(
                        out=x1[:, ts(oh, S)], in0=ps,
                        in1=qtile[:, ts(oh, S)], op=ALU.add,
                    )
                _layernorm(nc, p4sb, xln[:, st, :], x1, g1b, b1b, eps_ln_c)
                nc.scalar.activation(xlnb[:, st, :], xln[:, st, :], AF.Copy)
                for dblk in range(ND):
                    tp = p4tp.tile([P, P], bf16, tag="tp")
                    nc.tensor.transpose(
                        tp, xlnb[:, st, ts(dblk, P)], ident_b
                    )
                    nc.vector.tensor_copy(xlnT[:, dblk, ts(st, P)], tp)

        if debug:
            nc.sync.dma_start(out=dbg_xln_e[:, :, :], in_=xln)

        # =========== Phase 5: FFN1 + relu ===========
        with (
            tc.tile_pool(name="p5w", bufs=2) as p5w,
            tc.tile_pool(name="p5ps", bufs=2, space="PSUM") as p5ps,
        ):
            NGRP = 4
            FPG = NFF // NGRP  # 8 ff-blocks per group
            for grp in range(NGRP):
                wf1g = p5w.tile([P, ND, FPG * P], bf16, tag="wf1g")
                nc.sync.dma_start(
                    out=wf1g,
                    in_=wf1Tb_e[:, grp * FPG * P : (grp + 1) * FPG * P].rearrange(
                        "(n p) f -> p n f", p=P
                    ),
                )
                for fl in range(FPG):
                    ffb = grp * FPG + fl
                    ps = p5ps.tile([P, S], fp32, tag="fps")
                    for dblk in range(ND):
                        nc.tensor.matmul(
                            ps,
                            wf1g[:, dblk, ts(fl, P)],
                            xlnT[:, dblk, :],
                            start=(dblk == 0), stop=(dblk == ND - 1),
                        )
                    nc.scalar.activation(
                        y1T[:, ffb, :], ps, AF.Relu,
                        bias=bf1c[:, ffb : ffb + 1], scale=1.0,
                    )

        # =========== Phase 6: FFN2 + residual + LN2 + out ===========
        with (
            tc.tile_pool(name="p7sb", bufs=2) as p7sb,
            tc.tile_pool(name="p7w", bufs=2) as p7w,
            tc.tile_pool(name="p7ps", bufs=1, space="PSUM") as p7ps,
        ):
            fps2 = [
                [p7ps.tile([P, S], fp32, tag=f"f2{st * 2 + oh}", name=f"f2{st}{oh}")
                 for oh in range(2)]
                for st in range(NQT)
            ]
            for st in range(NQT):
                for oh in range(2):
                    nc.tensor.matmul(
                        fps2[st][oh], onesb[:, 0:P], bf2r[:, ts(oh, S)],
                        start=True, stop=False,
                    )
            for ffb in range(NFF):
                wch = p7w.tile([P, D], bf16, tag="wch")
                nc.sync.dma_start(out=wch, in_=wf2Tb_e[ffb * P : (ffb + 1) * P, :])
                for st in range(NQT):
                    for oh in range(2):
                        nc.tensor.matmul(
                            fps2[st][oh],
                            y1T[:, ffb, ts(st, P)],
                            wch[:, ts(oh, S)],
                            start=False,
                            stop=(ffb == NFF - 1),
                        )
            for st in range(NQT):
                x2 = p7sb.tile([P, D], fp32, tag="x2")
                for oh in range(2):
                    nc.vector.tensor_tensor(
                        out=x2[:, ts(oh, S)], in0=fps2[st][oh],
                        in1=xln[:, st, ts(oh, S)], op=ALU.add,
                    )
                xout = p7sb.tile([P, D], fp32, tag="xout")
                _layernorm(nc, p7sb, xout, x2, g2b, b2b, eps_ln_c)
                nc.sync.dma_start(out=out_e[st * P : (st + 1) * P, :], in_=xout)

        ffp_cm.__exit__(None, None, None)
        midp_cm.__exit__(None, None, None)
        const_cm.__exit__(None, None, None)

    nc.finalize()
    return nc


def _prep_inputs(q, kv, attn_map, Wq, bq, Wk, bk, Wv, bv, Wm, bm,
                 Wc1, bc1, Wc2, bc2, Wf1, bf1, Wf2, bf2, g1, b1, g2, b2):
    """Host-side packing. Returns (shared dict, per-core list of dicts)."""
    f32 = np.float32
    bf = ml_dtypes.bfloat16

    def c(a):
        return np.ascontiguousarray(np.asarray(a), dtype=f32)

    def cb(a):
        return np.ascontiguousarray(np.asarray(a, dtype=f32)).astype(bf)

    Wq, Wk, Wv, Wm = c(Wq), c(Wk), c(Wv), c(Wm)
    Wc1, Wc2 = c(Wc1), c(Wc2)
    bq, bk, bv, bm = c(bq), c(bk), c(bv), c(bm)
    bc1, bc2, bf1, bf2 = c(bc1), c(bc2), c(bf1), c(bf2)
    g1, b1, g2, b2 = c(g1), c(b1), c(g2), c(b2)

    shared = {
        "wqTb": cb(Wq.T), "wkTb": cb(Wk.T), "wvTb": cb(Wv.T), "wmTb": cb(Wm.T),
        "wf1Tb": cb(np.asarray(Wf1).T),
        "wf2Tb": cb(np.asarray(Wf2).T),
        "bqc": c((bq / 8.0).reshape(ND, P).T),
        "bkc": c(bk.reshape(ND, P).T),
        "bf1c": c(bf1.reshape(NFF, P).T),
        "bvr": cb(bv.reshape(1, D)),
        "bf2r": cb(bf2.reshape(1, D)),
        "onesb": np.ones((1, S), bf),
        "g1r": g1.reshape(1, D), "b1r": b1.reshape(1, D),
        "g2r": g2.reshape(1, D), "b2r": b2.reshape(1, D),
    }
    # conv block-diag lhsT [K, M]: out[(g,oh)] = sum_c lhsT[(g,c),(g,oh)] rhs[(g,c)]
    c1A = np.zeros((P, P), f32)
    c1B = np.zeros((P, P), f32)
    c2A = np.zeros((P, P), f32)
    c2B = np.zeros((P, P), f32)
    for g in range(8):
        sl = slice(g * 16, g * 16 + 16)
        c1A[sl, sl] = Wc1[0:16, :].T     # [c, oh]
        c1B[sl, sl] = Wc1[16:32, :].T
        c2A[sl, sl] = Wc2[:, 0:16].T     # [ci, h]
        c2B[sl, sl] = Wc2[:, 16:32].T
    shared["c1A"] = c1A.astype(bf)
    shared["c1B"] = c1B.astype(bf)
    shared["c2A"] = c2A.astype(bf)
    shared["c2B"] = c2B.astype(bf)
    shared["bc1Ar"] = np.tile(bc1[0:16], 8).reshape(1, P).astype(bf)
    shared["bc1Br"] = np.tile(bc1[16:32], 8).reshape(1, P).astype(bf)
    shared["bc2c"] = np.tile(bc2, 8).reshape(P, 1).astype(f32)

    q = c(q)
    kv = c(kv)
    per_core = []
    for b in range(B):
        per_core.append({
            "qTb": cb(q[b].T), "kvTb": cb(kv[b].T),
            "qbm": c(q[b] + bm.reshape(1, D)),
            "amapb": cb(np.asarray(attn_map[b])),
        })
    return shared, per_core


def kernel(**inputs):
    if "nc" not in _CACHED:
        _CACHED["nc"] = build_program()
    nc = _CACHED["nc"]
    shared, per_core = _prep_inputs(**inputs)
    in_maps = [dict(shared, **pc) for pc in per_core]
    res = run_bass_kernel_spmd(nc, in_maps, list(range(B)))
    out = np.stack([res.results[i]["out"] for i in range(B)], axis=0)
    return out.astype(np.float32)


# revision 27
# speedup vs baseline: 1.4422x; 1.4141x over previous
"""Trainium2 Bass kernel for nn_CrossAttention (dense transformer block).

Sharding: data-parallel over batch - 8 batch elements, one per NeuronCore.
Each core runs the full block for its batch element:
  bias = Conv1x1(gelu(Conv1x1(log(attn_map[1:,1:] + eps))))
  MHA(q, kv) with bias added to scores; residual + LN; FFN; residual + LN.

Perf structure (v2):
  - all matmuls bf16 (moving+stationary) with fp32 PSUM accumulation
  - strict phase order so the scalar engine loads each activation table once
    (ln -> gelu -> exp -> rsqrt)
  - conv bias kept SBUF-resident in q-major layout via SBUF->SBUF DMA shuffle
  - score bias added by PSUM-init matmul (identity x bias) instead of vector add
  - software-pipelined attention (head h+1 scores issued before head h ctx)
  - bulk DMA (attn_map loads + bias shuffle) on the idle gpsimd queue
"""

import numpy as np
import ml_dtypes

import concourse.bass as bass
import concourse.mybir as mybir
import concourse.tile as tile
from concourse import bacc
from concourse.bass import ts
from concourse.bass_utils import run_bass_kernel_spmd
from concourse.masks import make_identity

AF = mybir.ActivationFunctionType
ALU = mybir.AluOpType

B, S, D, H, DH, FF = 8, 512, 1024, 16, 64, 4096
CH, CHID = 16, 32
EPS_LOG = 1e-6
EPS_LN = 1e-6
P = 128
NQT = S // P          # 4 q-tiles
ND = D // P           # 8 d-blocks
NFF = FF // P         # 32 ff-blocks
AM = 513              # attn_map edge
NQI = 4               # q rows per partition-group in conv
NHALF = S // 32       # 16 conv halves (32 q rows each)

fp32 = mybir.dt.float32
bf16 = mybir.dt.bfloat16

_CACHED = {}


def _layernorm(nc, pool, out_ap, x_ap, gb, bb, eps_c):
    """out = (x - mean(x)) * rsqrt(var(x) + eps) * g + b over free dim (D)."""
    nsub = D // 512
    stats = pool.tile([P, nsub, nc.vector.BN_STATS_DIM], fp32, tag="ln_stats")
    for i in range(nsub):
        nc.vector.bn_stats(out=stats[:, i, :], in_=x_ap[:, ts(i, 512)])
    mv = pool.tile([P, nc.vector.BN_AGGR_DIM], fp32, tag="ln_mv")
    nc.vector.bn_aggr(out=mv, in_=stats)
    rstd = pool.tile([P, 1], fp32, tag="ln_rstd")
    nc.scalar.activation(rstd, mv[:, 1:2], AF.Sqrt, bias=eps_c, scale=1.0)
    nc.vector.reciprocal(out=rstd, in_=rstd)
    u = pool.tile([P, D], fp32, tag="ln_u")
    nc.vector.scalar_tensor_tensor(
        out=u, in0=x_ap, scalar=mv[:, 0:1], in1=gb,
        op0=ALU.subtract, op1=ALU.mult,
    )
    nc.vector.scalar_tensor_tensor(
        out=out_ap, in0=u, scalar=rstd[:, 0:1], in1=bb,
        op0=ALU.mult, op1=ALU.add,
    )


def build_program(debug=False):
    nc = bacc.Bacc(None)

    # ---------------- DRAM I/O ----------------
    qTb_e = nc.dram_tensor("qTb", [D, S], bf16, kind="ExternalInput")
    kvTb_e = nc.dram_tensor("kvTb", [D, S], bf16, kind="ExternalInput")
    qbm_e = nc.dram_tensor("qbm", [S, D], fp32, kind="ExternalInput")  # q + bm
    amapb_e = nc.dram_tensor("amapb", [CH, AM, AM], bf16, kind="ExternalInput")
    wqTb_e = nc.dram_tensor("wqTb", [D, D], bf16, kind="ExternalInput")
    wkTb_e = nc.dram_tensor("wkTb", [D, D], bf16, kind="ExternalInput")
    wvTb_e = nc.dram_tensor("wvTb", [D, D], bf16, kind="ExternalInput")
    wmTb_e = nc.dram_tensor("wmTb", [D, D], bf16, kind="ExternalInput")
    wf1Tb_e = nc.dram_tensor("wf1Tb", [D, FF], bf16, kind="ExternalInput")
    wf2Tb_e = nc.dram_tensor("wf2Tb", [FF, D], bf16, kind="ExternalInput")
    c1A_e = nc.dram_tensor("c1A", [P, P], bf16, kind="ExternalInput")
    c1B_e = nc.dram_tensor("c1B", [P, P], bf16, kind="ExternalInput")
    c2A_e = nc.dram_tensor("c2A", [P, P], bf16, kind="ExternalInput")
    c2B_e = nc.dram_tensor("c2B", [P, P], bf16, kind="ExternalInput")
    # per-partition bias columns
    bqc_e = nc.dram_tensor("bqc", [P, ND], fp32, kind="ExternalInput")   # bq/8
    bkc_e = nc.dram_tensor("bkc", [P, ND], fp32, kind="ExternalInput")
    bc1A_e = nc.dram_tensor("bc1A", [P, 1], fp32, kind="ExternalInput")
    bc1B_e = nc.dram_tensor("bc1B", [P, 1], fp32, kind="ExternalInput")
    bc2c_e = nc.dram_tensor("bc2c", [P, 1], fp32, kind="ExternalInput")
    bf1c_e = nc.dram_tensor("bf1c", [P, NFF], fp32, kind="ExternalInput")
    # bias rows (K=1 matmul trick)
    bvr_e = nc.dram_tensor("bvr", [1, D], bf16, kind="ExternalInput")
    bf2r_e = nc.dram_tensor("bf2r", [1, D], bf16, kind="ExternalInput")
    onesb_e = nc.dram_tensor("onesb", [1, S], bf16, kind="ExternalInput")
    # LN params as rows
    g1r_e = nc.dram_tensor("g1r", [1, D], fp32, kind="ExternalInput")
    b1r_e = nc.dram_tensor("b1r", [1, D], fp32, kind="ExternalInput")
    g2r_e = nc.dram_tensor("g2r", [1, D], fp32, kind="ExternalInput")
    b2r_e = nc.dram_tensor("b2r", [1, D], fp32, kind="ExternalInput")

    out_e = nc.dram_tensor("out", [S, D], fp32, kind="ExternalOutput")
    if debug:
        dbg_qt_e = nc.dram_tensor("dbg_qt", [P, ND, S], fp32, kind="ExternalOutput")
        dbg_kt_e = nc.dram_tensor("dbg_kt", [P, ND, S], fp32, kind="ExternalOutput")
        dbg_v_e = nc.dram_tensor("dbg_v", [P, NQT, D], fp32, kind="ExternalOutput")
        dbg_bias_e = nc.dram_tensor("dbg_bias", [P, H, S], fp32, kind="ExternalOutput")
        dbg_ctx_e = nc.dram_tensor("dbg_ctx", [P, ND, S], fp32, kind="ExternalOutput")
        dbg_xln_e = nc.dram_tensor("dbg_xln", [P, NQT, D], fp32, kind="ExternalOutput")

    with tile.TileContext(nc) as tc:
        # ------------- persistent pools -------------
        const_cm = tc.tile_pool(name="const", bufs=1)
        const = const_cm.__enter__()

        ident_b = const.tile([P, P], bf16)
        make_identity(nc, ident_b)

        eps_log_c = const.tile([P, 1], fp32)
        nc.vector.memset(eps_log_c, EPS_LOG)
        eps_ln_c = const.tile([P, 1], fp32)
        nc.vector.memset(eps_ln_c, EPS_LN)

        c1A = const.tile([P, P], bf16)
        c1B = const.tile([P, P], bf16)
        c2A = const.tile([P, P], bf16)
        c2B = const.tile([P, P], bf16)
        nc.sync.dma_start(out=c1A, in_=c1A_e[:, :])
        nc.sync.dma_start(out=c1B, in_=c1B_e[:, :])
        nc.sync.dma_start(out=c2A, in_=c2A_e[:, :])
        nc.sync.dma_start(out=c2B, in_=c2B_e[:, :])
        bc1A = const.tile([P, 1], fp32)
        bc1B = const.tile([P, 1], fp32)
        bc2c = const.tile([P, 1], fp32)
        nc.sync.dma_start(out=bc1A, in_=bc1A_e[:, :])
        nc.sync.dma_start(out=bc1B, in_=bc1B_e[:, :])
        nc.sync.dma_start(out=bc2c, in_=bc2c_e[:, :])
        bqc = const.tile([P, ND], fp32)
        bkc = const.tile([P, ND], fp32)
        bf1c = const.tile([P, NFF], fp32)
        nc.sync.dma_start(out=bqc, in_=bqc_e[:, :])
        nc.sync.dma_start(out=bkc, in_=bkc_e[:, :])
        nc.sync.dma_start(out=bf1c, in_=bf1c_e[:, :])
        bvr = const.tile([1, D], bf16)
        bf2r = const.tile([1, D], bf16)
        onesb = const.tile([1, S], bf16)
        nc.sync.dma_start(out=bvr, in_=bvr_e[:, :])
        nc.sync.dma_start(out=bf2r, in_=bf2r_e[:, :])
        nc.sync.dma_start(out=onesb, in_=onesb_e[:, :])

        # ctxT outlives attp (merge reads it); entered first for stack order
        midp_cm = tc.tile_pool(name="midp", bufs=1)
        midp = midp_cm.__enter__()
        ctxT = midp.tile([P, ND, S], bf16)     # [(h,dh)-part, blk, q]
        dram_cm = tc.tile_pool(name="dstage", bufs=1, space="DRAM")
        dram = dram_cm.__enter__()
        bstage = dram.tile([S, H, S], bf16)

        # ========== attention-lifetime pool ==========
        attp_cm = tc.tile_pool(name="attp", bufs=1)
        attp = attp_cm.__enter__()
        QtT = attp.tile([P, ND, S], bf16)      # [o-part, o-blk, s]  ((Wq x + bq)/8)
        KtT = attp.tile([P, ND, S], bf16)
        Vsb = attp.tile([P, NQT, D], bf16)     # [k-part, k-blk, (h dh)]
        biasq = attp.tile([P, NQT, H, S], bf16)  # [q-part, qt, h, k]

        # =========== Phase 1: projections ===========
        with (
            tc.tile_pool(name="p1x", bufs=1) as p1x,
            tc.tile_pool(name="p1w", bufs=2) as p1w,
            tc.tile_pool(name="p1ps", bufs=2, space="PSUM") as p1ps,
        ):
            qTb = p1x.tile([P, ND, S], bf16)
            nc.sync.dma_start(out=qTb, in_=qTb_e.rearrange("(n p) s -> p n s", p=P))
            kvTb = p1x.tile([P, ND, S], bf16)
            nc.sync.dma_start(out=kvTb, in_=kvTb_e.rearrange("(n p) s -> p n s", p=P))

            # Q/K: out[o, s] += w[d-blk, o].T @ xT[d-blk, s]
            for wsrc, xsb, dst, bcol, scl in (
                (wqTb_e, qTb, QtT, bqc, 0.125),
                (wkTb_e, kvTb, KtT, bkc, 1.0),
            ):
                wres = p1w.tile([P, ND, D], bf16, tag="wres")
                nc.sync.dma_start(
                    out=wres, in_=wsrc.rearrange("(n p) d -> p n d", p=P)
                )
                for ob in range(ND):
                    ps = p1ps.tile([P, S], fp32, tag="pjps")
                    for dblk in range(ND):
                        nc.tensor.matmul(
                            ps,
                            wres[:, dblk, ts(ob, P)],
                            xsb[:, dblk, :],
                            start=(dblk == 0),
                            stop=(dblk == ND - 1),
                        )
                    nc.vector.tensor_scalar(
                        out=dst[:, ob, :], in0=ps,
                        scalar1=scl, scalar2=bcol[:, ob : ob + 1],
                        op0=ALU.mult, op1=ALU.add,
                    )

            # V: out[k, o] += kvT[d-blk, k-tile].T @ wv[d-blk, o-half]
            wvres = p1w.tile([P, ND, D], bf16, tag="wres")
            nc.sync.dma_start(
                out=wvres, in_=wvTb_e.rearrange("(n p) d -> p n d", p=P)
            )
            for kt in range(NQT):
                for oh in range(2):
                    ps = p1ps.tile([P, S], fp32, tag="pjps")
                    nc.tensor.matmul(
                        ps, onesb[:, 0:P], bvr[:, ts(oh, S)],
                        start=True, stop=False,
                    )
                    for dblk in range(ND):
                        nc.tensor.matmul(
                            ps,
                            kvTb[:, dblk, ts(kt, P)],
                            wvres[:, dblk, ts(oh, S)],
                            start=False,
                            stop=(dblk == ND - 1),
                        )
                    nc.vector.tensor_copy(Vsb[:, kt, ts(oh, S)], ps)

            # =========== Phase 2: conv bias for all halves ===========
            # (shares pool scope so conv can overlap projection tail)
            with (
                tc.tile_pool(name="p2sb", bufs=2) as p2sb,
                tc.tile_pool(name="p2ps", bufs=1, space="PSUM") as p2ps,
            ):
                # stage 1: log of all halves (one table: Ln)
                logms = []
                for half in range(NHALF):
                    qbase = half * 32
                    amt = p2sb.tile([P, NQI, S], bf16, tag="amt")
                    for g in range(8):
                        src = bass.AP(
                            tensor=amapb_e,
                            offset=(1 + qbase + NQI * g) * AM + 1,
                            ap=[[AM * AM, CH], [AM, NQI], [1, S]],
                        )
                        nc.sync.dma_start(out=amt[CH * g : CH * (g + 1)], in_=src)
                    logm = p2sb.tile([P, NQI * S], bf16, tag="logm", bufs=6)
                    nc.scalar.activation(
                        logm, amt.rearrange("p a b -> p (a b)"), AF.Ln,
                        bias=eps_log_c, scale=1.0,
                    )
                    logms.append(logm)

                # stage 2: conv chain (one table: Gelu)
                for half in range(NHALF):
                    qt = half // 4
                    qoff = (half % 4) * 32
                    logm = logms[half]
                    c2sb = p2sb.tile([P, NQI, S], bf16, tag="c2sb")
                    for j in range(2):  # two [P, 1024] chunks
                        pA = p2ps.tile([P, 2 * S], fp32, tag="pA")
                        pB = p2ps.tile([P, 2 * S], fp32, tag="pB")
                        for c in range(2):
                            chk = 2 * j + c
                            nc.tensor.matmul(
                                pA[:, ts(c, S)], c1A, logm[:, ts(chk, S)],
                                start=True, stop=True,
                            )
                            nc.tensor.matmul(
                                pB[:, ts(c, S)], c1B, logm[:, ts(chk, S)],
                                start=True, stop=True,
                            )
                        gA = p2sb.tile([P, 2 * S], bf16, tag="gA")
                        gB = p2sb.tile([P, 2 * S], bf16, tag="gB")
                        nc.scalar.activation(gA, pA, AF.Gelu, bias=bc1A, scale=1.0)
                        nc.scalar.activation(gB, pB, AF.Gelu, bias=bc1B, scale=1.0)
                        pC = p2ps.tile([P, 2 * S], fp32, tag="pC")
                        for c in range(2):
                            nc.tensor.matmul(
                                pC[:, ts(c, S)], c2A, gA[:, ts(c, S)],
                                start=True, stop=False,
                            )
                            nc.tensor.matmul(
                                pC[:, ts(c, S)], c2B, gB[:, ts(c, S)],
                                start=False, stop=True,
                            )
                        nc.vector.tensor_scalar(
                            out=c2sb[:, 2 * j : 2 * j + 2, :].rearrange(
                                "p a b -> p (a b)"
                            ),
                            in0=pC, scalar1=bc2c[:, 0:1], scalar2=None,
                            op0=ALU.add,
                        )
                    # stage to DRAM in [q, h, k] order
                    q0 = half * 32
                    for g in range(8):
                        nc.sync.dma_start(
                            out=bstage[q0 + 4 * g : q0 + 4 * g + 4].rearrange(
                                "q h k -> h q k"
                            ),
                            in_=c2sb[CH * g : CH * (g + 1)],
                        )

                # read back q-major into SBUF-resident bias
                for qt in range(NQT):
                    nc.sync.dma_start(
                        out=biasq[:, qt], in_=bstage[qt * P : (qt + 1) * P]
                    )

        if debug:
            dbgq = const.tile([P, ND, S], fp32, name="dbgq")
            nc.vector.tensor_copy(dbgq.rearrange("p a b -> p (a b)"),
                                  QtT.rearrange("p a b -> p (a b)"))
            nc.sync.dma_start(out=dbg_qt_e[:, :, :], in_=dbgq)
            nc.vector.tensor_copy(dbgq.rearrange("p a b -> p (a b)"),
                                  KtT.rearrange("p a b -> p (a b)"))
            nc.sync.dma_start(out=dbg_kt_e[:, :, :], in_=dbgq)
            dbgv = const.tile([P, NQT, D], fp32, name="dbgv")
            nc.vector.tensor_copy(dbgv.rearrange("p a b -> p (a b)"),
                                  Vsb.rearrange("p a b -> p (a b)"))
            nc.sync.dma_start(out=dbg_v_e[:, :, :], in_=dbgv)
            dbgb = const.tile([P, H, S], fp32, name="dbgb")
            nc.vector.tensor_copy(dbgb.rearrange("p a b -> p (a b)"),
                                  biasq[:, 0].rearrange("p a b -> p (a b)"))
            nc.sync.dma_start(out=dbg_bias_e[:, :, :], in_=dbgb)

        # =========== Phase 3: attention (one table: Exp) ===========
        with (
            tc.tile_pool(name="p3sb", bufs=1) as p3sb,
            tc.tile_pool(name="p3ps", bufs=1, space="PSUM") as p3ps,
        ):
            sc_pool = [p3ps.tile([P, S], fp32, tag=f"sc{i}", name=f"sc{i}")
                       for i in range(4)]
            atu_pool = [p3ps.tile([P, NQT, P], bf16, tag=f"atu{i}", name=f"atu{i}")
                        for i in range(2)]
            cx_pool = [p3ps.tile([P, S], fp32, tag=f"cx{i}", name=f"cx{i}")
                       for i in range(2)]
            att_pool = [p3sb.tile([P, S], bf16, tag=f"att{i}", name=f"att{i}")
                        for i in range(4)]
            attn_pool = [p3sb.tile([P, S], bf16, tag=f"attn{i}", name=f"attn{i}")
                         for i in range(4)]
            den_pool = [p3sb.tile([P, 1], fp32, tag=f"den{i}", name=f"den{i}")
                        for i in range(4)]
            rec_pool = [p3sb.tile([P, 1], fp32, tag=f"rec{i}", name=f"rec{i}")
                        for i in range(4)]
            ath_pool = [p3sb.tile([P, NQT, S], bf16, tag=f"ath{i}", name=f"ath{i}")
                        for i in range(2)]

            def issue_scores(h):
                hb, ho = (h * DH) // P, (h * DH) % P
                for qt in range(NQT):
                    slot = (h * NQT + qt) % 4
                    sc = sc_pool[slot]
                    # psum <- bias, then += Qt^T K (Qt pre-scaled by 1/8)
                    nc.tensor.matmul(
                        sc, ident_b, biasq[:, qt, h, :],
                        start=True, stop=False,
                    )
                    nc.tensor.matmul(
                        sc,
                        QtT[ho : ho + DH, hb, ts(qt, P)],
                        KtT[ho : ho + DH, hb, :],
                        start=False, stop=True,
                    )
                    nc.scalar.activation(
                        att_pool[slot], sc, AF.Exp, accum_out=den_pool[slot]
                    )
                    nc.vector.reciprocal(out=rec_pool[slot], in_=den_pool[slot])
                    nc.vector.tensor_scalar_mul(
                        attn_pool[slot], att_pool[slot], rec_pool[slot][:, 0:1]
                    )

            def issue_transp(h):
                ath = ath_pool[h % 2]
                for qt in range(NQT):
                    slot = (h * NQT + qt) % 4
                    atu = atu_pool[qt % 2]
                    for kt in range(NQT):
                        nc.tensor.transpose(
                            atu[:, kt, :], attn_pool[slot][:, ts(kt, P)], ident_b
                        )
                    nc.vector.tensor_copy(ath[:, :, ts(qt, P)], atu)

            def issue_ctx(h):
                ath = ath_pool[h % 2]
                cx = cx_pool[(h // 2) % 2]
                prange = cx[(h % 2) * DH : (h % 2) * DH + DH, :]
                for kt in range(NQT):
                    nc.tensor.matmul(
                        prange,
                        Vsb[:, kt, h * DH : (h + 1) * DH],
                        ath[:, kt, :],
                        start=(kt == 0), stop=(kt == NQT - 1),
                    )
                if h % 2 == 1:
                    nc.vector.tensor_copy(ctxT[:, h // 2, :], cx)

            # software pipeline: transp(h-1) | scores(h) | ctx(h-1)
            issue_scores(0)
            for h in range(1, H):
                issue_transp(h - 1)
                issue_scores(h)
                issue_ctx(h - 1)
            issue_transp(H - 1)
            issue_ctx(H - 1)

        if debug:
            dbgc = const.tile([P, ND, S], fp32, name="dbgc")
            nc.vector.tensor_copy(dbgc.rearrange("p a b -> p (a b)"),
                                  ctxT.rearrange("p a b -> p (a b)"))
            nc.sync.dma_start(out=dbg_ctx_e[:, :, :], in_=dbgc)

        # free attention residents before FFN
        attp_cm.__exit__(None, None, None)

        ffp_cm = tc.tile_pool(name="ffp", bufs=1)
        ffp = ffp_cm.__enter__()
        xln = ffp.tile([P, NQT, D], fp32)
        xlnb = ffp.tile([P, NQT, D], bf16)
        xlnT = ffp.tile([P, ND, S], bf16)
        y1T = ffp.tile([P, NFF, S], bf16)
        # LN param broadcast rows -> [P, D]
        g1b = ffp.tile([P, D], fp32)
        b1b = ffp.tile([P, D], fp32)
        g2b = ffp.tile([P, D], fp32)
        b2b = ffp.tile([P, D], fp32)
        for dst, src_e in ((g1b, g1r_e), (b1b, b1r_e), (g2b, g2r_e), (b2b, b2r_e)):
            row = ffp.tile([1, D], fp32, tag="lnrow", name="lnrow")
            nc.sync.dma_start(out=row, in_=src_e[:, :])
            nc.gpsimd.partition_broadcast(dst, row[0:1, :])

        # =========== Phase 4: merge + residual + LN1 (+ transpose) ===========
        with (
            tc.tile_pool(name="p4sb", bufs=2) as p4sb,
            tc.tile_pool(name="p4w", bufs=1) as p4w,
            tc.tile_pool(name="p4ps", bufs=2, space="PSUM") as p4ps,
            tc.tile_pool(name="p4tp", bufs=2, space="PSUM") as p4tp,
        ):
            wmres = p4w.tile([P, ND, D], bf16)
            nc.sync.dma_start(out=wmres, in_=wmTb_e.rearrange("(n p) d -> p n d", p=P))
            for st in range(NQT):
                qtile = p4sb.tile([P, D], fp32, tag="qtile")
                nc.sync.dma_start(out=qtile, in_=qbm_e[st * P : (st + 1) * P, :])
                x1 = p4sb.tile([P, D], fp32, tag="x1")
                for oh in range(2):
                    ps = p4ps.tile([P, S], fp32, tag="mps")
                    for dblk in range(ND):
                        nc.tensor.matmul(
                            ps,
                            ctxT[:, dblk, ts(st, P)],
                            wmres[:, dblk, ts(oh, S)],
                            start=(dblk == 0),
                            stop=(dblk == ND - 1),
                        )
                    nc.vector.tensor_tensor(
                        out=x1[:, ts(oh, S)], in0=ps,
                        in1=qtile[:, ts(oh, S)], op=ALU.add,
                    )
                _layernorm(nc, p4sb, xln[:, st, :], x1, g1b, b1b, eps_ln_c)
                nc.scalar.activation(xlnb[:, st, :], xln[:, st, :], AF.Copy)
                for dblk in range(ND):
                    tp = p4tp.tile([P, P], bf16, tag="tp")
                    nc.tensor.transpose(
                        tp, xlnb[:, st, ts(dblk, P)], ident_b
                    )
                    nc.vector.tensor_copy(xlnT[:, dblk, ts(st, P)], tp)

        if debug:
            nc.sync.dma_start(out=dbg_xln_e[:, :, :], in_=xln)

        # =========== Phase 5: FFN1 + relu ===========
        with (
            tc.tile_pool(name="p5w", bufs=2) as p5w,
            tc.tile_pool(name="p5ps", bufs=2, space="PSUM") as p5ps,
        ):
            NGRP = 4
            FPG = NFF // NGRP  # 8 ff-blocks per group
            for grp in range(NGRP):
                wf1g = p5w.tile([P, ND, FPG * P], bf16, tag="wf1g")
                nc.sync.dma_start(
                    out=wf1g,
                    in_=wf1Tb_e[:, grp * FPG * P : (grp + 1) * FPG * P].rearrange(
                        "(n p) f -> p n f", p=P
                    ),
                )
                for fl in range(FPG):
                    ffb = grp * FPG + fl
                    ps = p5ps.tile([P, S], fp32, tag="fps")
                    for dblk in range(ND):
                        nc.tensor.matmul(
                            ps,
                            wf1g[:, dblk, ts(fl, P)],
                            xlnT[:, dblk, :],
                            start=(dblk == 0), stop=(dblk == ND - 1),
                        )
                    nc.scalar.activation(
                        y1T[:, ffb, :], ps, AF.Relu,
                        bias=bf1c[:, ffb : ffb + 1], scale=1.0,
                    )

        # =========== Phase 6: FFN2 + residual + LN2 + out ===========
        with (
            tc.tile_pool(name="p7sb", bufs=2) as p7sb,
            tc.tile_pool(name="p7w", bufs=2) as p7w,
            tc.tile_pool(name="p7ps", bufs=1, space="PSUM") as p7ps,
        ):
            fps2 = [
                [p7ps.tile([P, S], fp32, tag=f"f2{st * 2 + oh}", name=f"f2{st}{oh}")
                 for oh in range(2)]
                for st in range(NQT)
            ]
            for st in range(NQT):
                for oh in range(2):
                    nc.tensor.matmul(
                        fps2[st][oh], onesb[:, 0:P], bf2r[:, ts(oh, S)],
                        start=True, stop=False,
                    )
            for ffb in range(NFF):
                wch = p7w.tile([P, D], bf16, tag="wch")
                nc.sync.dma_start(out=wch, in_=wf2Tb_e[ffb * P : (ffb + 1) * P, :])
                for st in range(NQT):
                    for oh in range(2):
                        nc.tensor.matmul(
                            fps2[st][oh],
                            y1T[:, ffb, ts(st, P)],
                            wch[:, ts(oh, S)],
                            start=False,
                            stop=(ffb == NFF - 1),
                        )
            for st in range(NQT):
                x2 = p7sb.tile([P, D], fp32, tag="x2")
                for oh in range(2):
                    nc.vector.tensor_tensor(
                        out=x2[:, ts(oh, S)], in0=fps2[st][oh],
                        in1=xln[:, st, ts(oh, S)], op=ALU.add,
                    )
                xout = p7sb.tile([P, D], fp32, tag="xout")
                _layernorm(nc, p7sb, xout, x2, g2b, b2b, eps_ln_c)
                nc.sync.dma_start(out=out_e[st * P : (st + 1) * P, :], in_=xout)

        ffp_cm.__exit__(None, None, None)
        dram_cm.__exit__(None, None, None)
        midp_cm.__exit__(None, None, None)
        const_cm.__exit__(None, None, None)

    nc.finalize()
    return nc


def _prep_inputs(q, kv, attn_map, Wq, bq, Wk, bk, Wv, bv, Wm, bm,
                 Wc1, bc1, Wc2, bc2, Wf1, bf1, Wf2, bf2, g1, b1, g2, b2):
    """Host-side packing. Returns (shared dict, per-core list of dicts)."""
    f32 = np.float32
    bf = ml_dtypes.bfloat16

    def c(a):
        return np.ascontiguousarray(np.asarray(a), dtype=f32)

    def cb(a):
        return np.ascontiguousarray(np.asarray(a, dtype=f32)).astype(bf)

    Wq, Wk, Wv, Wm = c(Wq), c(Wk), c(Wv), c(Wm)
    Wc1, Wc2 = c(Wc1), c(Wc2)
    bq, bk, bv, bm = c(bq), c(bk), c(bv), c(bm)
    bc1, bc2, bf1, bf2 = c(bc1), c(bc2), c(bf1), c(bf2)
    g1, b1, g2, b2 = c(g1), c(b1), c(g2), c(b2)

    shared = {
        "wqTb": cb(Wq.T), "wkTb": cb(Wk.T), "wvTb": cb(Wv.T), "wmTb": cb(Wm.T),
        "wf1Tb": cb(np.asarray(Wf1).T),
        "wf2Tb": cb(np.asarray(Wf2).T),
        "bqc": c((bq / 8.0).reshape(ND, P).T),
        "bkc": c(bk.reshape(ND, P).T),
        "bf1c": c(bf1.reshape(NFF, P).T),
        "bvr": cb(bv.reshape(1, D)),
        "bf2r": cb(bf2.reshape(1, D)),
        "onesb": np.ones((1, S), bf),
        "g1r": g1.reshape(1, D), "b1r": b1.reshape(1, D),
        "g2r": g2.reshape(1, D), "b2r": b2.reshape(1, D),
    }
    # conv block-diag lhsT [K, M]: out[(g,oh)] = sum_c lhsT[(g,c),(g,oh)] rhs[(g,c)]
    c1A = np.zeros((P, P), f32)
    c1B = np.zeros((P, P), f32)
    c2A = np.zeros((P, P), f32)
    c2B = np.zeros((P, P), f32)
    for g in range(8):
        sl = slice(g * 16, g * 16 + 16)
        c1A[sl, sl] = Wc1[0:16, :].T     # [c, oh]
        c1B[sl, sl] = Wc1[16:32, :].T
        c2A[sl, sl] = Wc2[:, 0:16].T     # [ci, h]
        c2B[sl, sl] = Wc2[:, 16:32].T
    shared["c1A"] = c1A.astype(bf)
    shared["c1B"] = c1B.astype(bf)
    shared["c2A"] = c2A.astype(bf)
    shared["c2B"] = c2B.astype(bf)
    shared["bc1A"] = np.tile(bc1[0:16], 8).reshape(P, 1).astype(f32)
    shared["bc1B"] = np.tile(bc1[16:32], 8).reshape(P, 1).astype(f32)
    shared["bc2c"] = np.tile(bc2, 8).reshape(P, 1).astype(f32)

    q = c(q)
    kv = c(kv)
    per_core = []
    for b in range(B):
        per_core.append({
            "qTb": cb(q[b].T), "kvTb": cb(kv[b].T),
            "qbm": c(q[b] + bm.reshape(1, D)),
            "amapb": cb(np.asarray(attn_map[b])),
        })
    return shared, per_core


def kernel(**inputs):
    if "nc" not in _CACHED:
        _CACHED["nc"] = build_program()
    nc = _CACHED["nc"]
    shared, per_core = _prep_inputs(**inputs)
    in_maps = [dict(shared, **pc) for pc in per_core]
    res = run_bass_kernel_spmd(nc, in_maps, list(range(B)))
    out = np.stack([res.results[i]["out"] for i in range(B)], axis=0)
    return out.astype(np.float32)


# revision 32
# speedup vs baseline: 1.5402x; 1.0680x over previous
"""Trainium2 Bass kernel for nn_CrossAttention (dense transformer block).

Sharding: data-parallel over batch - 8 batch elements, one per NeuronCore.
Each core runs the full block for its batch element:
  bias = Conv1x1(gelu(Conv1x1(log(attn_map[1:,1:] + eps))))
  MHA(q, kv) with bias added to scores; residual + LN; FFN; residual + LN.

Perf structure (v2):
  - all matmuls bf16 (moving+stationary) with fp32 PSUM accumulation
  - strict phase order so the scalar engine loads each activation table once
    (ln -> gelu -> exp -> rsqrt)
  - conv bias kept SBUF-resident in q-major layout via SBUF->SBUF DMA shuffle
  - score bias added by PSUM-init matmul (identity x bias) instead of vector add
  - software-pipelined attention (head h+1 scores issued before head h ctx)
  - bulk DMA (attn_map loads + bias shuffle) on the idle gpsimd queue
"""

import numpy as np
import ml_dtypes

import concourse.bass as bass
import concourse.mybir as mybir
import concourse.tile as tile
from concourse import bacc
from concourse.bass import ts
from concourse.bass_utils import run_bass_kernel_spmd
from concourse.masks import make_identity

AF = mybir.ActivationFunctionType
ALU = mybir.AluOpType

B, S, D, H, DH, FF = 8, 512, 1024, 16, 64, 4096
CH, CHID = 16, 32
EPS_LOG = 1e-6
EPS_LN = 1e-6
P = 128
NQT = S // P          # 4 q-tiles
ND = D // P           # 8 d-blocks
NFF = FF // P         # 32 ff-blocks
AM = 513              # attn_map edge
NQI = 4               # q rows per partition-group in conv
NHALF = S // 32       # 16 conv halves (32 q rows each)

fp32 = mybir.dt.float32
bf16 = mybir.dt.bfloat16
fp8e4 = mybir.dt.float8e4

_CACHED = {}


def _layernorm(nc, pool, out_ap, x_ap, gb, bb, eps_c):
    """out = (x - mean(x)) * rsqrt(var(x) + eps) * g + b over free dim (D)."""
    nsub = D // 512
    stats = pool.tile([P, nsub, nc.vector.BN_STATS_DIM], fp32, tag="ln_stats")
    for i in range(nsub):
        nc.vector.bn_stats(out=stats[:, i, :], in_=x_ap[:, ts(i, 512)])
    mv = pool.tile([P, nc.vector.BN_AGGR_DIM], fp32, tag="ln_mv")
    nc.vector.bn_aggr(out=mv, in_=stats)
    rstd = pool.tile([P, 1], fp32, tag="ln_rstd")
    nc.scalar.activation(rstd, mv[:, 1:2], AF.Sqrt, bias=eps_c, scale=1.0)
    nc.vector.reciprocal(out=rstd, in_=rstd)
    u = pool.tile([P, D], fp32, tag="ln_u")
    nc.vector.scalar_tensor_tensor(
        out=u, in0=x_ap, scalar=mv[:, 0:1], in1=gb,
        op0=ALU.subtract, op1=ALU.mult,
    )
    nc.vector.scalar_tensor_tensor(
        out=out_ap, in0=u, scalar=rstd[:, 0:1], in1=bb,
        op0=ALU.mult, op1=ALU.add,
    )


def build_program(debug=False):
    nc = bacc.Bacc(None)

    # ---------------- DRAM I/O ----------------
    qTb_e = nc.dram_tensor("qTb", [D, S], bf16, kind="ExternalInput")
    kvTb_e = nc.dram_tensor("kvTb", [D, S], bf16, kind="ExternalInput")
    qbm_e = nc.dram_tensor("qbm", [S, D], fp32, kind="ExternalInput")  # q + bm
    amapb_e = nc.dram_tensor("amapb", [CH, AM, AM], bf16, kind="ExternalInput")
    wqTb_e = nc.dram_tensor("wqTb", [D, D], bf16, kind="ExternalInput")
    wkTb_e = nc.dram_tensor("wkTb", [D, D], bf16, kind="ExternalInput")
    wvTb_e = nc.dram_tensor("wvTb", [D, D], bf16, kind="ExternalInput")
    wmTb_e = nc.dram_tensor("wmTb", [D, D], bf16, kind="ExternalInput")
    wf1Tb_e = nc.dram_tensor("wf1Tb", [D, FF], bf16, kind="ExternalInput")
    wf2Tb_e = nc.dram_tensor("wf2Tb", [FF, D], bf16, kind="ExternalInput")
    c1A_e = nc.dram_tensor("c1A", [P, P], bf16, kind="ExternalInput")
    c1B_e = nc.dram_tensor("c1B", [P, P], bf16, kind="ExternalInput")
    c2A_e = nc.dram_tensor("c2A", [P, P], bf16, kind="ExternalInput")
    c2B_e = nc.dram_tensor("c2B", [P, P], bf16, kind="ExternalInput")
    # per-partition bias columns
    bqc_e = nc.dram_tensor("bqc", [P, ND], fp32, kind="ExternalInput")   # bq/8
    bkc_e = nc.dram_tensor("bkc", [P, ND], fp32, kind="ExternalInput")
    bc1A_e = nc.dram_tensor("bc1A", [P, 1], fp32, kind="ExternalInput")
    bc1B_e = nc.dram_tensor("bc1B", [P, 1], fp32, kind="ExternalInput")
    bc2c_e = nc.dram_tensor("bc2c", [P, 1], fp32, kind="ExternalInput")
    bf1c_e = nc.dram_tensor("bf1c", [P, NFF], fp32, kind="ExternalInput")
    # bias rows (K=1 matmul trick)
    bvr_e = nc.dram_tensor("bvr", [1, D], bf16, kind="ExternalInput")
    bf2r_e = nc.dram_tensor("bf2r", [1, D], bf16, kind="ExternalInput")
    onesb_e = nc.dram_tensor("onesb", [1, S], bf16, kind="ExternalInput")
    ident8_e = nc.dram_tensor("ident8", [P, P], fp8e4, kind="ExternalInput")
    # LN params as rows
    g1r_e = nc.dram_tensor("g1r", [1, D], fp32, kind="ExternalInput")
    b1r_e = nc.dram_tensor("b1r", [1, D], fp32, kind="ExternalInput")
    g2r_e = nc.dram_tensor("g2r", [1, D], fp32, kind="ExternalInput")
    b2r_e = nc.dram_tensor("b2r", [1, D], fp32, kind="ExternalInput")

    out_e = nc.dram_tensor("out", [S, D], fp32, kind="ExternalOutput")
    if debug:
        dbg_qt_e = nc.dram_tensor("dbg_qt", [P, ND, S], fp32, kind="ExternalOutput")
        dbg_kt_e = nc.dram_tensor("dbg_kt", [P, ND, S], fp32, kind="ExternalOutput")
        dbg_v_e = nc.dram_tensor("dbg_v", [P, NQT, D], fp32, kind="ExternalOutput")
        dbg_bias_e = nc.dram_tensor("dbg_bias", [P, H, S], fp32, kind="ExternalOutput")
        dbg_ctx_e = nc.dram_tensor("dbg_ctx", [P, ND, S], fp32, kind="ExternalOutput")
        dbg_xln_e = nc.dram_tensor("dbg_xln", [P, NQT, D], fp32, kind="ExternalOutput")

    with tile.TileContext(nc) as tc:
        # ------------- persistent pools -------------
        const_cm = tc.tile_pool(name="const", bufs=1)
        const = const_cm.__enter__()

        ident_b = const.tile([P, P], bf16)
        make_identity(nc, ident_b)
        ident8 = const.tile([P, P], fp8e4)
        nc.sync.dma_start(out=ident8, in_=ident8_e[:, :])

        eps_log_c = const.tile([P, 1], fp32)
        nc.vector.memset(eps_log_c, EPS_LOG)
        eps_ln_c = const.tile([P, 1], fp32)
        nc.vector.memset(eps_ln_c, EPS_LN)

        c1A = const.tile([P, P], bf16)
        c1B = const.tile([P, P], bf16)
        c2A = const.tile([P, P], bf16)
        c2B = const.tile([P, P], bf16)
        nc.sync.dma_start(out=c1A, in_=c1A_e[:, :])
        nc.sync.dma_start(out=c1B, in_=c1B_e[:, :])
        nc.sync.dma_start(out=c2A, in_=c2A_e[:, :])
        nc.sync.dma_start(out=c2B, in_=c2B_e[:, :])
        bc1A = const.tile([P, 1], fp32)
        bc1B = const.tile([P, 1], fp32)
        bc2c = const.tile([P, 1], fp32)
        nc.sync.dma_start(out=bc1A, in_=bc1A_e[:, :])
        nc.sync.dma_start(out=bc1B, in_=bc1B_e[:, :])
        nc.sync.dma_start(out=bc2c, in_=bc2c_e[:, :])
        bqc = const.tile([P, ND], fp32)
        bkc = const.tile([P, ND], fp32)
        bf1c = const.tile([P, NFF], fp32)
        nc.sync.dma_start(out=bqc, in_=bqc_e[:, :])
        nc.sync.dma_start(out=bkc, in_=bkc_e[:, :])
        nc.sync.dma_start(out=bf1c, in_=bf1c_e[:, :])
        bvr = const.tile([1, D], bf16)
        bf2r = const.tile([1, D], bf16)
        onesb = const.tile([1, S], bf16)
        nc.sync.dma_start(out=bvr, in_=bvr_e[:, :])
        nc.sync.dma_start(out=bf2r, in_=bf2r_e[:, :])
        nc.sync.dma_start(out=onesb, in_=onesb_e[:, :])

        # ctxT outlives attp (merge reads it); entered first for stack order
        midp_cm = tc.tile_pool(name="midp", bufs=1)
        midp = midp_cm.__enter__()
        ctxT = midp.tile([P, ND, S], bf16)     # [(h,dh)-part, blk, q]
        dram_cm = tc.tile_pool(name="dstage", bufs=1, space="DRAM")
        dram = dram_cm.__enter__()
        bstage = dram.tile([S, H, S], fp8e4)

        # ========== attention-lifetime pool ==========
        attp_cm = tc.tile_pool(name="attp", bufs=1)
        attp = attp_cm.__enter__()
        QtT = attp.tile([P, ND, S], bf16)      # [o-part, o-blk, s]  ((Wq x + bq)/8)
        KtT = attp.tile([P, ND, S], bf16)
        Vsb = attp.tile([P, NQT, D], bf16)     # [k-part, k-blk, (h dh)]
        biasq = attp.tile([P, NQT, H, S], fp8e4)  # [q-part, qt, h, k]

        # =========== Phase 1+2: projections + conv bias ===========
        NQI2 = 8          # q rows per partition-group per conv block
        NH2 = S // (8 * NQI2)  # 8 conv blocks of 64 q rows
        with (
            tc.tile_pool(name="p2sb", bufs=2) as p2sb,
            tc.tile_pool(name="p2ps", bufs=1, space="PSUM") as p2ps,
        ):
            with (
                tc.tile_pool(name="p1x", bufs=1) as p1x,
                tc.tile_pool(name="p1w", bufs=2) as p1w,
                tc.tile_pool(name="p1ps", bufs=2, space="PSUM") as p1ps,
            ):
                qTb = p1x.tile([P, ND, S], bf16)
                nc.sync.dma_start(
                    out=qTb, in_=qTb_e.rearrange("(n p) s -> p n s", p=P)
                )
                kvTb = p1x.tile([P, ND, S], bf16)
                nc.sync.dma_start(
                    out=kvTb, in_=kvTb_e.rearrange("(n p) s -> p n s", p=P)
                )

                # Q/K: out[o, s] += w[d-blk, o].T @ xT[d-blk, s]
                for wsrc, xsb, dst, bcol, scl in (
                    (wqTb_e, qTb, QtT, bqc, 0.125),
                    (wkTb_e, kvTb, KtT, bkc, 1.0),
                ):
                    wres = p1w.tile([P, ND, D], bf16, tag="wres")
                    nc.sync.dma_start(
                        out=wres, in_=wsrc.rearrange("(n p) d -> p n d", p=P)
                    )
                    for ob in range(ND):
                        ps = p1ps.tile([P, S], fp32, tag="pjps")
                        for dblk in range(ND):
                            nc.tensor.matmul(
                                ps,
                                wres[:, dblk, ts(ob, P)],
                                xsb[:, dblk, :],
                                start=(dblk == 0),
                                stop=(dblk == ND - 1),
                            )
                        nc.vector.tensor_scalar(
                            out=dst[:, ob, :], in0=ps,
                            scalar1=scl, scalar2=bcol[:, ob : ob + 1],
                            op0=ALU.mult, op1=ALU.add,
                        )

                # V: out[k, o] += kvT[d-blk, k-tile].T @ wv[d-blk, o-half]
                wvres = p1w.tile([P, ND, D], bf16, tag="wres")
                nc.sync.dma_start(
                    out=wvres, in_=wvTb_e.rearrange("(n p) d -> p n d", p=P)
                )
                for kt in range(NQT):
                    for oh in range(2):
                        ps = p1ps.tile([P, S], fp32, tag="pjps")
                        nc.tensor.matmul(
                            ps, onesb[:, 0:P], bvr[:, ts(oh, S)],
                            start=True, stop=False,
                        )
                        for dblk in range(ND):
                            nc.tensor.matmul(
                                ps,
                                kvTb[:, dblk, ts(kt, P)],
                                wvres[:, dblk, ts(oh, S)],
                                start=False,
                                stop=(dblk == ND - 1),
                            )
                        nc.vector.tensor_copy(Vsb[:, kt, ts(oh, S)], ps)

            # conv in 2 super-blocks: [Ln x4 | fence | conv x4] x2
            # so each stage loads its activation table exactly once
            HPB = NH2 // 2
            for sblk in range(2):
                logms = []
                for half in range(sblk * HPB, (sblk + 1) * HPB):
                    qbase = half * 64
                    amt = p2sb.tile([P, NQI2, S], bf16, tag="amt")
                    for g in range(8):
                        src = bass.AP(
                            tensor=amapb_e,
                            offset=(1 + qbase + NQI2 * g) * AM + 1,
                            ap=[[AM * AM, CH], [AM, NQI2], [1, S]],
                        )
                        nc.sync.dma_start(
                            out=amt[CH * g : CH * (g + 1)], in_=src
                        )
                    logm = p2sb.tile([P, NQI2 * S], bf16, tag="logm", bufs=4)
                    amtf = amt.rearrange("p a b -> p (a b)")
                    for i in range(2):
                        nc.scalar.activation(
                            logm[:, ts(i, 4 * S)], amtf[:, ts(i, 4 * S)], AF.Ln,
                            bias=eps_log_c, scale=1.0,
                        )
                    logms.append(logm)

                tc.no_sync_barrier()

                for hi, half in enumerate(range(sblk * HPB, (sblk + 1) * HPB)):
                    qbase = half * 64
                    logm = logms[hi]
                    c2sb = p2sb.tile([P, NQI2, S], fp8e4, tag="c2sb")
                    for j in range(4):  # four [P, 1024] chunks
                        pA = p2ps.tile([P, 2 * S], fp32, tag="pA")
                        pB = p2ps.tile([P, 2 * S], fp32, tag="pB")
                        for c in range(2):
                            chk = 2 * j + c
                            nc.tensor.matmul(
                                pA[:, ts(c, S)], c1A, logm[:, ts(chk, S)],
                                start=True, stop=True,
                            )
                            nc.tensor.matmul(
                                pB[:, ts(c, S)], c1B, logm[:, ts(chk, S)],
                                start=True, stop=True,
                            )
                        gA = p2sb.tile([P, 2 * S], bf16, tag="gA")
                        gB = p2sb.tile([P, 2 * S], bf16, tag="gB")
                        nc.scalar.activation(gA, pA, AF.Gelu, bias=bc1A, scale=1.0)
                        nc.scalar.activation(gB, pB, AF.Gelu, bias=bc1B, scale=1.0)
                        pC = p2ps.tile([P, 2 * S], fp32, tag="pC")
                        for c in range(2):
                            nc.tensor.matmul(
                                pC[:, ts(c, S)], c2A, gA[:, ts(c, S)],
                                start=True, stop=False,
                            )
                            nc.tensor.matmul(
                                pC[:, ts(c, S)], c2B, gB[:, ts(c, S)],
                                start=False, stop=True,
                            )
                        nc.vector.tensor_scalar(
                            out=c2sb[:, 2 * j : 2 * j + 2, :].rearrange(
                                "p a b -> p (a b)"
                            ),
                            in0=pC, scalar1=bc2c[:, 0:1], scalar2=None,
                            op0=ALU.add,
                        )
                    # stage to DRAM in [q, h, k] order
                    for g in range(8):
                        q0 = qbase + NQI2 * g
                        nc.sync.dma_start(
                            out=bstage[q0 : q0 + NQI2].rearrange(
                                "q h k -> h q k"
                            ),
                            in_=c2sb[CH * g : CH * (g + 1)],
                        )
                    # read back q-major as soon as a qtile's rows are staged
                    if half % 2 == 1:
                        qt = half // 2
                        nc.sync.dma_start(
                            out=biasq[:, qt],
                            in_=bstage[qt * P : (qt + 1) * P],
                        )
                if sblk == 0:
                    tc.no_sync_barrier()

        if debug:
            dbgq = const.tile([P, ND, S], fp32, name="dbgq")
            nc.vector.tensor_copy(dbgq.rearrange("p a b -> p (a b)"),
                                  QtT.rearrange("p a b -> p (a b)"))
            nc.sync.dma_start(out=dbg_qt_e[:, :, :], in_=dbgq)
            nc.vector.tensor_copy(dbgq.rearrange("p a b -> p (a b)"),
                                  KtT.rearrange("p a b -> p (a b)"))
            nc.sync.dma_start(out=dbg_kt_e[:, :, :], in_=dbgq)
            dbgv = const.tile([P, NQT, D], fp32, name="dbgv")
            nc.vector.tensor_copy(dbgv.rearrange("p a b -> p (a b)"),
                                  Vsb.rearrange("p a b -> p (a b)"))
            nc.sync.dma_start(out=dbg_v_e[:, :, :], in_=dbgv)
            dbgb = const.tile([P, H, S], fp32, name="dbgb")
            nc.vector.tensor_copy(dbgb.rearrange("p a b -> p (a b)"),
                                  biasq[:, 0].rearrange("p a b -> p (a b)"))
            nc.sync.dma_start(out=dbg_bias_e[:, :, :], in_=dbgb)

        # =========== Phase 3: attention (one table: Exp) ===========
        tc.no_sync_barrier()
        with (
            tc.tile_pool(name="p3sb", bufs=1) as p3sb,
            tc.tile_pool(name="p3ps", bufs=1, space="PSUM") as p3ps,
        ):
            sc_pool = [p3ps.tile([P, S], fp32, tag=f"sc{i}", name=f"sc{i}")
                       for i in range(4)]
            atu_pool = [p3ps.tile([P, NQT, P], bf16, tag=f"atu{i}", name=f"atu{i}")
                        for i in range(2)]
            cx_pool = [p3ps.tile([P, S], fp32, tag=f"cx{i}", name=f"cx{i}")
                       for i in range(2)]
            att_pool = [p3sb.tile([P, S], bf16, tag=f"att{i}", name=f"att{i}")
                        for i in range(4)]
            attn_pool = [p3sb.tile([P, S], bf16, tag=f"attn{i}", name=f"attn{i}")
                         for i in range(4)]
            den_pool = [p3sb.tile([P, 1], fp32, tag=f"den{i}", name=f"den{i}")
                        for i in range(4)]
            rec_pool = [p3sb.tile([P, 1], fp32, tag=f"rec{i}", name=f"rec{i}")
                        for i in range(4)]
            ath_pool = [p3sb.tile([P, NQT, S], bf16, tag=f"ath{i}", name=f"ath{i}")
                        for i in range(2)]

            def issue_scores(h):
                hb, ho = (h * DH) // P, (h * DH) % P
                for qt in range(NQT):
                    slot = (h * NQT + qt) % 4
                    sc = sc_pool[slot]
                    # psum <- bias, then += Qt^T K (Qt pre-scaled by 1/8)
                    nc.tensor.matmul(
                        sc, ident8, biasq[:, qt, h, :],
                        start=True, stop=False,
                    )
                    nc.tensor.matmul(
                        sc,
                        QtT[ho : ho + DH, hb, ts(qt, P)],
                        KtT[ho : ho + DH, hb, :],
                        start=False, stop=True,
                    )
                    nc.scalar.activation(
                        att_pool[slot], sc, AF.Exp, accum_out=den_pool[slot]
                    )
                    nc.vector.reciprocal(out=rec_pool[slot], in_=den_pool[slot])
                    nc.vector.tensor_scalar_mul(
                        attn_pool[slot], att_pool[slot], rec_pool[slot][:, 0:1]
                    )

            def issue_transp(h):
                ath = ath_pool[h % 2]
                for qt in range(NQT):
                    slot = (h * NQT + qt) % 4
                    atu = atu_pool[qt % 2]
                    for kt in range(NQT):
                        nc.tensor.transpose(
                            atu[:, kt, :], attn_pool[slot][:, ts(kt, P)], ident_b
                        )
                    nc.vector.tensor_copy(ath[:, :, ts(qt, P)], atu)

            def issue_ctx(h):
                ath = ath_pool[h % 2]
                cx = cx_pool[(h // 2) % 2]
                prange = cx[(h % 2) * DH : (h % 2) * DH + DH, :]
                for kt in range(NQT):
                    nc.tensor.matmul(
                        prange,
                        Vsb[:, kt, h * DH : (h + 1) * DH],
                        ath[:, kt, :],
                        start=(kt == 0), stop=(kt == NQT - 1),
                    )
                if h % 2 == 1:
                    nc.vector.tensor_copy(ctxT[:, h // 2, :], cx)

            # software pipeline: transp(h-1) | scores(h) | ctx(h-1)
            issue_scores(0)
            for h in range(1, H):
                issue_transp(h - 1)
                issue_scores(h)
                issue_ctx(h - 1)
            issue_transp(H - 1)
            issue_ctx(H - 1)

        if debug:
            dbgc = const.tile([P, ND, S], fp32, name="dbgc")
            nc.vector.tensor_copy(dbgc.rearrange("p a b -> p (a b)"),
                                  ctxT.rearrange("p a b -> p (a b)"))
            nc.sync.dma_start(out=dbg_ctx_e[:, :, :], in_=dbgc)

        # free attention residents before FFN
        attp_cm.__exit__(None, None, None)

        ffp_cm = tc.tile_pool(name="ffp", bufs=1)
        ffp = ffp_cm.__enter__()
        xln = ffp.tile([P, NQT, D], fp32)
        xlnb = ffp.tile([P, NQT, D], bf16)
        xlnT = ffp.tile([P, ND, S], bf16)
        y1T = ffp.tile([P, NFF, S], bf16)
        # LN param broadcast rows -> [P, D]
        g1b = ffp.tile([P, D], fp32)
        b1b = ffp.tile([P, D], fp32)
        g2b = ffp.tile([P, D], fp32)
        b2b = ffp.tile([P, D], fp32)
        for dst, src_e in ((g1b, g1r_e), (b1b, b1r_e), (g2b, g2r_e), (b2b, b2r_e)):
            row = ffp.tile([1, D], fp32, tag="lnrow", name="lnrow")
            nc.sync.dma_start(out=row, in_=src_e[:, :])
            nc.gpsimd.partition_broadcast(dst, row[0:1, :])

        # =========== Phase 4: merge + residual + LN1 (+ transpose) ===========
        tc.no_sync_barrier()
        with (
            tc.tile_pool(name="p4sb", bufs=2) as p4sb,
            tc.tile_pool(name="p4w", bufs=1) as p4w,
            tc.tile_pool(name="p4ps", bufs=2, space="PSUM") as p4ps,
            tc.tile_pool(name="p4tp", bufs=2, space="PSUM") as p4tp,
        ):
            wmres = p4w.tile([P, ND, D], bf16)
            nc.sync.dma_start(out=wmres, in_=wmTb_e.rearrange("(n p) d -> p n d", p=P))
            for st in range(NQT):
                qtile = p4sb.tile([P, D], fp32, tag="qtile")
                nc.sync.dma_start(out=qtile, in_=qbm_e[st * P : (st + 1) * P, :])
                x1 = p4sb.tile([P, D], fp32, tag="x1")
                for oh in range(2):
                    ps = p4ps.tile([P, S], fp32, tag="mps")
                    for dblk in range(ND):
                        nc.tensor.matmul(
                            ps,
                            ctxT[:, dblk, ts(st, P)],
                            wmres[:, dblk, ts(oh, S)],
                            start=(dblk == 0),
                            stop=(dblk == ND - 1),
                        )
                    nc.vector.tensor_tensor(
                        out=x1[:, ts(oh, S)], in0=ps,
                        in1=qtile[:, ts(oh, S)], op=ALU.add,
                    )
                _layernorm(nc, p4sb, xln[:, st, :], x1, g1b, b1b, eps_ln_c)
                nc.scalar.activation(xlnb[:, st, :], xln[:, st, :], AF.Copy)
                for dblk in range(ND):
                    tp = p4tp.tile([P, P], bf16, tag="tp")
                    nc.tensor.transpose(
                        tp, xlnb[:, st, ts(dblk, P)], ident_b
                    )
                    nc.vector.tensor_copy(xlnT[:, dblk, ts(st, P)], tp)

        if debug:
            nc.sync.dma_start(out=dbg_xln_e[:, :, :], in_=xln)

        # =========== Phase 5: FFN1 + relu ===========
        with (
            tc.tile_pool(name="p5w", bufs=2) as p5w,
            tc.tile_pool(name="p5ps", bufs=2, space="PSUM") as p5ps,
        ):
            NGRP = 4
            FPG = NFF // NGRP  # 8 ff-blocks per group
            for grp in range(NGRP):
                wf1g = p5w.tile([P, ND, FPG * P], bf16, tag="wf1g")
                nc.sync.dma_start(
                    out=wf1g,
                    in_=wf1Tb_e[:, grp * FPG * P : (grp + 1) * FPG * P].rearrange(
                        "(n p) f -> p n f", p=P
                    ),
                )
                for fl in range(FPG):
                    ffb = grp * FPG + fl
                    ps = p5ps.tile([P, S], fp32, tag="fps")
                    for dblk in range(ND):
                        nc.tensor.matmul(
                            ps,
                            wf1g[:, dblk, ts(fl, P)],
                            xlnT[:, dblk, :],
                            start=(dblk == 0), stop=(dblk == ND - 1),
                        )
                    nc.scalar.activation(
                        y1T[:, ffb, :], ps, AF.Relu,
                        bias=bf1c[:, ffb : ffb + 1], scale=1.0,
                    )

        # =========== Phase 6: FFN2 + residual + LN2 + out ===========
        with (
            tc.tile_pool(name="p7sb", bufs=2) as p7sb,
            tc.tile_pool(name="p7w", bufs=2) as p7w,
            tc.tile_pool(name="p7ps", bufs=1, space="PSUM") as p7ps,
        ):
            fps2 = [
                [p7ps.tile([P, S], fp32, tag=f"f2{st * 2 + oh}", name=f"f2{st}{oh}")
                 for oh in range(2)]
                for st in range(NQT)
            ]
            for st in range(NQT):
                for oh in range(2):
                    nc.tensor.matmul(
                        fps2[st][oh], onesb[:, 0:P], bf2r[:, ts(oh, S)],
                        start=True, stop=False,
                    )
            for ffb in range(NFF):
                wch = p7w.tile([P, D], bf16, tag="wch")
                nc.sync.dma_start(out=wch, in_=wf2Tb_e[ffb * P : (ffb + 1) * P, :])
                for st in range(NQT):
                    for oh in range(2):
                        nc.tensor.matmul(
                            fps2[st][oh],
                            y1T[:, ffb, ts(st, P)],
                            wch[:, ts(oh, S)],
                            start=False,
                            stop=(ffb == NFF - 1),
                        )
            for st in range(NQT):
                x2 = p7sb.tile([P, D], fp32, tag="x2")
                for oh in range(2):
                    nc.vector.tensor_tensor(
                        out=x2[:, ts(oh, S)], in0=fps2[st][oh],
                        in1=xln[:, st, ts(oh, S)], op=ALU.add,
                    )
                xout = p7sb.tile([P, D], fp32, tag="xout")
                _layernorm(nc, p7sb, xout, x2, g2b, b2b, eps_ln_c)
                nc.sync.dma_start(out=out_e[st * P : (st + 1) * P, :], in_=xout)

        ffp_cm.__exit__(None, None, None)
        dram_cm.__exit__(None, None, None)
        midp_cm.__exit__(None, None, None)
        const_cm.__exit__(None, None, None)

    nc.finalize()
    return nc


def _prep_inputs(q, kv, attn_map, Wq, bq, Wk, bk, Wv, bv, Wm, bm,
                 Wc1, bc1, Wc2, bc2, Wf1, bf1, Wf2, bf2, g1, b1, g2, b2):
    """Host-side packing. Returns (shared dict, per-core list of dicts)."""
    f32 = np.float32
    bf = ml_dtypes.bfloat16

    def c(a):
        return np.ascontiguousarray(np.asarray(a), dtype=f32)

    def cb(a):
        return np.ascontiguousarray(np.asarray(a, dtype=f32)).astype(bf)

    Wq, Wk, Wv, Wm = c(Wq), c(Wk), c(Wv), c(Wm)
    Wc1, Wc2 = c(Wc1), c(Wc2)
    bq, bk, bv, bm = c(bq), c(bk), c(bv), c(bm)
    bc1, bc2, bf1, bf2 = c(bc1), c(bc2), c(bf1), c(bf2)
    g1, b1, g2, b2 = c(g1), c(b1), c(g2), c(b2)

    shared = {
        "wqTb": cb(Wq.T), "wkTb": cb(Wk.T), "wvTb": cb(Wv.T), "wmTb": cb(Wm.T),
        "wf1Tb": cb(np.asarray(Wf1).T),
        "wf2Tb": cb(np.asarray(Wf2).T),
        "bqc": c((bq / 8.0).reshape(ND, P).T),
        "bkc": c(bk.reshape(ND, P).T),
        "bf1c": c(bf1.reshape(NFF, P).T),
        "bvr": cb(bv.reshape(1, D)),
        "bf2r": cb(bf2.reshape(1, D)),
        "onesb": np.ones((1, S), bf),
        "ident8": np.eye(P, dtype=ml_dtypes.float8_e4m3),
        "g1r": g1.reshape(1, D), "b1r": b1.reshape(1, D),
        "g2r": g2.reshape(1, D), "b2r": b2.reshape(1, D),
    }
    # conv block-diag lhsT [K, M]: out[(g,oh)] = sum_c lhsT[(g,c),(g,oh)] rhs[(g,c)]
    c1A = np.zeros((P, P), f32)
    c1B = np.zeros((P, P), f32)
    c2A = np.zeros((P, P), f32)
    c2B = np.zeros((P, P), f32)
    for g in range(8):
        sl = slice(g * 16, g * 16 + 16)
        c1A[sl, sl] = Wc1[0:16, :].T     # [c, oh]
        c1B[sl, sl] = Wc1[16:32, :].T
        c2A[sl, sl] = Wc2[:, 0:16].T     # [ci, h]
        c2B[sl, sl] = Wc2[:, 16:32].T
    shared["c1A"] = c1A.astype(bf)
    shared["c1B"] = c1B.astype(bf)
    shared["c2A"] = c2A.astype(bf)
    shared["c2B"] = c2B.astype(bf)
    shared["bc1A"] = np.tile(bc1[0:16], 8).reshape(P, 1).astype(f32)
    shared["bc1B"] = np.tile(bc1[16:32], 8).reshape(P, 1).astype(f32)
    shared["bc2c"] = np.tile(bc2, 8).reshape(P, 1).astype(f32)

    q = c(q)
    kv = c(kv)
    per_core = []
    for b in range(B):
        per_core.append({
            "qTb": cb(q[b].T), "kvTb": cb(kv[b].T),
            "qbm": c(q[b] + bm.reshape(1, D)),
            "amapb": cb(np.asarray(attn_map[b])),
        })
    return shared, per_core


def kernel(**inputs):
    if "nc" not in _CACHED:
        _CACHED["nc"] = build_program()
    nc = _CACHED["nc"]
    shared, per_core = _prep_inputs(**inputs)
    in_maps = [dict(shared, **pc) for pc in per_core]
    res = run_bass_kernel_spmd(nc, in_maps, list(range(B)))
    out = np.stack([res.results[i]["out"] for i in range(B)], axis=0)
    return out.astype(np.float32)


# revision 36
# speedup vs baseline: 1.6801x; 1.0908x over previous
"""Trainium2 Bass kernel for nn_CrossAttention (dense transformer block).

Sharding: data-parallel over batch - 8 batch elements, one per NeuronCore.
Each core runs the full block for its batch element:
  bias = Conv1x1(gelu(Conv1x1(log(attn_map[1:,1:] + eps))))
  MHA(q, kv) with bias added to scores; residual + LN; FFN; residual + LN.

Perf structure (v2):
  - all matmuls bf16 (moving+stationary) with fp32 PSUM accumulation
  - strict phase order so the scalar engine loads each activation table once
    (ln -> gelu -> exp -> rsqrt)
  - conv bias kept SBUF-resident in q-major layout via SBUF->SBUF DMA shuffle
  - score bias added by PSUM-init matmul (identity x bias) instead of vector add
  - software-pipelined attention (head h+1 scores issued before head h ctx)
  - bulk DMA (attn_map loads + bias shuffle) on the idle gpsimd queue
"""

import numpy as np
import ml_dtypes

import concourse.bass as bass
import concourse.mybir as mybir
import concourse.tile as tile
from concourse import bacc
from concourse.bass import ts
from concourse.bass_utils import run_bass_kernel_spmd
from concourse.masks import make_identity

AF = mybir.ActivationFunctionType
ALU = mybir.AluOpType

B, S, D, H, DH, FF = 8, 512, 1024, 16, 64, 4096
CH, CHID = 16, 32
EPS_LOG = 1e-6
EPS_LN = 1e-6
P = 128
NQT = S // P          # 4 q-tiles
ND = D // P           # 8 d-blocks
NFF = FF // P         # 32 ff-blocks
AM = 513              # attn_map edge
NQI = 4               # q rows per partition-group in conv
NHALF = S // 32       # 16 conv halves (32 q rows each)

fp32 = mybir.dt.float32
bf16 = mybir.dt.bfloat16
fp8e4 = mybir.dt.float8e4

_CACHED = {}


def _layernorm(nc, pool, out_ap, x_ap, gb, bb, eps_c):
    """out = (x - mean(x)) * rsqrt(var(x) + eps) * g + b over free dim (D)."""
    nsub = D // 512
    stats = pool.tile([P, nsub, nc.vector.BN_STATS_DIM], fp32, tag="ln_stats")
    for i in range(nsub):
        nc.vector.bn_stats(out=stats[:, i, :], in_=x_ap[:, ts(i, 512)])
    mv = pool.tile([P, nc.vector.BN_AGGR_DIM], fp32, tag="ln_mv")
    nc.vector.bn_aggr(out=mv, in_=stats)
    rstd = pool.tile([P, 1], fp32, tag="ln_rstd")
    nc.scalar.activation(rstd, mv[:, 1:2], AF.Sqrt, bias=eps_c, scale=1.0)
    nc.vector.reciprocal(out=rstd, in_=rstd)
    u = pool.tile([P, D], fp32, tag="ln_u")
    nc.vector.scalar_tensor_tensor(
        out=u, in0=x_ap, scalar=mv[:, 0:1], in1=gb,
        op0=ALU.subtract, op1=ALU.mult,
    )
    nc.vector.scalar_tensor_tensor(
        out=out_ap, in0=u, scalar=rstd[:, 0:1], in1=bb,
        op0=ALU.mult, op1=ALU.add,
    )


def build_program(debug=False):
    nc = bacc.Bacc(None)

    # ---------------- DRAM I/O ----------------
    qTb_e = nc.dram_tensor("qTb", [D, S], bf16, kind="ExternalInput")
    kvTb_e = nc.dram_tensor("kvTb", [D, S], bf16, kind="ExternalInput")
    qbm_e = nc.dram_tensor("qbm", [S, D], bf16, kind="ExternalInput")  # q + bm
    amapb_e = nc.dram_tensor("amapb", [CH, AM, AM], bf16, kind="ExternalInput")
    wqTb_e = nc.dram_tensor("wqTb", [D, D], bf16, kind="ExternalInput")
    wkTb_e = nc.dram_tensor("wkTb", [D, D], bf16, kind="ExternalInput")
    wvTb_e = nc.dram_tensor("wvTb", [D, D], bf16, kind="ExternalInput")
    wmTb_e = nc.dram_tensor("wmTb", [D, D], bf16, kind="ExternalInput")
    wf1Tb_e = nc.dram_tensor("wf1Tb", [D, FF], bf16, kind="ExternalInput")
    wf2Tb_e = nc.dram_tensor("wf2Tb", [FF, D], bf16, kind="ExternalInput")
    c1A_e = nc.dram_tensor("c1A", [P, P], bf16, kind="ExternalInput")
    c1B_e = nc.dram_tensor("c1B", [P, P], bf16, kind="ExternalInput")
    c2A_e = nc.dram_tensor("c2A", [P, P], bf16, kind="ExternalInput")
    c2B_e = nc.dram_tensor("c2B", [P, P], bf16, kind="ExternalInput")
    # per-partition bias columns
    bqc_e = nc.dram_tensor("bqc", [P, ND], fp32, kind="ExternalInput")   # bq/8
    bkc_e = nc.dram_tensor("bkc", [P, ND], fp32, kind="ExternalInput")
    bc1A_e = nc.dram_tensor("bc1A", [P, 1], fp32, kind="ExternalInput")
    bc1B_e = nc.dram_tensor("bc1B", [P, 1], fp32, kind="ExternalInput")
    bc2c_e = nc.dram_tensor("bc2c", [P, 1], fp32, kind="ExternalInput")
    bf1c_e = nc.dram_tensor("bf1c", [P, NFF], fp32, kind="ExternalInput")
    # bias rows (K=1 matmul trick)
    bvr_e = nc.dram_tensor("bvr", [1, D], bf16, kind="ExternalInput")
    bf2r_e = nc.dram_tensor("bf2r", [1, D], bf16, kind="ExternalInput")
    onesb_e = nc.dram_tensor("onesb", [1, S], bf16, kind="ExternalInput")
    ident8_e = nc.dram_tensor("ident8", [P, P], fp8e4, kind="ExternalInput")
    # LN params as rows
    g1r_e = nc.dram_tensor("g1r", [1, D], fp32, kind="ExternalInput")
    b1r_e = nc.dram_tensor("b1r", [1, D], fp32, kind="ExternalInput")
    g2r_e = nc.dram_tensor("g2r", [1, D], fp32, kind="ExternalInput")
    b2r_e = nc.dram_tensor("b2r", [1, D], fp32, kind="ExternalInput")

    out_e = nc.dram_tensor("out", [S, D], fp32, kind="ExternalOutput")
    if debug:
        dbg_qt_e = nc.dram_tensor("dbg_qt", [P, ND, S], fp32, kind="ExternalOutput")
        dbg_kt_e = nc.dram_tensor("dbg_kt", [P, ND, S], fp32, kind="ExternalOutput")
        dbg_v_e = nc.dram_tensor("dbg_v", [P, NQT, D], fp32, kind="ExternalOutput")
        dbg_bias_e = nc.dram_tensor("dbg_bias", [P, H, S], fp32, kind="ExternalOutput")
        dbg_ctx_e = nc.dram_tensor("dbg_ctx", [P, ND, S], fp32, kind="ExternalOutput")
        dbg_xln_e = nc.dram_tensor("dbg_xln", [P, NQT, D], fp32, kind="ExternalOutput")

    with tile.TileContext(nc) as tc:
        # ------------- persistent pools -------------
        const_cm = tc.tile_pool(name="const", bufs=1)
        const = const_cm.__enter__()

        ident_b = const.tile([P, P], bf16)
        make_identity(nc, ident_b)
        ident8 = const.tile([P, P], fp8e4)
        nc.sync.dma_start(out=ident8, in_=ident8_e[:, :])

        eps_log_c = const.tile([P, 1], fp32)
        nc.vector.memset(eps_log_c, EPS_LOG)
        eps_ln_c = const.tile([P, 1], fp32)
        nc.vector.memset(eps_ln_c, EPS_LN)

        c1A = const.tile([P, P], bf16)
        c1B = const.tile([P, P], bf16)
        c2A = const.tile([P, P], bf16)
        c2B = const.tile([P, P], bf16)
        nc.sync.dma_start(out=c1A, in_=c1A_e[:, :])
        nc.sync.dma_start(out=c1B, in_=c1B_e[:, :])
        nc.sync.dma_start(out=c2A, in_=c2A_e[:, :])
        nc.sync.dma_start(out=c2B, in_=c2B_e[:, :])
        bc1A = const.tile([P, 1], fp32)
        bc1B = const.tile([P, 1], fp32)
        bc2c = const.tile([P, 1], fp32)
        nc.sync.dma_start(out=bc1A, in_=bc1A_e[:, :])
        nc.sync.dma_start(out=bc1B, in_=bc1B_e[:, :])
        nc.sync.dma_start(out=bc2c, in_=bc2c_e[:, :])
        bqc = const.tile([P, ND], fp32)
        bkc = const.tile([P, ND], fp32)
        bf1c = const.tile([P, NFF], fp32)
        nc.sync.dma_start(out=bqc, in_=bqc_e[:, :])
        nc.sync.dma_start(out=bkc, in_=bkc_e[:, :])
        nc.sync.dma_start(out=bf1c, in_=bf1c_e[:, :])
        bvr = const.tile([1, D], bf16)
        bf2r = const.tile([1, D], bf16)
        onesb = const.tile([1, S], bf16)
        nc.sync.dma_start(out=bvr, in_=bvr_e[:, :])
        nc.sync.dma_start(out=bf2r, in_=bf2r_e[:, :])
        nc.sync.dma_start(out=onesb, in_=onesb_e[:, :])

        # ctxT outlives attp (merge reads it); entered first for stack order
        midp_cm = tc.tile_pool(name="midp", bufs=1)
        midp = midp_cm.__enter__()
        ctxT = midp.tile([P, ND, S], bf16)     # [(h,dh)-part, blk, q]
        wmres = midp.tile([P, ND, D], bf16)    # merge weights (prefetched)
        qbm = midp.tile([P, NQT, D], bf16)     # residual q + bm (prefetched)
        dram_cm = tc.tile_pool(name="dstage", bufs=1, space="DRAM")
        dram = dram_cm.__enter__()
        bstage = dram.tile([S, H, S], fp8e4)

        # ========== attention-lifetime pool ==========
        attp_cm = tc.tile_pool(name="attp", bufs=1)
        attp = attp_cm.__enter__()
        QtT = attp.tile([P, ND, S], bf16)      # [o-part, o-blk, s]  ((Wq x + bq)/8)
        KtT = attp.tile([P, ND, S], bf16)
        Vsb = attp.tile([P, NQT, D], bf16)     # [k-part, k-blk, (h dh)]
        biasq = attp.tile([P, NQT, H, S], fp8e4)  # [q-part, qt, h, k]

        # =========== Phase 1+2: projections + conv bias ===========
        NQI2 = 8          # q rows per partition-group per conv block
        NH2 = S // (8 * NQI2)  # 8 conv blocks of 64 q rows
        with (
            tc.tile_pool(name="p2sb", bufs=2) as p2sb,
            tc.tile_pool(name="p2ps", bufs=1, space="PSUM") as p2ps,
        ):
            with (
                tc.tile_pool(name="p1x", bufs=1) as p1x,
                tc.tile_pool(name="p1w", bufs=2) as p1w,
                tc.tile_pool(name="p1ps", bufs=2, space="PSUM") as p1ps,
            ):
                qTb = p1x.tile([P, ND, S], bf16)
                nc.sync.dma_start(
                    out=qTb, in_=qTb_e.rearrange("(n p) s -> p n s", p=P)
                )
                kvTb = p1x.tile([P, ND, S], bf16)
                nc.sync.dma_start(
                    out=kvTb, in_=kvTb_e.rearrange("(n p) s -> p n s", p=P)
                )

                # Q/K: out[o, s] += w[d-blk, o].T @ xT[d-blk, s]
                for wsrc, xsb, dst, bcol, scl in (
                    (wqTb_e, qTb, QtT, bqc, 0.125),
                    (wkTb_e, kvTb, KtT, bkc, 1.0),
                ):
                    wres = p1w.tile([P, ND, D], bf16, tag="wres")
                    nc.sync.dma_start(
                        out=wres, in_=wsrc.rearrange("(n p) d -> p n d", p=P)
                    )
                    for ob in range(ND):
                        ps = p1ps.tile([P, S], fp32, tag="pjps")
                        for dblk in range(ND):
                            nc.tensor.matmul(
                                ps,
                                wres[:, dblk, ts(ob, P)],
                                xsb[:, dblk, :],
                                start=(dblk == 0),
                                stop=(dblk == ND - 1),
                            )
                        nc.vector.tensor_scalar(
                            out=dst[:, ob, :], in0=ps,
                            scalar1=scl, scalar2=bcol[:, ob : ob + 1],
                            op0=ALU.mult, op1=ALU.add,
                        )

                # V: out[k, o] += kvT[d-blk, k-tile].T @ wv[d-blk, o-half]
                wvres = p1w.tile([P, ND, D], bf16, tag="wres")
                nc.sync.dma_start(
                    out=wvres, in_=wvTb_e.rearrange("(n p) d -> p n d", p=P)
                )
                for kt in range(NQT):
                    for oh in range(2):
                        ps = p1ps.tile([P, S], fp32, tag="pjps")
                        nc.tensor.matmul(
                            ps, onesb[:, 0:P], bvr[:, ts(oh, S)],
                            start=True, stop=False,
                        )
                        for dblk in range(ND):
                            nc.tensor.matmul(
                                ps,
                                kvTb[:, dblk, ts(kt, P)],
                                wvres[:, dblk, ts(oh, S)],
                                start=False,
                                stop=(dblk == ND - 1),
                            )
                        nc.vector.tensor_copy(Vsb[:, kt, ts(oh, S)], ps)

            # conv in 2 super-blocks: [Ln x4 | fence | conv x4] x2
            # so each stage loads its activation table exactly once.
            # amap(h) is issued right after Ln(h-2) (the last reader of the
            # ring slot it overwrites) so block-2 loads prefetch during
            # block-1 compute.
            amts = {}

            def issue_amap(half):
                qbase = half * 64
                amt = p2sb.tile([P, NQI2, S], bf16, tag="amt", bufs=2)
                for g in range(8):
                    src = bass.AP(
                        tensor=amapb_e,
                        offset=(1 + qbase + NQI2 * g) * AM + 1,
                        ap=[[AM * AM, CH], [AM, NQI2], [1, S]],
                    )
                    nc.sync.dma_start(out=amt[CH * g : CH * (g + 1)], in_=src)
                amts[half] = amt

            issue_amap(0)
            issue_amap(1)
            # prefetch merge-phase tensors while conv/attention run
            nc.sync.dma_start(
                out=wmres, in_=wmTb_e.rearrange("(n p) d -> p n d", p=P)
            )
            nc.sync.dma_start(out=qbm, in_=qbm_e.rearrange("(n p) d -> p n d", p=P))

            HPB = NH2 // 2
            for sblk in range(2):
                logms = []
                for half in range(sblk * HPB, (sblk + 1) * HPB):
                    amt = amts[half]
                    logm = p2sb.tile([P, NQI2 * S], bf16, tag="logm", bufs=4)
                    amtf = amt.rearrange("p a b -> p (a b)")
                    for i in range(2):
                        nc.scalar.activation(
                            logm[:, ts(i, 4 * S)], amtf[:, ts(i, 4 * S)], AF.Ln,
                            bias=eps_log_c, scale=1.0,
                        )
                    logms.append(logm)
                    if half + 2 < NH2:
                        issue_amap(half + 2)

                tc.no_sync_barrier()

                for hi, half in enumerate(range(sblk * HPB, (sblk + 1) * HPB)):
                    qbase = half * 64
                    logm = logms[hi]
                    c2sb = p2sb.tile([P, NQI2, S], fp8e4, tag="c2sb")
                    for j in range(4):  # four [P, 1024] chunks
                        pA = p2ps.tile([P, 2 * S], fp32, tag="pA")
                        pB = p2ps.tile([P, 2 * S], fp32, tag="pB")
                        for c in range(2):
                            chk = 2 * j + c
                            nc.tensor.matmul(
                                pA[:, ts(c, S)], c1A, logm[:, ts(chk, S)],
                                start=True, stop=True,
                            )
                            nc.tensor.matmul(
                                pB[:, ts(c, S)], c1B, logm[:, ts(chk, S)],
                                start=True, stop=True,
                            )
                        gA = p2sb.tile([P, 2 * S], bf16, tag="gA", bufs=1)
                        gB = p2sb.tile([P, 2 * S], bf16, tag="gB", bufs=1)
                        nc.scalar.activation(gA, pA, AF.Gelu, bias=bc1A, scale=1.0)
                        nc.scalar.activation(gB, pB, AF.Gelu, bias=bc1B, scale=1.0)
                        pC = p2ps.tile([P, 2 * S], fp32, tag="pC")
                        for c in range(2):
                            nc.tensor.matmul(
                                pC[:, ts(c, S)], c2A, gA[:, ts(c, S)],
                                start=True, stop=False,
                            )
                            nc.tensor.matmul(
                                pC[:, ts(c, S)], c2B, gB[:, ts(c, S)],
                                start=False, stop=True,
                            )
                        nc.vector.tensor_scalar(
                            out=c2sb[:, 2 * j : 2 * j + 2, :].rearrange(
                                "p a b -> p (a b)"
                            ),
                            in0=pC, scalar1=bc2c[:, 0:1], scalar2=None,
                            op0=ALU.add,
                        )
                    # stage to DRAM in [q, h, k] order
                    for g in range(8):
                        q0 = qbase + NQI2 * g
                        nc.sync.dma_start(
                            out=bstage[q0 : q0 + NQI2].rearrange(
                                "q h k -> h q k"
                            ),
                            in_=c2sb[CH * g : CH * (g + 1)],
                        )
                    # read back q-major as soon as a qtile's rows are staged
                    if half % 2 == 1:
                        qt = half // 2
                        nc.sync.dma_start(
                            out=biasq[:, qt],
                            in_=bstage[qt * P : (qt + 1) * P],
                        )
                if sblk == 0:
                    tc.no_sync_barrier()

        if debug:
            dbgq = const.tile([P, ND, S], fp32, name="dbgq")
            nc.vector.tensor_copy(dbgq.rearrange("p a b -> p (a b)"),
                                  QtT.rearrange("p a b -> p (a b)"))
            nc.sync.dma_start(out=dbg_qt_e[:, :, :], in_=dbgq)
            nc.vector.tensor_copy(dbgq.rearrange("p a b -> p (a b)"),
                                  KtT.rearrange("p a b -> p (a b)"))
            nc.sync.dma_start(out=dbg_kt_e[:, :, :], in_=dbgq)
            dbgv = const.tile([P, NQT, D], fp32, name="dbgv")
            nc.vector.tensor_copy(dbgv.rearrange("p a b -> p (a b)"),
                                  Vsb.rearrange("p a b -> p (a b)"))
            nc.sync.dma_start(out=dbg_v_e[:, :, :], in_=dbgv)
            dbgb = const.tile([P, H, S], fp32, name="dbgb")
            nc.vector.tensor_copy(dbgb.rearrange("p a b -> p (a b)"),
                                  biasq[:, 0].rearrange("p a b -> p (a b)"))
            nc.sync.dma_start(out=dbg_bias_e[:, :, :], in_=dbgb)

        # =========== Phase 3: attention (one table: Exp) ===========
        tc.no_sync_barrier()
        with (
            tc.tile_pool(name="p3sb", bufs=1) as p3sb,
            tc.tile_pool(name="p3ps", bufs=1, space="PSUM") as p3ps,
        ):
            sc_pool = [p3ps.tile([P, S], fp32, tag=f"sc{i}", name=f"sc{i}")
                       for i in range(4)]
            atu_pool = [p3ps.tile([P, NQT, P], bf16, tag=f"atu{i}", name=f"atu{i}")
                        for i in range(2)]
            cx_pool = [p3ps.tile([P, S], fp32, tag=f"cx{i}", name=f"cx{i}")
                       for i in range(2)]
            att_pool = [p3sb.tile([P, S], bf16, tag=f"att{i}", name=f"att{i}")
                        for i in range(4)]
            attn_pool = [p3sb.tile([P, S], bf16, tag=f"attn{i}", name=f"attn{i}")
                         for i in range(4)]
            den_pool = [p3sb.tile([P, 1], fp32, tag=f"den{i}", name=f"den{i}")
                        for i in range(4)]
            rec_pool = [p3sb.tile([P, 1], fp32, tag=f"rec{i}", name=f"rec{i}")
                        for i in range(4)]
            ath_pool = [p3sb.tile([P, NQT, S], bf16, tag=f"ath{i}", name=f"ath{i}")
                        for i in range(2)]

            def issue_scores(h):
                hb, ho = (h * DH) // P, (h * DH) % P
                for qt in range(NQT):
                    slot = (h * NQT + qt) % 4
                    sc = sc_pool[slot]
                    # psum <- bias, then += Qt^T K (Qt pre-scaled by 1/8)
                    nc.tensor.matmul(
                        sc, ident8, biasq[:, qt, h, :],
                        start=True, stop=False,
                    )
                    nc.tensor.matmul(
                        sc,
                        QtT[ho : ho + DH, hb, ts(qt, P)],
                        KtT[ho : ho + DH, hb, :],
                        start=False, stop=True,
                    )
                    nc.scalar.activation(
                        att_pool[slot], sc, AF.Exp, accum_out=den_pool[slot]
                    )
                    nc.vector.reciprocal(out=rec_pool[slot], in_=den_pool[slot])
                    nc.vector.tensor_scalar_mul(
                        attn_pool[slot], att_pool[slot], rec_pool[slot][:, 0:1]
                    )

            def issue_transp(h):
                ath = ath_pool[h % 2]
                for qt in range(NQT):
                    slot = (h * NQT + qt) % 4
                    atu = atu_pool[qt % 2]
                    for kt in range(NQT):
                        nc.tensor.transpose(
                            atu[:, kt, :], attn_pool[slot][:, ts(kt, P)], ident_b
                        )
                    nc.vector.tensor_copy(ath[:, :, ts(qt, P)], atu)

            def issue_ctx(h):
                ath = ath_pool[h % 2]
                cx = cx_pool[(h // 2) % 2]
                prange = cx[(h % 2) * DH : (h % 2) * DH + DH, :]
                for kt in range(NQT):
                    nc.tensor.matmul(
                        prange,
                        Vsb[:, kt, h * DH : (h + 1) * DH],
                        ath[:, kt, :],
                        start=(kt == 0), stop=(kt == NQT - 1),
                    )
                if h % 2 == 1:
                    nc.vector.tensor_copy(ctxT[:, h // 2, :], cx)

            # software pipeline: transp(h-1) | scores(h) | ctx(h-1)
            issue_scores(0)
            for h in range(1, H):
                issue_transp(h - 1)
                issue_scores(h)
                issue_ctx(h - 1)
            issue_transp(H - 1)
            issue_ctx(H - 1)

        if debug:
            dbgc = const.tile([P, ND, S], fp32, name="dbgc")
            nc.vector.tensor_copy(dbgc.rearrange("p a b -> p (a b)"),
                                  ctxT.rearrange("p a b -> p (a b)"))
            nc.sync.dma_start(out=dbg_ctx_e[:, :, :], in_=dbgc)

        # free attention residents before FFN
        attp_cm.__exit__(None, None, None)

        ffp_cm = tc.tile_pool(name="ffp", bufs=1)
        ffp = ffp_cm.__enter__()
        xln = ffp.tile([P, NQT, D], fp32)
        xlnb = ffp.tile([P, NQT, D], bf16)
        xlnT = ffp.tile([P, ND, S], bf16)
        y1T = ffp.tile([P, NFF, S], bf16)
        # LN param broadcast rows -> [P, D]
        g1b = ffp.tile([P, D], fp32)
        b1b = ffp.tile([P, D], fp32)
        g2b = ffp.tile([P, D], fp32)
        b2b = ffp.tile([P, D], fp32)
        for dst, src_e in ((g1b, g1r_e), (b1b, b1r_e), (g2b, g2r_e), (b2b, b2r_e)):
            row = ffp.tile([1, D], fp32, tag="lnrow", name="lnrow")
            nc.sync.dma_start(out=row, in_=src_e[:, :])
            nc.gpsimd.partition_broadcast(dst, row[0:1, :])

        # =========== Phase 4: merge + residual + LN1 (+ transpose) ===========
        tc.no_sync_barrier()
        with (
            tc.tile_pool(name="p4sb", bufs=2) as p4sb,
            tc.tile_pool(name="p4ps", bufs=2, space="PSUM") as p4ps,
            tc.tile_pool(name="p4tp", bufs=2, space="PSUM") as p4tp,
        ):
            for st in range(NQT):
                qtile = qbm[:, st, :]
                x1 = p4sb.tile([P, D], fp32, tag="x1")
                for oh in range(2):
                    ps = p4ps.tile([P, S], fp32, tag="mps")
                    for dblk in range(ND):
                        nc.tensor.matmul(
                            ps,
                            ctxT[:, dblk, ts(st, P)],
                            wmres[:, dblk, ts(oh, S)],
                            start=(dblk == 0),
                            stop=(dblk == ND - 1),
                        )
                    nc.vector.tensor_tensor(
                        out=x1[:, ts(oh, S)], in0=ps,
                        in1=qtile[:, ts(oh, S)], op=ALU.add,
                    )
                _layernorm(nc, p4sb, xln[:, st, :], x1, g1b, b1b, eps_ln_c)
                nc.scalar.activation(xlnb[:, st, :], xln[:, st, :], AF.Copy)
                for dblk in range(ND):
                    tp = p4tp.tile([P, P], bf16, tag="tp")
                    nc.tensor.transpose(
                        tp, xlnb[:, st, ts(dblk, P)], ident_b
                    )
                    nc.vector.tensor_copy(xlnT[:, dblk, ts(st, P)], tp)

        if debug:
            nc.sync.dma_start(out=dbg_xln_e[:, :, :], in_=xln)

        # =========== Phase 5: FFN1 + relu ===========
        with (
            tc.tile_pool(name="p5w", bufs=2) as p5w,
            tc.tile_pool(name="p5ps", bufs=2, space="PSUM") as p5ps,
        ):
            NGRP = 4
            FPG = NFF // NGRP  # 8 ff-blocks per group
            for grp in range(NGRP):
                wf1g = p5w.tile([P, ND, FPG * P], bf16, tag="wf1g")
                nc.sync.dma_start(
                    out=wf1g,
                    in_=wf1Tb_e[:, grp * FPG * P : (grp + 1) * FPG * P].rearrange(
                        "(n p) f -> p n f", p=P
                    ),
                )
                for fl in range(FPG):
                    ffb = grp * FPG + fl
                    ps = p5ps.tile([P, S], fp32, tag="fps")
                    for dblk in range(ND):
                        nc.tensor.matmul(
                            ps,
                            wf1g[:, dblk, ts(fl, P)],
                            xlnT[:, dblk, :],
                            start=(dblk == 0), stop=(dblk == ND - 1),
                        )
                    nc.scalar.activation(
                        y1T[:, ffb, :], ps, AF.Relu,
                        bias=bf1c[:, ffb : ffb + 1], scale=1.0,
                    )

        # =========== Phase 6: FFN2 + residual + LN2 + out ===========
        with (
            tc.tile_pool(name="p7sb", bufs=2) as p7sb,
            tc.tile_pool(name="p7w", bufs=2) as p7w,
            tc.tile_pool(name="p7ps", bufs=1, space="PSUM") as p7ps,
        ):
            fps2 = [
                [p7ps.tile([P, S], fp32, tag=f"f2{st * 2 + oh}", name=f"f2{st}{oh}")
                 for oh in range(2)]
                for st in range(NQT)
            ]
            for st in range(NQT):
                for oh in range(2):
                    nc.tensor.matmul(
                        fps2[st][oh], onesb[:, 0:P], bf2r[:, ts(oh, S)],
                        start=True, stop=False,
                    )
            for fpair in range(NFF // 2):
                wch = p7w.tile([P, 2, D], bf16, tag="wch")
                nc.sync.dma_start(
                    out=wch,
                    in_=wf2Tb_e[fpair * 2 * P : (fpair + 1) * 2 * P, :].rearrange(
                        "(a p) d -> p a d", p=P
                    ),
                )
                for a in range(2):
                    ffb = fpair * 2 + a
                    for st in range(NQT):
                        for oh in range(2):
                            nc.tensor.matmul(
                                fps2[st][oh],
                                y1T[:, ffb, ts(st, P)],
                                wch[:, a, ts(oh, S)],
                                start=False,
                                stop=(ffb == NFF - 1),
                            )
            for st in range(NQT):
                x2 = p7sb.tile([P, D], fp32, tag="x2")
                for oh in range(2):
                    nc.vector.tensor_tensor(
                        out=x2[:, ts(oh, S)], in0=fps2[st][oh],
                        in1=xln[:, st, ts(oh, S)], op=ALU.add,
                    )
                xout = p7sb.tile([P, D], fp32, tag="xout")
                _layernorm(nc, p7sb, xout, x2, g2b, b2b, eps_ln_c)
                nc.sync.dma_start(out=out_e[st * P : (st + 1) * P, :], in_=xout)

        ffp_cm.__exit__(None, None, None)
        dram_cm.__exit__(None, None, None)
        midp_cm.__exit__(None, None, None)
        const_cm.__exit__(None, None, None)

    nc.finalize()
    return nc


def _prep_inputs(q, kv, attn_map, Wq, bq, Wk, bk, Wv, bv, Wm, bm,
                 Wc1, bc1, Wc2, bc2, Wf1, bf1, Wf2, bf2, g1, b1, g2, b2):
    """Host-side packing. Returns (shared dict, per-core list of dicts)."""
    f32 = np.float32
    bf = ml_dtypes.bfloat16

    def c(a):
        return np.ascontiguousarray(np.asarray(a), dtype=f32)

    def cb(a):
        return np.ascontiguousarray(np.asarray(a, dtype=f32)).astype(bf)

    Wq, Wk, Wv, Wm = c(Wq), c(Wk), c(Wv), c(Wm)
    Wc1, Wc2 = c(Wc1), c(Wc2)
    bq, bk, bv, bm = c(bq), c(bk), c(bv), c(bm)
    bc1, bc2, bf1, bf2 = c(bc1), c(bc2), c(bf1), c(bf2)
    g1, b1, g2, b2 = c(g1), c(b1), c(g2), c(b2)

    shared = {
        "wqTb": cb(Wq.T), "wkTb": cb(Wk.T), "wvTb": cb(Wv.T), "wmTb": cb(Wm.T),
        "wf1Tb": cb(np.asarray(Wf1).T),
        "wf2Tb": cb(np.asarray(Wf2).T),
        "bqc": c((bq / 8.0).reshape(ND, P).T),
        "bkc": c(bk.reshape(ND, P).T),
        "bf1c": c(bf1.reshape(NFF, P).T),
        "bvr": cb(bv.reshape(1, D)),
        "bf2r": cb(bf2.reshape(1, D)),
        "onesb": np.ones((1, S), bf),
        "ident8": np.eye(P, dtype=ml_dtypes.float8_e4m3),
        "g1r": g1.reshape(1, D), "b1r": b1.reshape(1, D),
        "g2r": g2.reshape(1, D), "b2r": b2.reshape(1, D),
    }
    # conv block-diag lhsT [K, M]: out[(g,oh)] = sum_c lhsT[(g,c),(g,oh)] rhs[(g,c)]
    c1A = np.zeros((P, P), f32)
    c1B = np.zeros((P, P), f32)
    c2A = np.zeros((P, P), f32)
    c2B = np.zeros((P, P), f32)
    for g in range(8):
        sl = slice(g * 16, g * 16 + 16)
        c1A[sl, sl] = Wc1[0:16, :].T     # [c, oh]
        c1B[sl, sl] = Wc1[16:32, :].T
        c2A[sl, sl] = Wc2[:, 0:16].T     # [ci, h]
        c2B[sl, sl] = Wc2[:, 16:32].T
    shared["c1A"] = c1A.astype(bf)
    shared["c1B"] = c1B.astype(bf)
    shared["c2A"] = c2A.astype(bf)
    shared["c2B"] = c2B.astype(bf)
    shared["bc1A"] = np.tile(bc1[0:16], 8).reshape(P, 1).astype(f32)
    shared["bc1B"] = np.tile(bc1[16:32], 8).reshape(P, 1).astype(f32)
    shared["bc2c"] = np.tile(bc2, 8).reshape(P, 1).astype(f32)

    q = c(q)
    kv = c(kv)
    per_core = []
    for b in range(B):
        per_core.append({
            "qTb": cb(q[b].T), "kvTb": cb(kv[b].T),
            "qbm": cb(q[b] + bm.reshape(1, D)),
            "amapb": cb(np.asarray(attn_map[b])),
        })
    return shared, per_core


def kernel(**inputs):
    if "nc" not in _CACHED:
        _CACHED["nc"] = build_program()
    nc = _CACHED["nc"]
    shared, per_core = _prep_inputs(**inputs)
    in_maps = [dict(shared, **pc) for pc in per_core]
    res = run_bass_kernel_spmd(nc, in_maps, list(range(B)))
    out = np.stack([res.results[i]["out"] for i in range(B)], axis=0)
    return out.astype(np.float32)


# revision 40
# speedup vs baseline: 1.7671x; 1.0518x over previous
"""Trainium2 Bass kernel for nn_CrossAttention (dense transformer block).

Sharding: data-parallel over batch - 8 batch elements, one per NeuronCore.
Each core runs the full block for its batch element:
  bias = Conv1x1(gelu(Conv1x1(log(attn_map[1:,1:] + eps))))
  MHA(q, kv) with bias added to scores; residual + LN; FFN; residual + LN.

Perf structure (v2):
  - all matmuls bf16 (moving+stationary) with fp32 PSUM accumulation
  - strict phase order so the scalar engine loads each activation table once
    (ln -> gelu -> exp -> rsqrt)
  - conv bias kept SBUF-resident in q-major layout via SBUF->SBUF DMA shuffle
  - score bias added by PSUM-init matmul (identity x bias) instead of vector add
  - software-pipelined attention (head h+1 scores issued before head h ctx)
  - bulk DMA (attn_map loads + bias shuffle) on the idle gpsimd queue
"""

import numpy as np
import ml_dtypes

import concourse.bass as bass
import concourse.mybir as mybir
import concourse.tile as tile
from concourse import bacc
from concourse.bass import ts
from concourse.bass_utils import run_bass_kernel_spmd
from concourse.masks import make_identity

AF = mybir.ActivationFunctionType
ALU = mybir.AluOpType

B, S, D, H, DH, FF = 8, 512, 1024, 16, 64, 4096
CH, CHID = 16, 32
EPS_LOG = 1e-6
EPS_LN = 1e-6
P = 128
NQT = S // P          # 4 q-tiles
ND = D // P           # 8 d-blocks
NFF = FF // P         # 32 ff-blocks
AM = 513              # attn_map edge
NQI = 4               # q rows per partition-group in conv
NHALF = S // 32       # 16 conv halves (32 q rows each)

fp32 = mybir.dt.float32
bf16 = mybir.dt.bfloat16
fp8e4 = mybir.dt.float8e4

_CACHED = {}


def _layernorm(nc, pool, out_ap, x_ap, gb, bb, eps_c):
    """out = (x - mean(x)) * rsqrt(var(x) + eps) * g + b over free dim (D)."""
    nsub = D // 512
    stats = pool.tile([P, nsub, nc.vector.BN_STATS_DIM], fp32, tag="ln_stats")
    for i in range(nsub):
        nc.vector.bn_stats(out=stats[:, i, :], in_=x_ap[:, ts(i, 512)])
    mv = pool.tile([P, nc.vector.BN_AGGR_DIM], fp32, tag="ln_mv")
    nc.vector.bn_aggr(out=mv, in_=stats)
    rstd = pool.tile([P, 1], fp32, tag="ln_rstd")
    nc.scalar.activation(rstd, mv[:, 1:2], AF.Sqrt, bias=eps_c, scale=1.0)
    nc.vector.reciprocal(out=rstd, in_=rstd)
    u = pool.tile([P, D], fp32, tag="ln_u")
    nc.vector.scalar_tensor_tensor(
        out=u, in0=x_ap, scalar=mv[:, 0:1], in1=gb,
        op0=ALU.subtract, op1=ALU.mult,
    )
    nc.vector.scalar_tensor_tensor(
        out=out_ap, in0=u, scalar=rstd[:, 0:1], in1=bb,
        op0=ALU.mult, op1=ALU.add,
    )


def build_program(debug=False):
    nc = bacc.Bacc(None)

    # ---------------- DRAM I/O ----------------
    qTb_e = nc.dram_tensor("qTb", [D, S], bf16, kind="ExternalInput")
    kvTb_e = nc.dram_tensor("kvTb", [D, S], bf16, kind="ExternalInput")
    qbm_e = nc.dram_tensor("qbm", [S, D], bf16, kind="ExternalInput")  # q + bm
    amapb_e = nc.dram_tensor("amapb", [CH, AM, AM], fp8e4, kind="ExternalInput")
    wqTb_e = nc.dram_tensor("wqTb", [D, D], bf16, kind="ExternalInput")
    wkTb_e = nc.dram_tensor("wkTb", [D, D], bf16, kind="ExternalInput")
    wvTb_e = nc.dram_tensor("wvTb", [D, D], bf16, kind="ExternalInput")
    wmTb_e = nc.dram_tensor("wmTb", [D, D], bf16, kind="ExternalInput")
    wf1Tb_e = nc.dram_tensor("wf1Tb", [D, FF], bf16, kind="ExternalInput")
    wf2Tb_e = nc.dram_tensor("wf2Tb", [FF, D], bf16, kind="ExternalInput")
    c1A_e = nc.dram_tensor("c1A", [P, P], bf16, kind="ExternalInput")
    c1B_e = nc.dram_tensor("c1B", [P, P], bf16, kind="ExternalInput")
    c2A_e = nc.dram_tensor("c2A", [P, P], bf16, kind="ExternalInput")
    c2B_e = nc.dram_tensor("c2B", [P, P], bf16, kind="ExternalInput")
    # per-partition bias columns
    bqc_e = nc.dram_tensor("bqc", [P, ND], fp32, kind="ExternalInput")   # bq/8
    bkc_e = nc.dram_tensor("bkc", [P, ND], fp32, kind="ExternalInput")
    bc1A_e = nc.dram_tensor("bc1A", [P, 1], fp32, kind="ExternalInput")
    bc1B_e = nc.dram_tensor("bc1B", [P, 1], fp32, kind="ExternalInput")
    bc2c_e = nc.dram_tensor("bc2c", [P, 1], fp32, kind="ExternalInput")
    bf1c_e = nc.dram_tensor("bf1c", [P, NFF], fp32, kind="ExternalInput")
    # bias rows (K=1 matmul trick)
    bvr_e = nc.dram_tensor("bvr", [1, D], bf16, kind="ExternalInput")
    bf2r_e = nc.dram_tensor("bf2r", [1, D], bf16, kind="ExternalInput")
    onesb_e = nc.dram_tensor("onesb", [1, S], bf16, kind="ExternalInput")
    ident8_e = nc.dram_tensor("ident8", [P, P], fp8e4, kind="ExternalInput")
    # LN params as rows
    g1r_e = nc.dram_tensor("g1r", [1, D], fp32, kind="ExternalInput")
    b1r_e = nc.dram_tensor("b1r", [1, D], fp32, kind="ExternalInput")
    g2r_e = nc.dram_tensor("g2r", [1, D], fp32, kind="ExternalInput")
    b2r_e = nc.dram_tensor("b2r", [1, D], fp32, kind="ExternalInput")

    out_e = nc.dram_tensor("out", [S, D], fp32, kind="ExternalOutput")
    if debug:
        dbg_qt_e = nc.dram_tensor("dbg_qt", [P, ND, S], fp32, kind="ExternalOutput")
        dbg_kt_e = nc.dram_tensor("dbg_kt", [P, ND, S], fp32, kind="ExternalOutput")
        dbg_v_e = nc.dram_tensor("dbg_v", [P, NQT, D], fp32, kind="ExternalOutput")
        dbg_bias_e = nc.dram_tensor("dbg_bias", [P, H, S], fp32, kind="ExternalOutput")
        dbg_ctx_e = nc.dram_tensor("dbg_ctx", [P, ND, S], fp32, kind="ExternalOutput")
        dbg_xln_e = nc.dram_tensor("dbg_xln", [P, NQT, D], fp32, kind="ExternalOutput")

    with tile.TileContext(nc) as tc:
        # ------------- persistent pools -------------
        const_cm = tc.tile_pool(name="const", bufs=1)
        const = const_cm.__enter__()

        ident_b = const.tile([P, P], bf16)
        make_identity(nc, ident_b)
        ident8 = const.tile([P, P], fp8e4)
        nc.scalar.dma_start(out=ident8, in_=ident8_e[:, :])

        eps_log_c = const.tile([P, 1], fp32)
        nc.vector.memset(eps_log_c, EPS_LOG)
        eps_ln_c = const.tile([P, 1], fp32)
        nc.vector.memset(eps_ln_c, EPS_LN)

        c1A = const.tile([P, P], bf16)
        c1B = const.tile([P, P], bf16)
        c2A = const.tile([P, P], bf16)
        c2B = const.tile([P, P], bf16)
        nc.scalar.dma_start(out=c1A, in_=c1A_e[:, :])
        nc.scalar.dma_start(out=c1B, in_=c1B_e[:, :])
        nc.scalar.dma_start(out=c2A, in_=c2A_e[:, :])
        nc.scalar.dma_start(out=c2B, in_=c2B_e[:, :])
        bc1A = const.tile([P, 1], fp32)
        bc1B = const.tile([P, 1], fp32)
        bc2c = const.tile([P, 1], fp32)
        nc.scalar.dma_start(out=bc1A, in_=bc1A_e[:, :])
        nc.scalar.dma_start(out=bc1B, in_=bc1B_e[:, :])
        nc.scalar.dma_start(out=bc2c, in_=bc2c_e[:, :])
        bqc = const.tile([P, ND], fp32)
        bkc = const.tile([P, ND], fp32)
        bf1c = const.tile([P, NFF], fp32)
        nc.scalar.dma_start(out=bqc, in_=bqc_e[:, :])
        nc.scalar.dma_start(out=bkc, in_=bkc_e[:, :])
        nc.scalar.dma_start(out=bf1c, in_=bf1c_e[:, :])
        bvr = const.tile([1, D], bf16)
        bf2r = const.tile([1, D], bf16)
        onesb = const.tile([1, S], bf16)
        nc.scalar.dma_start(out=bvr, in_=bvr_e[:, :])
        nc.scalar.dma_start(out=bf2r, in_=bf2r_e[:, :])
        nc.scalar.dma_start(out=onesb, in_=onesb_e[:, :])

        # ctxT outlives attp (merge reads it); entered first for stack order
        midp_cm = tc.tile_pool(name="midp", bufs=1)
        midp = midp_cm.__enter__()
        ctxT = midp.tile([P, ND, S], bf16)     # [(h,dh)-part, blk, q]
        wmres = midp.tile([P, ND, D], bf16)    # merge weights (prefetched)
        qbm = midp.tile([P, NQT, D], bf16)     # residual q + bm (prefetched)
        dram_cm = tc.tile_pool(name="dstage", bufs=1, space="DRAM")
        dram = dram_cm.__enter__()
        bstage = dram.tile([S, H, S], fp8e4)

        # ========== attention-lifetime pool ==========
        attp_cm = tc.tile_pool(name="attp", bufs=1)
        attp = attp_cm.__enter__()
        QtT = attp.tile([P, ND, S], bf16)      # [o-part, o-blk, s]  ((Wq x + bq)/8)
        KtT = attp.tile([P, ND, S], bf16)
        Vsb = attp.tile([P, NQT, D], bf16)     # [k-part, k-blk, (h dh)]
        biasq = attp.tile([P, NQT, H, S], fp8e4)  # [q-part, qt, h, k]

        # =========== Phase 1+2: projections + conv bias ===========
        NQI2 = 8          # q rows per partition-group per conv block
        NH2 = S // (8 * NQI2)  # 8 conv blocks of 64 q rows
        with (
            tc.tile_pool(name="p2sb", bufs=2) as p2sb,
            tc.tile_pool(name="p2ps", bufs=1, space="PSUM") as p2ps,
        ):
            with (
                tc.tile_pool(name="p1x", bufs=1) as p1x,
                tc.tile_pool(name="p1w", bufs=2) as p1w,
                tc.tile_pool(name="p1ps", bufs=2, space="PSUM") as p1ps,
            ):
                qTb = p1x.tile([P, ND, S], bf16)
                nc.sync.dma_start(
                    out=qTb, in_=qTb_e.rearrange("(n p) s -> p n s", p=P)
                )
                kvTb = p1x.tile([P, ND, S], bf16)
                nc.sync.dma_start(
                    out=kvTb, in_=kvTb_e.rearrange("(n p) s -> p n s", p=P)
                )

                # Q/K: out[o, s] += w[d-blk, o].T @ xT[d-blk, s]
                for wsrc, xsb, dst, bcol, scl in (
                    (wqTb_e, qTb, QtT, bqc, 0.125),
                    (wkTb_e, kvTb, KtT, bkc, 1.0),
                ):
                    wres = p1w.tile([P, ND, D], bf16, tag="wres")
                    nc.sync.dma_start(
                        out=wres, in_=wsrc.rearrange("(n p) d -> p n d", p=P)
                    )
                    for ob in range(ND):
                        ps = p1ps.tile([P, S], fp32, tag="pjps")
                        for dblk in range(ND):
                            nc.tensor.matmul(
                                ps,
                                wres[:, dblk, ts(ob, P)],
                                xsb[:, dblk, :],
                                start=(dblk == 0),
                                stop=(dblk == ND - 1),
                            )
                        nc.vector.tensor_scalar(
                            out=dst[:, ob, :], in0=ps,
                            scalar1=scl, scalar2=bcol[:, ob : ob + 1],
                            op0=ALU.mult, op1=ALU.add,
                        )

                # V: out[k, o] += kvT[d-blk, k-tile].T @ wv[d-blk, o-half]
                wvres = p1w.tile([P, ND, D], bf16, tag="wres")
                nc.sync.dma_start(
                    out=wvres, in_=wvTb_e.rearrange("(n p) d -> p n d", p=P)
                )
                for kt in range(NQT):
                    for oh in range(2):
                        ps = p1ps.tile([P, S], fp32, tag="pjps")
                        nc.tensor.matmul(
                            ps, onesb[:, 0:P], bvr[:, ts(oh, S)],
                            start=True, stop=False,
                        )
                        for dblk in range(ND):
                            nc.tensor.matmul(
                                ps,
                                kvTb[:, dblk, ts(kt, P)],
                                wvres[:, dblk, ts(oh, S)],
                                start=False,
                                stop=(dblk == ND - 1),
                            )
                        nc.vector.tensor_copy(Vsb[:, kt, ts(oh, S)], ps)

            # conv in 2 super-blocks: [Ln x4 | fence | conv x4] x2
            # so each stage loads its activation table exactly once.
            # amap(h) is issued right after Ln(h-2) (the last reader of the
            # ring slot it overwrites) so block-2 loads prefetch during
            # block-1 compute.
            amts = {}

            def issue_amap(half):
                qbase = half * 64
                amt = p2sb.tile([P, NQI2, S], fp8e4, tag="amt", bufs=2)
                for g in range(8):
                    src = bass.AP(
                        tensor=amapb_e,
                        offset=(1 + qbase + NQI2 * g) * AM + 1,
                        ap=[[AM * AM, CH], [AM, NQI2], [1, S]],
                    )
                    nc.sync.dma_start(out=amt[CH * g : CH * (g + 1)], in_=src)
                amts[half] = amt

            issue_amap(0)
            issue_amap(1)
            # prefetch merge-phase tensors while conv/attention run
            nc.sync.dma_start(
                out=wmres, in_=wmTb_e.rearrange("(n p) d -> p n d", p=P)
            )
            nc.sync.dma_start(out=qbm, in_=qbm_e.rearrange("(n p) d -> p n d", p=P))

            HPB = NH2 // 2
            for sblk in range(2):
                logms = []
                for half in range(sblk * HPB, (sblk + 1) * HPB):
                    amt = amts[half]
                    logm = p2sb.tile([P, NQI2 * S], bf16, tag="logm", bufs=4)
                    amtf = amt.rearrange("p a b -> p (a b)")
                    for i in range(2):
                        nc.scalar.activation(
                            logm[:, ts(i, 4 * S)], amtf[:, ts(i, 4 * S)], AF.Ln,
                            bias=eps_log_c, scale=1.0,
                        )
                    logms.append(logm)
                    if half + 2 < NH2:
                        issue_amap(half + 2)

                tc.no_sync_barrier()

                for hi, half in enumerate(range(sblk * HPB, (sblk + 1) * HPB)):
                    qbase = half * 64
                    logm = logms[hi]
                    c2sb = p2sb.tile([P, NQI2, S], fp8e4, tag="c2sb")
                    for j in range(4):  # four [P, 1024] chunks
                        pA = p2ps.tile([P, 2 * S], fp32, tag="pA")
                        pB = p2ps.tile([P, 2 * S], fp32, tag="pB")
                        for c in range(2):
                            chk = 2 * j + c
                            nc.tensor.matmul(
                                pA[:, ts(c, S)], c1A, logm[:, ts(chk, S)],
                                start=True, stop=True,
                            )
                            nc.tensor.matmul(
                                pB[:, ts(c, S)], c1B, logm[:, ts(chk, S)],
                                start=True, stop=True,
                            )
                        gA = p2sb.tile([P, 2 * S], bf16, tag="gA", bufs=1)
                        gB = p2sb.tile([P, 2 * S], bf16, tag="gB", bufs=1)
                        nc.scalar.activation(gA, pA, AF.Gelu, bias=bc1A, scale=1.0)
                        nc.scalar.activation(gB, pB, AF.Gelu, bias=bc1B, scale=1.0)
                        pC = p2ps.tile([P, 2 * S], fp32, tag="pC")
                        for c in range(2):
                            nc.tensor.matmul(
                                pC[:, ts(c, S)], c2A, gA[:, ts(c, S)],
                                start=True, stop=False,
                            )
                            nc.tensor.matmul(
                                pC[:, ts(c, S)], c2B, gB[:, ts(c, S)],
                                start=False, stop=True,
                            )
                        nc.vector.tensor_scalar(
                            out=c2sb[:, 2 * j : 2 * j + 2, :].rearrange(
                                "p a b -> p (a b)"
                            ),
                            in0=pC, scalar1=bc2c[:, 0:1], scalar2=None,
                            op0=ALU.add,
                        )
                    # stage to DRAM in [q, h, k] order
                    for g in range(8):
                        q0 = qbase + NQI2 * g
                        nc.sync.dma_start(
                            out=bstage[q0 : q0 + NQI2].rearrange(
                                "q h k -> h q k"
                            ),
                            in_=c2sb[CH * g : CH * (g + 1)],
                        )
                    # read back q-major as soon as a qtile's rows are staged
                    if half % 2 == 1:
                        qt = half // 2
                        nc.sync.dma_start(
                            out=biasq[:, qt],
                            in_=bstage[qt * P : (qt + 1) * P],
                        )
                if sblk == 0:
                    tc.no_sync_barrier()

        if debug:
            dbgq = const.tile([P, ND, S], fp32, name="dbgq")
            nc.vector.tensor_copy(dbgq.rearrange("p a b -> p (a b)"),
                                  QtT.rearrange("p a b -> p (a b)"))
            nc.sync.dma_start(out=dbg_qt_e[:, :, :], in_=dbgq)
            nc.vector.tensor_copy(dbgq.rearrange("p a b -> p (a b)"),
                                  KtT.rearrange("p a b -> p (a b)"))
            nc.sync.dma_start(out=dbg_kt_e[:, :, :], in_=dbgq)
            dbgv = const.tile([P, NQT, D], fp32, name="dbgv")
            nc.vector.tensor_copy(dbgv.rearrange("p a b -> p (a b)"),
                                  Vsb.rearrange("p a b -> p (a b)"))
            nc.sync.dma_start(out=dbg_v_e[:, :, :], in_=dbgv)
            dbgb = const.tile([P, H, S], fp32, name="dbgb")
            nc.vector.tensor_copy(dbgb.rearrange("p a b -> p (a b)"),
                                  biasq[:, 0].rearrange("p a b -> p (a b)"))
            nc.sync.dma_start(out=dbg_bias_e[:, :, :], in_=dbgb)

        # =========== Phase 3: attention (one table: Exp) ===========
        tc.no_sync_barrier()
        with (
            tc.tile_pool(name="p3sb", bufs=1) as p3sb,
            tc.tile_pool(name="p3ps", bufs=1, space="PSUM") as p3ps,
        ):
            sc_pool = [p3ps.tile([P, S], fp32, tag=f"sc{i}", name=f"sc{i}")
                       for i in range(4)]
            atu_pool = [p3ps.tile([P, NQT, P], bf16, tag=f"atu{i}", name=f"atu{i}")
                        for i in range(2)]
            cx_pool = [p3ps.tile([P, S], fp32, tag=f"cx{i}", name=f"cx{i}")
                       for i in range(2)]
            att_pool = [p3sb.tile([P, S], bf16, tag=f"att{i}", name=f"att{i}")
                        for i in range(4)]
            attn_pool = [p3sb.tile([P, S], bf16, tag=f"attn{i}", name=f"attn{i}")
                         for i in range(4)]
            den_pool = [p3sb.tile([P, 1], fp32, tag=f"den{i}", name=f"den{i}")
                        for i in range(4)]
            rec_pool = [p3sb.tile([P, 1], fp32, tag=f"rec{i}", name=f"rec{i}")
                        for i in range(4)]
            ath_pool = [p3sb.tile([P, NQT, S], bf16, tag=f"ath{i}", name=f"ath{i}")
                        for i in range(2)]

            def issue_scores(h):
                hb, ho = (h * DH) // P, (h * DH) % P
                for qt in range(NQT):
                    slot = (h * NQT + qt) % 4
                    sc = sc_pool[slot]
                    # psum <- bias, then += Qt^T K (Qt pre-scaled by 1/8)
                    nc.tensor.matmul(
                        sc, ident8, biasq[:, qt, h, :],
                        start=True, stop=False,
                    )
                    nc.tensor.matmul(
                        sc,
                        QtT[ho : ho + DH, hb, ts(qt, P)],
                        KtT[ho : ho + DH, hb, :],
                        start=False, stop=True,
                    )
                    nc.scalar.activation(
                        att_pool[slot], sc, AF.Exp, accum_out=den_pool[slot]
                    )
                    nc.vector.reciprocal(out=rec_pool[slot], in_=den_pool[slot])
                    nc.vector.tensor_scalar_mul(
                        attn_pool[slot], att_pool[slot], rec_pool[slot][:, 0:1]
                    )

            def issue_transp(h):
                ath = ath_pool[h % 2]
                for qt in range(NQT):
                    slot = (h * NQT + qt) % 4
                    atu = atu_pool[qt % 2]
                    for kt in range(NQT):
                        nc.tensor.transpose(
                            atu[:, kt, :], attn_pool[slot][:, ts(kt, P)], ident_b
                        )
                    nc.vector.tensor_copy(ath[:, :, ts(qt, P)], atu)

            def issue_ctx(h):
                ath = ath_pool[h % 2]
                cx = cx_pool[(h // 2) % 2]
                prange = cx[(h % 2) * DH : (h % 2) * DH + DH, :]
                for kt in range(NQT):
                    nc.tensor.matmul(
                        prange,
                        Vsb[:, kt, h * DH : (h + 1) * DH],
                        ath[:, kt, :],
                        start=(kt == 0), stop=(kt == NQT - 1),
                    )
                if h % 2 == 1:
                    nc.vector.tensor_copy(ctxT[:, h // 2, :], cx)

            # software pipeline: transp(h-1) | scores(h) | ctx(h-1)
            issue_scores(0)
            for h in range(1, H):
                issue_transp(h - 1)
                issue_scores(h)
                issue_ctx(h - 1)
            issue_transp(H - 1)
            issue_ctx(H - 1)

        if debug:
            dbgc = const.tile([P, ND, S], fp32, name="dbgc")
            nc.vector.tensor_copy(dbgc.rearrange("p a b -> p (a b)"),
                                  ctxT.rearrange("p a b -> p (a b)"))
            nc.sync.dma_start(out=dbg_ctx_e[:, :, :], in_=dbgc)

        # free attention residents before FFN
        attp_cm.__exit__(None, None, None)

        ffp_cm = tc.tile_pool(name="ffp", bufs=1)
        ffp = ffp_cm.__enter__()
        xln = ffp.tile([P, NQT, D], fp32)
        xlnb = ffp.tile([P, NQT, D], bf16)
        xlnT = ffp.tile([P, ND, S], bf16)
        y1T = ffp.tile([P, NFF, S], bf16)
        # LN param broadcast rows -> [P, D]
        g1b = ffp.tile([P, D], fp32)
        b1b = ffp.tile([P, D], fp32)
        g2b = ffp.tile([P, D], fp32)
        b2b = ffp.tile([P, D], fp32)
        for dst, src_e in ((g1b, g1r_e), (b1b, b1r_e), (g2b, g2r_e), (b2b, b2r_e)):
            row = ffp.tile([1, D], fp32, tag="lnrow", name="lnrow")
            nc.sync.dma_start(out=row, in_=src_e[:, :])
            nc.gpsimd.partition_broadcast(dst, row[0:1, :])

        # =========== Phase 4: merge + residual + LN1 (+ transpose) ===========
        tc.no_sync_barrier()
        with (
            tc.tile_pool(name="p4sb", bufs=2) as p4sb,
            tc.tile_pool(name="p4ps", bufs=2, space="PSUM") as p4ps,
            tc.tile_pool(name="p4tp", bufs=2, space="PSUM") as p4tp,
        ):
            for st in range(NQT):
                qtile = qbm[:, st, :]
                x1 = p4sb.tile([P, D], fp32, tag="x1")
                for oh in range(2):
                    ps = p4ps.tile([P, S], fp32, tag="mps")
                    for dblk in range(ND):
                        nc.tensor.matmul(
                            ps,
                            ctxT[:, dblk, ts(st, P)],
                            wmres[:, dblk, ts(oh, S)],
                            start=(dblk == 0),
                            stop=(dblk == ND - 1),
                        )
                    nc.vector.tensor_tensor(
                        out=x1[:, ts(oh, S)], in0=ps,
                        in1=qtile[:, ts(oh, S)], op=ALU.add,
                    )
                _layernorm(nc, p4sb, xln[:, st, :], x1, g1b, b1b, eps_ln_c)
                nc.scalar.activation(xlnb[:, st, :], xln[:, st, :], AF.Copy)
                for dblk in range(ND):
                    tp = p4tp.tile([P, P], bf16, tag="tp")
                    nc.tensor.transpose(
                        tp, xlnb[:, st, ts(dblk, P)], ident_b
                    )
                    nc.vector.tensor_copy(xlnT[:, dblk, ts(st, P)], tp)

        if debug:
            nc.sync.dma_start(out=dbg_xln_e[:, :, :], in_=xln)

        # =========== Phase 5: FFN1 + relu ===========
        with (
            tc.tile_pool(name="p5w", bufs=2) as p5w,
            tc.tile_pool(name="p5ps", bufs=2, space="PSUM") as p5ps,
        ):
            NGRP = 4
            FPG = NFF // NGRP  # 8 ff-blocks per group
            for grp in range(NGRP):
                wf1g = p5w.tile([P, ND, FPG * P], bf16, tag="wf1g")
                nc.sync.dma_start(
                    out=wf1g,
                    in_=wf1Tb_e[
                        :, grp * FPG * P : (grp + 1) * FPG * P
                    ].rearrange("(n p) f -> p n f", p=P),
                )
                for sh in range(2):
                    for fl in range(FPG):
                        ffb = grp * FPG + fl
                        ps = p5ps.tile([P, S // 2], fp32, tag="fps")
                        for dblk in range(ND):
                            nc.tensor.matmul(
                                ps,
                                wf1g[:, dblk, ts(fl, P)],
                                xlnT[:, dblk, ts(sh, S // 2)],
                                start=(dblk == 0), stop=(dblk == ND - 1),
                            )
                        nc.scalar.activation(
                            y1T[:, ffb, ts(sh, S // 2)], ps, AF.Relu,
                            bias=bf1c[:, ffb : ffb + 1], scale=1.0,
                        )

        # =========== Phase 6: FFN2 + residual + LN2 + out ===========
        with (
            tc.tile_pool(name="p7sb", bufs=2) as p7sb,
            tc.tile_pool(name="p7w", bufs=2) as p7w,
            tc.tile_pool(name="p7ps", bufs=1, space="PSUM") as p7ps,
        ):
            fps2 = [
                [p7ps.tile([P, S], fp32, tag=f"f2{st * 2 + oh}", name=f"f2{st}{oh}")
                 for oh in range(2)]
                for st in range(NQT)
            ]
            for st in range(NQT):
                for oh in range(2):
                    nc.tensor.matmul(
                        fps2[st][oh], onesb[:, 0:P], bf2r[:, ts(oh, S)],
                        start=True, stop=False,
                    )
            for fpair in range(NFF // 2):
                wch = p7w.tile([P, 2, D], bf16, tag="wch")
                nc.sync.dma_start(
                    out=wch,
                    in_=wf2Tb_e[fpair * 2 * P : (fpair + 1) * 2 * P, :].rearrange(
                        "(a p) d -> p a d", p=P
                    ),
                )
                for a in range(2):
                    ffb = fpair * 2 + a
                    for st in range(NQT):
                        for oh in range(2):
                            nc.tensor.matmul(
                                fps2[st][oh],
                                y1T[:, ffb, ts(st, P)],
                                wch[:, a, ts(oh, S)],
                                start=False,
                                stop=(ffb == NFF - 1),
                            )
            for st in range(NQT):
                x2 = p7sb.tile([P, D], fp32, tag="x2")
                for oh in range(2):
                    nc.vector.tensor_tensor(
                        out=x2[:, ts(oh, S)], in0=fps2[st][oh],
                        in1=xln[:, st, ts(oh, S)], op=ALU.add,
                    )
                xout = p7sb.tile([P, D], fp32, tag="xout")
                _layernorm(nc, p7sb, xout, x2, g2b, b2b, eps_ln_c)
                nc.sync.dma_start(out=out_e[st * P : (st + 1) * P, :], in_=xout)

        ffp_cm.__exit__(None, None, None)
        dram_cm.__exit__(None, None, None)
        midp_cm.__exit__(None, None, None)
        const_cm.__exit__(None, None, None)

    nc.finalize()
    return nc


def _prep_inputs(q, kv, attn_map, Wq, bq, Wk, bk, Wv, bv, Wm, bm,
                 Wc1, bc1, Wc2, bc2, Wf1, bf1, Wf2, bf2, g1, b1, g2, b2):
    """Host-side packing. Returns (shared dict, per-core list of dicts)."""
    f32 = np.float32
    bf = ml_dtypes.bfloat16

    def c(a):
        return np.ascontiguousarray(np.asarray(a), dtype=f32)

    def cb(a):
        return np.ascontiguousarray(np.asarray(a, dtype=f32)).astype(bf)

    Wq, Wk, Wv, Wm = c(Wq), c(Wk), c(Wv), c(Wm)
    Wc1, Wc2 = c(Wc1), c(Wc2)
    bq, bk, bv, bm = c(bq), c(bk), c(bv), c(bm)
    bc1, bc2, bf1, bf2 = c(bc1), c(bc2), c(bf1), c(bf2)
    g1, b1, g2, b2 = c(g1), c(b1), c(g2), c(b2)

    shared = {
        "wqTb": cb(Wq.T), "wkTb": cb(Wk.T), "wvTb": cb(Wv.T), "wmTb": cb(Wm.T),
        "wf1Tb": cb(np.asarray(Wf1).T),
        "wf2Tb": cb(np.asarray(Wf2).T),
        "bqc": c((bq / 8.0).reshape(ND, P).T),
        "bkc": c(bk.reshape(ND, P).T),
        "bf1c": c(bf1.reshape(NFF, P).T),
        "bvr": cb(bv.reshape(1, D)),
        "bf2r": cb(bf2.reshape(1, D)),
        "onesb": np.ones((1, S), bf),
        "ident8": np.eye(P, dtype=ml_dtypes.float8_e4m3),
        "g1r": g1.reshape(1, D), "b1r": b1.reshape(1, D),
        "g2r": g2.reshape(1, D), "b2r": b2.reshape(1, D),
    }
    # conv block-diag lhsT [K, M]: out[(g,oh)] = sum_c lhsT[(g,c),(g,oh)] rhs[(g,c)]
    c1A = np.zeros((P, P), f32)
    c1B = np.zeros((P, P), f32)
    c2A = np.zeros((P, P), f32)
    c2B = np.zeros((P, P), f32)
    for g in range(8):
        sl = slice(g * 16, g * 16 + 16)
        c1A[sl, sl] = Wc1[0:16, :].T     # [c, oh]
        c1B[sl, sl] = Wc1[16:32, :].T
        c2A[sl, sl] = Wc2[:, 0:16].T     # [ci, h]
        c2B[sl, sl] = Wc2[:, 16:32].T
    shared["c1A"] = c1A.astype(bf)
    shared["c1B"] = c1B.astype(bf)
    shared["c2A"] = c2A.astype(bf)
    shared["c2B"] = c2B.astype(bf)
    shared["bc1A"] = np.tile(bc1[0:16], 8).reshape(P, 1).astype(f32)
    shared["bc1B"] = np.tile(bc1[16:32], 8).reshape(P, 1).astype(f32)
    shared["bc2c"] = np.tile(bc2, 8).reshape(P, 1).astype(f32)

    q = c(q)
    kv = c(kv)
    per_core = []
    for b in range(B):
        per_core.append({
            "qTb": cb(q[b].T), "kvTb": cb(kv[b].T),
            "qbm": cb(q[b] + bm.reshape(1, D)),
            "amapb": np.asarray(attn_map[b], dtype=np.float32).astype(ml_dtypes.float8_e4m3),
        })
    return shared, per_core


def kernel(**inputs):
    if "nc" not in _CACHED:
        _CACHED["nc"] = build_program()
    nc = _CACHED["nc"]
    shared, per_core = _prep_inputs(**inputs)
    in_maps = [dict(shared, **pc) for pc in per_core]
    res = run_bass_kernel_spmd(nc, in_maps, list(range(B)))
    out = np.stack([res.results[i]["out"] for i in range(B)], axis=0)
    return out.astype(np.float32)


# revision 43
# speedup vs baseline: 1.8545x; 1.0494x over previous
"""Trainium2 Bass kernel for nn_CrossAttention (dense transformer block).

Sharding: data-parallel over batch - 8 batch elements, one per NeuronCore.
Each core runs the full block for its batch element:
  bias = Conv1x1(gelu(Conv1x1(log(attn_map[1:,1:] + eps))))
  MHA(q, kv) with bias added to scores; residual + LN; FFN; residual + LN.

Perf structure (v2):
  - all matmuls bf16 (moving+stationary) with fp32 PSUM accumulation
  - strict phase order so the scalar engine loads each activation table once
    (ln -> gelu -> exp -> rsqrt)
  - conv bias kept SBUF-resident in q-major layout via SBUF->SBUF DMA shuffle
  - score bias added by PSUM-init matmul (identity x bias) instead of vector add
  - software-pipelined attention (head h+1 scores issued before head h ctx)
  - bulk DMA (attn_map loads + bias shuffle) on the idle gpsimd queue
"""

import numpy as np
import ml_dtypes

import concourse.bass as bass
import concourse.mybir as mybir
import concourse.tile as tile
from concourse import bacc
from concourse.bass import ts
from concourse.bass_utils import run_bass_kernel_spmd
from concourse.masks import make_identity

AF = mybir.ActivationFunctionType
ALU = mybir.AluOpType

B, S, D, H, DH, FF = 8, 512, 1024, 16, 64, 4096
CH, CHID = 16, 32
EPS_LOG = 1e-6
EPS_LN = 1e-6
P = 128
NQT = S // P          # 4 q-tiles
ND = D // P           # 8 d-blocks
NFF = FF // P         # 32 ff-blocks
AM = 513              # attn_map edge
NQI = 4               # q rows per partition-group in conv
NHALF = S // 32       # 16 conv halves (32 q rows each)

fp32 = mybir.dt.float32
bf16 = mybir.dt.bfloat16
fp8e4 = mybir.dt.float8e4

_CACHED = {}


def _layernorm(nc, pool, out_ap, x_ap, gb, bb, eps_c):
    """out = (x - mean(x)) * rsqrt(var(x) + eps) * g + b over free dim (D)."""
    nsub = D // 512
    stats = pool.tile([P, nsub, nc.vector.BN_STATS_DIM], fp32, tag="ln_stats")
    for i in range(nsub):
        nc.vector.bn_stats(out=stats[:, i, :], in_=x_ap[:, ts(i, 512)])
    mv = pool.tile([P, nc.vector.BN_AGGR_DIM], fp32, tag="ln_mv")
    nc.vector.bn_aggr(out=mv, in_=stats)
    rstd = pool.tile([P, 1], fp32, tag="ln_rstd")
    nc.scalar.activation(rstd, mv[:, 1:2], AF.Sqrt, bias=eps_c, scale=1.0)
    nc.vector.reciprocal(out=rstd, in_=rstd)
    u = pool.tile([P, D], fp32, tag="ln_u")
    nc.vector.scalar_tensor_tensor(
        out=u, in0=x_ap, scalar=mv[:, 0:1], in1=gb,
        op0=ALU.subtract, op1=ALU.mult,
    )
    nc.vector.scalar_tensor_tensor(
        out=out_ap, in0=u, scalar=rstd[:, 0:1], in1=bb,
        op0=ALU.mult, op1=ALU.add,
    )


def build_program(debug=False):
    nc = bacc.Bacc(None)

    # ---------------- DRAM I/O ----------------
    qTb_e = nc.dram_tensor("qTb", [D, S], bf16, kind="ExternalInput")
    kvTb_e = nc.dram_tensor("kvTb", [D, S], bf16, kind="ExternalInput")
    qbm_e = nc.dram_tensor("qbm", [S, D], bf16, kind="ExternalInput")  # q + bm
    amapb_e = nc.dram_tensor("amapb", [CH, AM, AM], fp8e4, kind="ExternalInput")
    wqTb_e = nc.dram_tensor("wqTb", [D, D], bf16, kind="ExternalInput")
    wkTb_e = nc.dram_tensor("wkTb", [D, D], bf16, kind="ExternalInput")
    wvTb_e = nc.dram_tensor("wvTb", [D, D], bf16, kind="ExternalInput")
    wmTb_e = nc.dram_tensor("wmTb", [D, D], bf16, kind="ExternalInput")
    wf1Tb_e = nc.dram_tensor("wf1Tb", [D, FF], bf16, kind="ExternalInput")
    wf2Tb_e = nc.dram_tensor("wf2Tb", [FF, D], bf16, kind="ExternalInput")
    c1A_e = nc.dram_tensor("c1A", [P, P], bf16, kind="ExternalInput")
    c1B_e = nc.dram_tensor("c1B", [P, P], bf16, kind="ExternalInput")
    c2A_e = nc.dram_tensor("c2A", [P, P], bf16, kind="ExternalInput")
    c2B_e = nc.dram_tensor("c2B", [P, P], bf16, kind="ExternalInput")
    # per-partition bias columns
    bqc_e = nc.dram_tensor("bqc", [P, ND], fp32, kind="ExternalInput")   # bq/8
    bkc_e = nc.dram_tensor("bkc", [P, ND], fp32, kind="ExternalInput")
    bc1A_e = nc.dram_tensor("bc1A", [P, 1], fp32, kind="ExternalInput")
    bc1B_e = nc.dram_tensor("bc1B", [P, 1], fp32, kind="ExternalInput")
    bc2c_e = nc.dram_tensor("bc2c", [P, 1], fp32, kind="ExternalInput")
    bf1c_e = nc.dram_tensor("bf1c", [P, NFF], fp32, kind="ExternalInput")
    # bias rows (K=1 matmul trick)
    bvr_e = nc.dram_tensor("bvr", [1, D], bf16, kind="ExternalInput")
    bf2r_e = nc.dram_tensor("bf2r", [1, D], bf16, kind="ExternalInput")
    onesb_e = nc.dram_tensor("onesb", [1, S], bf16, kind="ExternalInput")
    ident8_e = nc.dram_tensor("ident8", [P, P], fp8e4, kind="ExternalInput")
    # LN params as rows
    g1r_e = nc.dram_tensor("g1r", [1, D], fp32, kind="ExternalInput")
    b1r_e = nc.dram_tensor("b1r", [1, D], fp32, kind="ExternalInput")
    g2r_e = nc.dram_tensor("g2r", [1, D], fp32, kind="ExternalInput")
    b2r_e = nc.dram_tensor("b2r", [1, D], fp32, kind="ExternalInput")

    out_e = nc.dram_tensor("out", [S, D], fp32, kind="ExternalOutput")
    if debug:
        dbg_qt_e = nc.dram_tensor("dbg_qt", [P, ND, S], fp32, kind="ExternalOutput")
        dbg_kt_e = nc.dram_tensor("dbg_kt", [P, ND, S], fp32, kind="ExternalOutput")
        dbg_v_e = nc.dram_tensor("dbg_v", [P, NQT, D], fp32, kind="ExternalOutput")
        dbg_bias_e = nc.dram_tensor("dbg_bias", [P, H, S], fp32, kind="ExternalOutput")
        dbg_ctx_e = nc.dram_tensor("dbg_ctx", [P, ND, S], fp32, kind="ExternalOutput")
        dbg_xln_e = nc.dram_tensor("dbg_xln", [P, NQT, D], fp32, kind="ExternalOutput")

    with tile.TileContext(nc) as tc:
        # ------------- persistent pools -------------
        const_cm = tc.tile_pool(name="const", bufs=1)
        const = const_cm.__enter__()

        ident_b = const.tile([P, P], bf16)
        make_identity(nc, ident_b)
        ident8 = const.tile([P, P], fp8e4)
        nc.scalar.dma_start(out=ident8, in_=ident8_e[:, :])

        eps_log_c = const.tile([P, 1], fp32)
        nc.vector.memset(eps_log_c, EPS_LOG)
        eps_ln_c = const.tile([P, 1], fp32)
        nc.vector.memset(eps_ln_c, EPS_LN)

        c1A = const.tile([P, P], bf16)
        c1B = const.tile([P, P], bf16)
        c2A = const.tile([P, P], bf16)
        c2B = const.tile([P, P], bf16)
        nc.scalar.dma_start(out=c1A, in_=c1A_e[:, :])
        nc.scalar.dma_start(out=c1B, in_=c1B_e[:, :])
        nc.scalar.dma_start(out=c2A, in_=c2A_e[:, :])
        nc.scalar.dma_start(out=c2B, in_=c2B_e[:, :])
        bc1A = const.tile([P, 1], fp32)
        bc1B = const.tile([P, 1], fp32)
        bc2c = const.tile([P, 1], fp32)
        nc.scalar.dma_start(out=bc1A, in_=bc1A_e[:, :])
        nc.scalar.dma_start(out=bc1B, in_=bc1B_e[:, :])
        nc.scalar.dma_start(out=bc2c, in_=bc2c_e[:, :])
        bqc = const.tile([P, ND], fp32)
        bkc = const.tile([P, ND], fp32)
        bf1c = const.tile([P, NFF], fp32)
        nc.scalar.dma_start(out=bqc, in_=bqc_e[:, :])
        nc.scalar.dma_start(out=bkc, in_=bkc_e[:, :])
        nc.scalar.dma_start(out=bf1c, in_=bf1c_e[:, :])
        bvr = const.tile([1, D], bf16)
        bf2r = const.tile([1, D], bf16)
        onesb = const.tile([1, S], bf16)
        nc.scalar.dma_start(out=bvr, in_=bvr_e[:, :])
        nc.scalar.dma_start(out=bf2r, in_=bf2r_e[:, :])
        nc.scalar.dma_start(out=onesb, in_=onesb_e[:, :])

        # ctxT outlives attp (merge reads it); entered first for stack order
        midp_cm = tc.tile_pool(name="midp", bufs=1)
        midp = midp_cm.__enter__()
        ctxT = midp.tile([P, ND, S], bf16)     # [(h,dh)-part, blk, q]
        wmres = midp.tile([P, ND, D], bf16)    # merge weights (prefetched)
        qbm = midp.tile([P, NQT, D], bf16)     # residual q + bm (prefetched)
        dram_cm = tc.tile_pool(name="dstage", bufs=1, space="DRAM")
        dram = dram_cm.__enter__()
        bstage = dram.tile([S, H, S], fp8e4)

        # ========== attention-lifetime pool ==========
        attp_cm = tc.tile_pool(name="attp", bufs=1)
        attp = attp_cm.__enter__()
        QtT = attp.tile([P, ND, S], bf16)      # [o-part, o-blk, s]  ((Wq x + bq)/8)
        KtT = attp.tile([P, ND, S], bf16)
        Vsb = attp.tile([P, NQT, D], bf16)     # [k-part, k-blk, (h dh)]
        biasq = attp.tile([P, NQT, H, S], fp8e4)  # [q-part, qt, h, k]

        # =========== Phase 1+2: projections + conv bias ===========
        NQI2 = 8          # q rows per partition-group per conv block
        NH2 = S // (8 * NQI2)  # 8 conv blocks of 64 q rows
        with (
            tc.tile_pool(name="p2sb", bufs=2) as p2sb,
            tc.tile_pool(name="p2ps", bufs=1, space="PSUM") as p2ps,
        ):
            with (
                tc.tile_pool(name="p1x", bufs=1) as p1x,
                tc.tile_pool(name="p1w", bufs=2) as p1w,
                tc.tile_pool(name="p1ps", bufs=2, space="PSUM") as p1ps,
            ):
                qTb = p1x.tile([P, ND, S], bf16)
                nc.sync.dma_start(
                    out=qTb, in_=qTb_e.rearrange("(n p) s -> p n s", p=P)
                )
                kvTb = p1x.tile([P, ND, S], bf16)
                nc.sync.dma_start(
                    out=kvTb, in_=kvTb_e.rearrange("(n p) s -> p n s", p=P)
                )

                # Q/K: out[o, s] += w[d-blk, o].T @ xT[d-blk, s]
                for wsrc, xsb, dst, bcol, scl in (
                    (wqTb_e, qTb, QtT, bqc, 0.125),
                    (wkTb_e, kvTb, KtT, bkc, 1.0),
                ):
                    wres = p1w.tile([P, ND, D], bf16, tag="wres")
                    nc.sync.dma_start(
                        out=wres, in_=wsrc.rearrange("(n p) d -> p n d", p=P)
                    )
                    for ob in range(ND):
                        ps = p1ps.tile([P, S], fp32, tag="pjps")
                        for dblk in range(ND):
                            nc.tensor.matmul(
                                ps,
                                wres[:, dblk, ts(ob, P)],
                                xsb[:, dblk, :],
                                start=(dblk == 0),
                                stop=(dblk == ND - 1),
                            )
                        nc.vector.tensor_scalar(
                            out=dst[:, ob, :], in0=ps,
                            scalar1=scl, scalar2=bcol[:, ob : ob + 1],
                            op0=ALU.mult, op1=ALU.add,
                        )

                # V: out[k, o] += kvT[d-blk, k-tile].T @ wv[d-blk, o-half]
                wvres = p1w.tile([P, ND, D], bf16, tag="wres")
                nc.sync.dma_start(
                    out=wvres, in_=wvTb_e.rearrange("(n p) d -> p n d", p=P)
                )
                for kt in range(NQT):
                    for oh in range(2):
                        ps = p1ps.tile([P, S], fp32, tag="pjps")
                        nc.tensor.matmul(
                            ps, onesb[:, 0:P], bvr[:, ts(oh, S)],
                            start=True, stop=False,
                        )
                        for dblk in range(ND):
                            nc.tensor.matmul(
                                ps,
                                kvTb[:, dblk, ts(kt, P)],
                                wvres[:, dblk, ts(oh, S)],
                                start=False,
                                stop=(dblk == ND - 1),
                            )
                        nc.vector.tensor_copy(Vsb[:, kt, ts(oh, S)], ps)

            # conv in 2 super-blocks: [Ln x4 | fence | conv x4] x2
            # so each stage loads its activation table exactly once.
            # amap(h) is issued right after Ln(h-2) (the last reader of the
            # ring slot it overwrites) so block-2 loads prefetch during
            # block-1 compute.
            amts = {}

            def issue_amap(half):
                qbase = half * 64
                amt = p2sb.tile([P, NQI2, S], fp8e4, tag="amt", bufs=2)
                for g in range(8):
                    src = bass.AP(
                        tensor=amapb_e,
                        offset=(1 + qbase + NQI2 * g) * AM + 1,
                        ap=[[AM * AM, CH], [AM, NQI2], [1, S]],
                    )
                    nc.sync.dma_start(out=amt[CH * g : CH * (g + 1)], in_=src)
                amts[half] = amt

            issue_amap(0)
            issue_amap(1)
            # prefetch merge-phase tensors while conv/attention run
            nc.sync.dma_start(
                out=wmres, in_=wmTb_e.rearrange("(n p) d -> p n d", p=P)
            )
            nc.sync.dma_start(out=qbm, in_=qbm_e.rearrange("(n p) d -> p n d", p=P))

            HPB = NH2 // 2
            for sblk in range(2):
                logms = []
                for half in range(sblk * HPB, (sblk + 1) * HPB):
                    amt = amts[half]
                    logm = p2sb.tile([P, NQI2 * S], bf16, tag="logm", bufs=4)
                    amtf = amt.rearrange("p a b -> p (a b)")
                    for i in range(2):
                        nc.scalar.activation(
                            logm[:, ts(i, 4 * S)], amtf[:, ts(i, 4 * S)], AF.Ln,
                            bias=eps_log_c, scale=1.0,
                        )
                    logms.append(logm)
                    if half + 2 < NH2:
                        issue_amap(half + 2)

                tc.no_sync_barrier()

                # flattened chunk pipeline: c1(k+1) issued before c2(k) so
                # the PE never queues behind a gelu wait; pA/pB rings of 3.
                NCHK = NQI2  # 8 chunks of 512 per half
                halves = list(range(sblk * HPB, (sblk + 1) * HPB))
                nchunks = len(halves) * NCHK
                c2sbs = {}
                pAs, pBs = {}, {}

                def conv_c1(k):
                    hi, chk = divmod(k, NCHK)
                    logm = logms[hi]
                    pA = p2ps.tile([P, S], fp32, tag="pA", bufs=2, name=f"pA{k % 2}")
                    pB = p2ps.tile([P, S], fp32, tag="pB", bufs=2, name=f"pB{k % 2}")
                    nc.tensor.matmul(pA, c1A, logm[:, ts(chk, S)],
                                     start=True, stop=True)
                    nc.tensor.matmul(pB, c1B, logm[:, ts(chk, S)],
                                     start=True, stop=True)
                    pAs[k], pBs[k] = pA, pB

                def conv_c2(k):
                    hi, chk = divmod(k, NCHK)
                    half = halves[hi]
                    if chk == 0:
                        c2sbs[hi] = p2sb.tile([P, NQI2, S], fp8e4, tag="c2sb", name=f"c2sb{hi % 2}")
                    gA = p2sb.tile([P, S], bf16, tag="gA", bufs=2, name=f"gA{k % 2}")
                    gB = p2sb.tile([P, S], bf16, tag="gB", bufs=2, name=f"gB{k % 2}")
                    nc.scalar.activation(gA, pAs.pop(k), AF.Gelu,
                                         bias=bc1A, scale=1.0)
                    nc.scalar.activation(gB, pBs.pop(k), AF.Gelu,
                                         bias=bc1B, scale=1.0)
                    pC = p2ps.tile([P, S], fp32, tag="pC", bufs=2, name=f"pC{k % 2}")
                    nc.tensor.matmul(pC, c2A, gA, start=True, stop=False)
                    nc.tensor.matmul(pC, c2B, gB, start=False, stop=True)
                    nc.vector.tensor_scalar(
                        out=c2sbs[hi][:, chk, :], in0=pC,
                        scalar1=bc2c[:, 0:1], scalar2=None, op0=ALU.add,
                    )
                    if chk == NCHK - 1:
                        qbase = half * 64
                        c2sb = c2sbs.pop(hi)
                        for g in range(8):
                            q0 = qbase + NQI2 * g
                            nc.sync.dma_start(
                                out=bstage[q0 : q0 + NQI2].rearrange(
                                    "q h k -> h q k"
                                ),
                                in_=c2sb[CH * g : CH * (g + 1)],
                            )
                        if half % 2 == 1:
                            qt = half // 2
                            nc.sync.dma_start(
                                out=biasq[:, qt],
                                in_=bstage[qt * P : (qt + 1) * P],
                            )

                conv_c1(0)
                for k in range(1, nchunks):
                    conv_c1(k)
                    conv_c2(k - 1)
                conv_c2(nchunks - 1)

                if sblk == 0:
                    tc.no_sync_barrier()

        if debug:
            dbgq = const.tile([P, ND, S], fp32, name="dbgq")
            nc.vector.tensor_copy(dbgq.rearrange("p a b -> p (a b)"),
                                  QtT.rearrange("p a b -> p (a b)"))
            nc.sync.dma_start(out=dbg_qt_e[:, :, :], in_=dbgq)
            nc.vector.tensor_copy(dbgq.rearrange("p a b -> p (a b)"),
                                  KtT.rearrange("p a b -> p (a b)"))
            nc.sync.dma_start(out=dbg_kt_e[:, :, :], in_=dbgq)
            dbgv = const.tile([P, NQT, D], fp32, name="dbgv")
            nc.vector.tensor_copy(dbgv.rearrange("p a b -> p (a b)"),
                                  Vsb.rearrange("p a b -> p (a b)"))
            nc.sync.dma_start(out=dbg_v_e[:, :, :], in_=dbgv)
            dbgb = const.tile([P, H, S], fp32, name="dbgb")
            nc.vector.tensor_copy(dbgb.rearrange("p a b -> p (a b)"),
                                  biasq[:, 0].rearrange("p a b -> p (a b)"))
            nc.sync.dma_start(out=dbg_bias_e[:, :, :], in_=dbgb)

        # =========== Phase 3: attention (one table: Exp) ===========
        tc.no_sync_barrier()
        with (
            tc.tile_pool(name="p3sb", bufs=1) as p3sb,
            tc.tile_pool(name="p3ps", bufs=1, space="PSUM") as p3ps,
        ):
            sc_pool = [p3ps.tile([P, S], fp32, tag=f"sc{i}", name=f"sc{i}")
                       for i in range(4)]
            atu_pool = [p3ps.tile([P, NQT, P], bf16, tag=f"atu{i}", name=f"atu{i}")
                        for i in range(2)]
            cx_pool = [p3ps.tile([P, S], fp32, tag=f"cx{i}", name=f"cx{i}")
                       for i in range(2)]
            att_pool = [p3sb.tile([P, S], bf16, tag=f"att{i}", name=f"att{i}")
                        for i in range(4)]
            attn_pool = [p3sb.tile([P, S], bf16, tag=f"attn{i}", name=f"attn{i}")
                         for i in range(4)]
            den_pool = [p3sb.tile([P, 1], fp32, tag=f"den{i}", name=f"den{i}")
                        for i in range(4)]
            rec_pool = [p3sb.tile([P, 1], fp32, tag=f"rec{i}", name=f"rec{i}")
                        for i in range(4)]
            ath_pool = [p3sb.tile([P, NQT, S], bf16, tag=f"ath{i}", name=f"ath{i}")
                        for i in range(2)]

            def issue_scores(h):
                hb, ho = (h * DH) // P, (h * DH) % P
                for qt in range(NQT):
                    slot = (h * NQT + qt) % 4
                    sc = sc_pool[slot]
                    # psum <- bias, then += Qt^T K (Qt pre-scaled by 1/8)
                    nc.tensor.matmul(
                        sc, ident8, biasq[:, qt, h, :],
                        start=True, stop=False,
                    )
                    nc.tensor.matmul(
                        sc,
                        QtT[ho : ho + DH, hb, ts(qt, P)],
                        KtT[ho : ho + DH, hb, :],
                        start=False, stop=True,
                    )
                    nc.scalar.activation(
                        att_pool[slot], sc, AF.Exp, accum_out=den_pool[slot]
                    )
                    nc.vector.reciprocal(out=rec_pool[slot], in_=den_pool[slot])
                    nc.vector.tensor_scalar_mul(
                        attn_pool[slot], att_pool[slot], rec_pool[slot][:, 0:1]
                    )

            def issue_transp(h):
                ath = ath_pool[h % 2]
                for qt in range(NQT):
                    slot = (h * NQT + qt) % 4
                    atu = atu_pool[qt % 2]
                    for kt in range(NQT):
                        nc.tensor.transpose(
                            atu[:, kt, :], attn_pool[slot][:, ts(kt, P)], ident_b
                        )
                    nc.vector.tensor_copy(ath[:, :, ts(qt, P)], atu)

            def issue_ctx(h):
                ath = ath_pool[h % 2]
                cx = cx_pool[(h // 2) % 2]
                prange = cx[(h % 2) * DH : (h % 2) * DH + DH, :]
                for kt in range(NQT):
                    nc.tensor.matmul(
                        prange,
                        Vsb[:, kt, h * DH : (h + 1) * DH],
                        ath[:, kt, :],
                        start=(kt == 0), stop=(kt == NQT - 1),
                    )
                if h % 2 == 1:
                    nc.vector.tensor_copy(ctxT[:, h // 2, :], cx)

            # software pipeline: transp(h-1) | scores(h) | ctx(h-1)
            issue_scores(0)
            for h in range(1, H):
                issue_transp(h - 1)
                issue_scores(h)
                issue_ctx(h - 1)
            issue_transp(H - 1)
            issue_ctx(H - 1)

        if debug:
            dbgc = const.tile([P, ND, S], fp32, name="dbgc")
            nc.vector.tensor_copy(dbgc.rearrange("p a b -> p (a b)"),
                                  ctxT.rearrange("p a b -> p (a b)"))
            nc.sync.dma_start(out=dbg_ctx_e[:, :, :], in_=dbgc)

        # free attention residents before FFN
        attp_cm.__exit__(None, None, None)

        ffp_cm = tc.tile_pool(name="ffp", bufs=1)
        ffp = ffp_cm.__enter__()
        xln = ffp.tile([P, NQT, D], fp32)
        xlnb = ffp.tile([P, NQT, D], bf16)
        xlnT = ffp.tile([P, ND, S], bf16)
        y1T = ffp.tile([P, NFF, S], bf16)
        # LN param broadcast rows -> [P, D]
        g1b = ffp.tile([P, D], fp32)
        b1b = ffp.tile([P, D], fp32)
        g2b = ffp.tile([P, D], fp32)
        b2b = ffp.tile([P, D], fp32)
        for dst, src_e in ((g1b, g1r_e), (b1b, b1r_e), (g2b, g2r_e), (b2b, b2r_e)):
            row = ffp.tile([1, D], fp32, tag="lnrow", name="lnrow")
            nc.sync.dma_start(out=row, in_=src_e[:, :])
            nc.gpsimd.partition_broadcast(dst, row[0:1, :])

        # =========== Phase 4: merge + residual + LN1 (+ transpose) ===========
        tc.no_sync_barrier()
        with (
            tc.tile_pool(name="p4sb", bufs=2) as p4sb,
            tc.tile_pool(name="p4ps", bufs=2, space="PSUM") as p4ps,
            tc.tile_pool(name="p4tp", bufs=2, space="PSUM") as p4tp,
        ):
            for st in range(NQT):
                qtile = qbm[:, st, :]
                x1 = p4sb.tile([P, D], fp32, tag="x1")
                for oh in range(2):
                    ps = p4ps.tile([P, S], fp32, tag="mps")
                    for dblk in range(ND):
                        nc.tensor.matmul(
                            ps,
                            ctxT[:, dblk, ts(st, P)],
                            wmres[:, dblk, ts(oh, S)],
                            start=(dblk == 0),
                            stop=(dblk == ND - 1),
                        )
                    nc.vector.tensor_tensor(
                        out=x1[:, ts(oh, S)], in0=ps,
                        in1=qtile[:, ts(oh, S)], op=ALU.add,
                    )
                _layernorm(nc, p4sb, xln[:, st, :], x1, g1b, b1b, eps_ln_c)
                nc.scalar.activation(xlnb[:, st, :], xln[:, st, :], AF.Copy)
                for dblk in range(ND):
                    tp = p4tp.tile([P, P], bf16, tag="tp")
                    nc.tensor.transpose(
                        tp, xlnb[:, st, ts(dblk, P)], ident_b
                    )
                    nc.vector.tensor_copy(xlnT[:, dblk, ts(st, P)], tp)

        if debug:
            nc.sync.dma_start(out=dbg_xln_e[:, :, :], in_=xln)

        # =========== Phase 5: FFN1 + relu ===========
        with (
            tc.tile_pool(name="p5w", bufs=2) as p5w,
            tc.tile_pool(name="p5ps", bufs=2, space="PSUM") as p5ps,
        ):
            NGRP = 4
            FPG = NFF // NGRP  # 8 ff-blocks per group
            for grp in range(NGRP):
                wf1g = p5w.tile([P, ND, FPG * P], bf16, tag="wf1g")
                nc.sync.dma_start(
                    out=wf1g,
                    in_=wf1Tb_e[
                        :, grp * FPG * P : (grp + 1) * FPG * P
                    ].rearrange("(n p) f -> p n f", p=P),
                )
                for sh in range(2):
                    for fl in range(FPG):
                        ffb = grp * FPG + fl
                        ps = p5ps.tile([P, S // 2], fp32, tag="fps")
                        for dblk in range(ND):
                            nc.tensor.matmul(
                                ps,
                                wf1g[:, dblk, ts(fl, P)],
                                xlnT[:, dblk, ts(sh, S // 2)],
                                start=(dblk == 0), stop=(dblk == ND - 1),
                            )
                        nc.scalar.activation(
                            y1T[:, ffb, ts(sh, S // 2)], ps, AF.Relu,
                            bias=bf1c[:, ffb : ffb + 1], scale=1.0,
                        )

        # =========== Phase 6: FFN2 + residual + LN2 + out ===========
        with (
            tc.tile_pool(name="p7sb", bufs=2) as p7sb,
            tc.tile_pool(name="p7w", bufs=2) as p7w,
            tc.tile_pool(name="p7ps", bufs=1, space="PSUM") as p7ps,
        ):
            fps2 = [
                [p7ps.tile([P, S], fp32, tag=f"f2{st * 2 + oh}", name=f"f2{st}{oh}")
                 for oh in range(2)]
                for st in range(NQT)
            ]
            for st in range(NQT):
                for oh in range(2):
                    nc.tensor.matmul(
                        fps2[st][oh], onesb[:, 0:P], bf2r[:, ts(oh, S)],
                        start=True, stop=False,
                    )
            for fpair in range(NFF // 2):
                wch = p7w.tile([P, 2, D], bf16, tag="wch")
                nc.sync.dma_start(
                    out=wch,
                    in_=wf2Tb_e[fpair * 2 * P : (fpair + 1) * 2 * P, :].rearrange(
                        "(a p) d -> p a d", p=P
                    ),
                )
                for a in range(2):
                    ffb = fpair * 2 + a
                    for st in range(NQT):
                        for oh in range(2):
                            nc.tensor.matmul(
                                fps2[st][oh],
                                y1T[:, ffb, ts(st, P)],
                                wch[:, a, ts(oh, S)],
                                start=False,
                                stop=(ffb == NFF - 1),
                            )
            for st in range(NQT):
                x2 = p7sb.tile([P, D], fp32, tag="x2")
                for oh in range(2):
                    nc.vector.tensor_tensor(
                        out=x2[:, ts(oh, S)], in0=fps2[st][oh],
                        in1=xln[:, st, ts(oh, S)], op=ALU.add,
                    )
                xout = p7sb.tile([P, D], fp32, tag="xout")
                _layernorm(nc, p7sb, xout, x2, g2b, b2b, eps_ln_c)
                nc.sync.dma_start(out=out_e[st * P : (st + 1) * P, :], in_=xout)

        ffp_cm.__exit__(None, None, None)
        dram_cm.__exit__(None, None, None)
        midp_cm.__exit__(None, None, None)
        const_cm.__exit__(None, None, None)

    nc.finalize()
    return nc


def _prep_inputs(q, kv, attn_map, Wq, bq, Wk, bk, Wv, bv, Wm, bm,
                 Wc1, bc1, Wc2, bc2, Wf1, bf1, Wf2, bf2, g1, b1, g2, b2):
    """Host-side packing. Returns (shared dict, per-core list of dicts)."""
    f32 = np.float32
    bf = ml_dtypes.bfloat16

    def c(a):
        return np.ascontiguousarray(np.asarray(a), dtype=f32)

    def cb(a):
        return np.ascontiguousarray(np.asarray(a, dtype=f32)).astype(bf)

    Wq, Wk, Wv, Wm = c(Wq), c(Wk), c(Wv), c(Wm)
    Wc1, Wc2 = c(Wc1), c(Wc2)
    bq, bk, bv, bm = c(bq), c(bk), c(bv), c(bm)
    bc1, bc2, bf1, bf2 = c(bc1), c(bc2), c(bf1), c(bf2)
    g1, b1, g2, b2 = c(g1), c(b1), c(g2), c(b2)

    shared = {
        "wqTb": cb(Wq.T), "wkTb": cb(Wk.T), "wvTb": cb(Wv.T), "wmTb": cb(Wm.T),
        "wf1Tb": cb(np.asarray(Wf1).T),
        "wf2Tb": cb(np.asarray(Wf2).T),
        "bqc": c((bq / 8.0).reshape(ND, P).T),
        "bkc": c(bk.reshape(ND, P).T),
        "bf1c": c(bf1.reshape(NFF, P).T),
        "bvr": cb(bv.reshape(1, D)),
        "bf2r": cb(bf2.reshape(1, D)),
        "onesb": np.ones((1, S), bf),
        "ident8": np.eye(P, dtype=ml_dtypes.float8_e4m3),
        "g1r": g1.reshape(1, D), "b1r": b1.reshape(1, D),
        "g2r": g2.reshape(1, D), "b2r": b2.reshape(1, D),
    }
    # conv block-diag lhsT [K, M]: out[(g,oh)] = sum_c lhsT[(g,c),(g,oh)] rhs[(g,c)]
    c1A = np.zeros((P, P), f32)
    c1B = np.zeros((P, P), f32)
    c2A = np.zeros((P, P), f32)
    c2B = np.zeros((P, P), f32)
    for g in range(8):
        sl = slice(g * 16, g * 16 + 16)
        c1A[sl, sl] = Wc1[0:16, :].T     # [c, oh]
        c1B[sl, sl] = Wc1[16:32, :].T
        c2A[sl, sl] = Wc2[:, 0:16].T     # [ci, h]
        c2B[sl, sl] = Wc2[:, 16:32].T
    shared["c1A"] = c1A.astype(bf)
    shared["c1B"] = c1B.astype(bf)
    shared["c2A"] = c2A.astype(bf)
    shared["c2B"] = c2B.astype(bf)
    shared["bc1A"] = np.tile(bc1[0:16], 8).reshape(P, 1).astype(f32)
    shared["bc1B"] = np.tile(bc1[16:32], 8).reshape(P, 1).astype(f32)
    shared["bc2c"] = np.tile(bc2, 8).reshape(P, 1).astype(f32)

    q = c(q)
    kv = c(kv)
    per_core = []
    for b in range(B):
        per_core.append({
            "qTb": cb(q[b].T), "kvTb": cb(kv[b].T),
            "qbm": cb(q[b] + bm.reshape(1, D)),
            "amapb": np.asarray(attn_map[b], dtype=np.float32).astype(ml_dtypes.float8_e4m3),
        })
    return shared, per_core


def kernel(**inputs):
    if "nc" not in _CACHED:
        _CACHED["nc"] = build_program()
    nc = _CACHED["nc"]
    shared, per_core = _prep_inputs(**inputs)
    in_maps = [dict(shared, **pc) for pc in per_core]
    res = run_bass_kernel_spmd(nc, in_maps, list(range(B)))
    out = np.stack([res.results[i]["out"] for i in range(B)], axis=0)
    return out.astype(np.float32)
